# revision 40
# baseline (speedup 1.0000x reference)
"""GAT (2-layer, PyG-style) Trainium2 kernel, 8-core SPMD.

Strategy:
  - Nodes assigned to (core, 128-chunk) slots, load-balanced by in-degree;
    L2 chunks are co-located with the L1 column chunks (same membership), so
    layer-2 dst attention values stay core-local.
  - Aggregation in x-space (256-wide) with TRANSPOSED layout: the gathered
    source features are the matmul STATIONARY operand; the moving operand is
    a per-edge alpha-scaled one-hot mask block M8[e, (head, dstpos)] built in
    ONE fused DVE op (scalar_tensor_tensor: (iota==drel)*alpha) per edge
    tile. Output lands feature-major, which is exactly the layout the W1/W2
    projections need, so no transpose roundtrip.
  - Attention: a_src rides the feature gather (f32 cols in the same 768B
    table row); a_dst is expanded per-edge with tiny matmuls against
    host-uploaded static one-hot masks (m = [e,dst], mT = [dst,e]; bf16
    hi/lo splits keep the lookups near-exact); softmax denominators via
    m-matmuls; masks are pre-normalized by 1/den (alpha), so no
    post-scaling of the aggregate.
  - Software pipelining: chunk c's attention chain (DVE/ScalarE/small PE
    lookups) runs while chunk c-1's aggregation matmuls keep the PE busy;
    the layer-1->layer-2 projections (phases 5/6) run per 512-node group
    inside the same loop with small rotating buffers.
  - Cross-core: two AllGathers of the 768B-row node tables (G1, G2).
"""
import os, sys
import numpy as np
import ml_dtypes

sys.path.insert(0, "/opt/trn_rl_repo")
import concourse.bass as bass
import concourse.mybir as mybir
import concourse.tile as tile
import concourse.bacc as bacc
from concourse import bass_utils

F32 = mybir.dt.float32
BF16 = mybir.dt.bfloat16
I16 = mybir.dt.int16
BF = ml_dtypes.bfloat16

# ---------------- problem constants ----------------
NC_NODES = 4000
NCOL = 16000
N = NC_NODES + NCOL
NF, CF = 64, 128
HID = 256
H = 8
EMB = 128
NEG = 0.2

NCORES = 8
CON_CH = 4
COL_CH = 16
CPC1 = CON_CH + COL_CH          # 20
SLOT1 = CPC1 * 128              # 2560
CPC2 = 16
SLOT2 = CPC2 * 128              # 2048
GW = 384                        # bf16 table width (768B stride); f32 view 192
                                # (dma_gather elem size must be a multiple of
                                #  256B: 384*2 = 768B)

_prog_cache = {}
LAST_EXEC_NS = None
LAST_RESULTS = None


# ================= host-side preprocessing =================

def _balance(nodes, deg, n_chunks, cap=128):
    import heapq
    order = nodes[np.argsort(-deg[nodes], kind="stable")]
    loads = np.zeros(n_chunks, dtype=np.int64)
    counts = np.zeros(n_chunks, dtype=np.int64)
    heap = [(0, c) for c in range(n_chunks)]
    heapq.heapify(heap)
    members = [[] for _ in range(n_chunks)]
    for nd in order:
        while True:
            _, c = heapq.heappop(heap)
            if counts[c] < cap:
                break
        members[c].append(int(nd))
        counts[c] += 1
        loads[c] += int(deg[nd])
        if counts[c] < cap:
            heapq.heappush(heap, (loads[c], c))
    return members, loads


def _wrap_idx(idx):
    """dma_gather int16 index layout: [128, n/16]; row p holds idx[s*16+p%16]."""
    idx = np.asarray(idx, dtype=np.int16)
    n = len(idx)
    assert n % 16 == 0
    m = idx.reshape(n // 16, 16).T
    return np.tile(m, (8, 1)).copy()


def _onehots(drel, cpc, T):
    """drel: [cpc, T*128] float (dst position in chunk, or -1 pad).
    Returns m  [128(e), cpc*T, 128(p)]  and mT [128(p), cpc*T, 128(e)]  bf16."""
    d = drel.reshape(cpc, T, 128).astype(np.int32)       # [c, j, e]
    oh = (d[:, :, :, None] == np.arange(128)[None, None, None, :])  # [c,j,e,p]
    m = np.ascontiguousarray(
        oh.transpose(2, 0, 1, 3).reshape(128, cpc * T, 128)
    ).astype(BF)
    mT = np.ascontiguousarray(
        oh.transpose(3, 0, 1, 2).reshape(128, cpc * T, 128)
    ).astype(BF)
    return m, mT


def _prep(edges):
    src1 = np.concatenate([edges[0], np.arange(N)]).astype(np.int64)
    dst1 = np.concatenate([edges[1], np.arange(N)]).astype(np.int64)
    s2 = np.concatenate([edges[1], np.arange(N)]).astype(np.int64)
    d2 = np.concatenate([edges[0], np.arange(N)]).astype(np.int64)
    keep = d2 >= NC_NODES
    src2, dst2 = s2[keep], d2[keep]

    deg1 = np.bincount(dst1, minlength=N)
    deg2 = np.bincount(dst2, minlength=N)

    con_members, con_loads = _balance(np.arange(NC_NODES), deg1, NCORES * CON_CH)
    # column chunks serve BOTH layers (L2 chunks == L1 col chunks); balance on
    # deg1 (the larger layer) and accept the resulting T2
    col_members, _ = _balance(np.arange(NC_NODES, N), deg1, NCORES * COL_CH)
    gslot1 = np.full(N, -1, dtype=np.int64)
    chunks1 = [[] for _ in range(NCORES * CPC1)]
    for g, mem in enumerate(con_members):
        core, lc = g % NCORES, g // NCORES
        chunks1[core * CPC1 + lc] = mem
    for g, mem in enumerate(col_members):
        core, lc = g % NCORES, CON_CH + g // NCORES
        chunks1[core * CPC1 + lc] = mem
    for ci, mem in enumerate(chunks1):
        core, lc = divmod(ci, CPC1)
        for pos, nd in enumerate(mem):
            gslot1[nd] = core * SLOT1 + lc * 128 + pos
    assert (gslot1 >= 0).all()

    # chunks2 = the column chunks of layer 1 (identity co-location)
    chunks2 = [
        chunks1[core * CPC1 + CON_CH + lc]
        for core in range(NCORES) for lc in range(CPC2)
    ]
    # realized per-chunk loads determine the tile counts
    def chunk_load(members_list, deg):
        return max(
            (sum(deg[nd] for nd in mem) for mem in members_list if mem),
            default=0,
        )
    T1 = max(4, int(np.ceil(max(
        chunk_load([chunks1[i] for i in range(len(chunks1))], deg1), 1
    ) / 128)))
    T2 = max(4, int(np.ceil(max(chunk_load(chunks2, deg2), 1) / 128)))

    # table row layout is half-major (for split AllGathers):
    # row = half*(NCORES*HR) + core*HR + loc%HR,  HR = SLOT1//2
    HR = SLOT1 // 2
    def row_of(g):
        core, loc = g // SLOT1, g % SLOT1
        return (loc // HR) * (NCORES * HR) + core * HR + loc % HR

    # dst slot mapping for L2: position within the L1 col-chunk
    def edge_tables(src, dst, cpc, T, chunk_of_node, pos_of_node, remap):
        """Per core: src gather idx, drel, and static one-hot masks."""
        dcore = gslot1[dst] // SLOT1
        order = np.argsort(
            dcore * (cpc * 128) + chunk_of_node[dst] * 128 + pos_of_node[dst],
            kind="stable",
        )
        so, do = src[order], dst[order]
        core_of = dcore[order]
        cm_all, pm_all = chunk_of_node[do], pos_of_node[do]
        res = []
        for mcore in range(NCORES):
            esrc = np.zeros((cpc, T * 128), dtype=np.int64)
            drel = np.full((cpc, T * 128), -1.0, dtype=np.float32)
            sel = core_of == mcore
            sm, cm, pm = so[sel], cm_all[sel], pm_all[sel]
            for lc in range(cpc):
                s = cm == lc
                k = int(s.sum())
                assert k <= T * 128, f"chunk overflow {k} > {T*128}"
                esrc[lc, :k] = remap(gslot1[sm[s]])
                drel[lc, :k] = pm[s]
            idx = _wrap_idx(esrc.reshape(-1))
            m, mT = _onehots(drel, cpc, T)
            drel_dev = np.ascontiguousarray(
                drel.reshape(cpc, T, 128).transpose(2, 0, 1).reshape(128, cpc * T)
            )
            res.append((idx, drel_dev, m, mT))
        return res

    chunk1_of = (gslot1 % SLOT1) // 128          # L1 chunk index per node
    pos_of = gslot1 % 128
    chunk2_of = chunk1_of - CON_CH               # L2 chunk index (col nodes)
    et1 = edge_tables(src1, dst1, CPC1, T1, chunk1_of, pos_of, lambda g: g)
    et2 = edge_tables(src2, dst2, CPC2, T2, chunk2_of, pos_of, lambda g: g)
    return dict(gslot1=gslot1, chunks1=chunks1, chunks2=chunks2,
                T1=T1, T2=T2, et1=et1, et2=et2)


def _weights_prep(inp):
    W1 = inp["W1"].astype(np.float32)       # [2048, 256]
    W2 = inp["W2"].astype(np.float32)       # [256, 2048]
    out = {}
    out["wnodet"] = np.ascontiguousarray(inp["W_node"].T).astype(np.float32)  # [128,256]
    wct = inp["W_col"].T.astype(np.float32)  # [256, 256]
    out["wcolt"] = np.stack([wct[0:128], wct[128:256]], axis=1)  # [128, 2, 256]
    V1 = np.zeros((256, 16), np.float32)
    for h in range(H):
        Wh = W1[h * HID:(h + 1) * HID, :]
        V1[:, h] = Wh.T @ inp["att_src1"][h]
        V1[:, 8 + h] = Wh.T @ inp["att_dst1"][h]
    out["v1"] = np.stack([V1[0:128], V1[128:256]], axis=1)       # [128, 2, 16]
    W1T = W1.T                                                   # [256, 2048]
    w1tb = np.zeros((128, 32, 128), BF)
    for h in range(H):
        for os_ in range(2):
            for fs in range(2):
                w1tb[:, h * 4 + os_ * 2 + fs, :] = W1T[
                    fs * 128:(fs + 1) * 128,
                    h * 256 + os_ * 128: h * 256 + (os_ + 1) * 128,
                ].astype(BF)
    out["w1tb"] = w1tb
    W2T = W2.T                                                   # [2048, 256]
    w2tb = np.zeros((128, 32, 128), BF)
    for f16 in range(16):
        for os_ in range(2):
            w2tb[:, f16 * 2 + os_, :] = W2T[
                f16 * 128:(f16 + 1) * 128, os_ * 128:(os_ + 1) * 128
            ].astype(BF)
    out["w2tb"] = w2tb
    a2 = np.stack([inp["att_src2"][0], inp["att_dst2"][0]], axis=1)  # [256, 2]
    out["att2"] = np.stack([a2[0:128], a2[128:256]], axis=1).astype(BF)  # [128,2,2]
    wot = inp["W_out"].T.astype(np.float32)  # [256, 128]
    out["woutt"] = np.stack([wot[0:128], wot[128:256]], axis=1).astype(BF)  # [128,2,128]
    # iota8i[e, p*8+h] = p  (interleaved one-hot comparison pattern, L1)
    out["iota8i"] = np.broadcast_to(
        (np.arange(1024) // 8).astype(BF), (128, 1024)
    ).copy()
    # iota128[e, p] = p (L2)
    out["iota128"] = np.broadcast_to(
        np.arange(128).astype(BF), (128, 128)
    ).copy()
    return out


# ================= device program =================

def _build_program(T1, T2):
    nc = bacc.Bacc(None, target_bir_lowering=False)
    NT1, NT2 = CPC1 * T1, CPC2 * T2

    xct = nc.dram_tensor("xct", [128, CON_CH * 128], F32, kind="ExternalInput")
    xcolt = nc.dram_tensor("xcolt", [128, 2, COL_CH * 128], F32, kind="ExternalInput")
    wnodet = nc.dram_tensor("wnodet", [128, 256], F32, kind="ExternalInput")
    wcolt = nc.dram_tensor("wcolt", [128, 2, 256], F32, kind="ExternalInput")
    v1 = nc.dram_tensor("v1", [128, 2, 16], F32, kind="ExternalInput")
    w1tb = nc.dram_tensor("w1tb", [128, 32, 128], BF16, kind="ExternalInput")
    w2tb = nc.dram_tensor("w2tb", [128, 32, 128], BF16, kind="ExternalInput")
    att2 = nc.dram_tensor("att2", [128, 2, 2], BF16, kind="ExternalInput")
    woutt = nc.dram_tensor("woutt", [128, 2, 128], BF16, kind="ExternalInput")
    iota8i = nc.dram_tensor("iota8i", [128, 1024], BF16, kind="ExternalInput")
    iota128 = nc.dram_tensor("iota128", [128, 128], BF16, kind="ExternalInput")
    esrc1 = nc.dram_tensor("esrc1", [128, NT1 * 8], I16, kind="ExternalInput")
    drel1 = nc.dram_tensor("drel1", [128, NT1], F32, kind="ExternalInput")
    m1d = nc.dram_tensor("m1", [128, NT1, 128], BF16, kind="ExternalInput")
    mT1d = nc.dram_tensor("mT1", [128, NT1, 128], BF16, kind="ExternalInput")
    esrc2 = nc.dram_tensor("esrc2", [128, NT2 * 8], I16, kind="ExternalInput")
    drel2 = nc.dram_tensor("drel2", [128, NT2], F32, kind="ExternalInput")
    m2d = nc.dram_tensor("m2", [128, NT2, 128], BF16, kind="ExternalInput")
    mT2d = nc.dram_tensor("mT2", [128, NT2, 128], BF16, kind="ExternalInput")
    out_dram = nc.dram_tensor("out", [SLOT2, EMB], F32, kind="ExternalOutput")

    Copy = mybir.ActivationFunctionType.Copy
    Relu = mybir.ActivationFunctionType.Relu
    Exp = mybir.ActivationFunctionType.Exp
    ADD, EQ, MUL, MAX, SUB = (
        mybir.AluOpType.add, mybir.AluOpType.is_equal,
        mybir.AluOpType.mult, mybir.AluOpType.max,
        mybir.AluOpType.subtract,
    )

    with tile.TileContext(nc) as tc:
        with (
            tc.tile_pool(name="const", bufs=1) as cpool,
            tc.tile_pool(name="sb", bufs=3) as sb,
            tc.tile_pool(name="dram", bufs=1, space="DRAM") as dram,
        ):
            def cload(t, shape, dtype):
                nm = t.name + "_sb"
                s = cpool.tile(shape, dtype, name=nm, tag=nm)
                nc.sync.dma_start(s[:], t[:])
                return s

            iota8i_sb = cload(iota8i, [128, 1024], BF16)
            iota128_sb = cload(iota128, [128, 128], BF16)
            wnodet_sb = cload(wnodet, [128, 256], F32)
            wcolt_sb = cload(wcolt, [128, 2, 256], F32)
            v1_sb = cload(v1, [128, 2, 16], F32)
            w1tb_sb = cload(w1tb, [128, 32, 128], BF16)
            w2tb_sb = cload(w2tb, [128, 32, 128], BF16)
            att2_sb = cload(att2, [128, 2, 2], BF16)
            woutt_sb = cload(woutt, [128, 2, 128], BF16)
            idx1_sb = cload(esrc1, [128, NT1 * 8], I16)
            drel1_sb = cload(drel1, [128, NT1], F32)
            idx2_sb = cload(esrc2, [128, NT2 * 8], I16)
            drel2_sb = cload(drel2, [128, NT2], F32)

            g1_loc = dram.tile([SLOT1, GW], BF16)
            g1_full = dram.tile([NCORES * SLOT1, GW], BF16, addr_space="Shared")
            g2_loc = dram.tile([SLOT1, GW], BF16)
            g2_full = dram.tile([NCORES * SLOT1, GW], BF16, addr_space="Shared")

            ad2f = cpool.tile([128, CPC2, 1], F32, name="ad2f", tag="ad2f")
            ad2hla = cpool.tile([128, CPC2, 2], BF16, name="ad2hla", tag="ad2hla")

            # long-lived L1 pool (adhl written in phase 1, read through L1)
            l1 = tc.alloc_tile_pool(name="l1", bufs=1)
            aggnT4 = l1.tile([128, 2, 4, 8, 128], BF16, tag="aggnT4")
            adhl = l1.tile([128, CPC1, 16], BF16, tag="adhl")

            # ======== phase 1: input MLPs ========
            p1 = tc.alloc_tile_pool(name="p1", bufs=1)
            psA = tc.alloc_tile_pool(name="psA", bufs=2, space="PSUM")
            xct_sb = p1.tile([128, CON_CH * 128], F32, tag="xct_sb")
            nc.sync.dma_start(xct_sb[:], xct[:])
            xcolt_sb = p1.tile([128, 2, COL_CH * 128], F32, tag="xcolt_sb")
            nc.sync.dma_start(xcolt_sb[:], xcolt[:])
            xT = p1.tile([128, 2, SLOT1], F32, tag="xT")
            for os_ in range(2):
                p = psA.tile([128, CON_CH * 128], F32, tag="pmlp")
                nc.tensor.matmul(
                    p[:], wnodet_sb[:, os_ * 128:(os_ + 1) * 128], xct_sb[:],
                    start=True, stop=True,
                )
                nc.scalar.activation(xT[:, os_, 0:CON_CH * 128], p[:], Relu)
                for nch in range(4):
                    p2 = psA.tile([128, 512], F32, tag="pmlp2")
                    for fs in range(2):
                        nc.tensor.matmul(
                            p2[:],
                            wcolt_sb[:, fs, os_ * 128:(os_ + 1) * 128],
                            xcolt_sb[:, fs, nch * 512:(nch + 1) * 512],
                            start=(fs == 0), stop=(fs == 1),
                        )
                    nc.scalar.activation(
                        xT[:, os_, CON_CH * 128 + nch * 512: CON_CH * 128 + (nch + 1) * 512],
                        p2[:], Relu,
                    )

            # node-major x + attention values -> G1 rows; keep a_d locally.
            # Emitted as two dense passes (all px matmuls, then all pa
            # matmuls) so the PE stream has no per-chunk stalls.
            g1sb = p1.tile([128, CPC1, GW], BF16, tag="g1sb")
            g1sb_f32 = g1sb[:].bitcast(F32)       # [128, CPC1, 192]
            pa_all = p1.tile([128, CPC1, 16], F32, tag="pa_all")
            for c in range(CPC1):
                nsl = slice(c * 128, (c + 1) * 128)
                px = psA.tile([128, 256], F32, tag="px")
                if c < CON_CH:
                    nc.tensor.matmul(
                        px[:], xct_sb[:, nsl], wnodet_sb[:], start=True, stop=True
                    )
                else:
                    ksl = slice((c - CON_CH) * 128, (c - CON_CH) * 128 + 128)
                    for fs in range(2):
                        nc.tensor.matmul(
                            px[:], xcolt_sb[:, fs, ksl], wcolt_sb[:, fs, :],
                            start=(fs == 0), stop=(fs == 1),
                        )
                nc.scalar.activation(g1sb[:, c, 0:256], px[:], Relu)
            for c in range(CPC1):
                nsl = slice(c * 128, (c + 1) * 128)
                pa = psA.tile([128, 16], F32, tag="pa")
                for fs in range(2):
                    nc.tensor.matmul(
                        pa[:], xT[:, fs, nsl], v1_sb[:, fs, :],
                        start=(fs == 0), stop=(fs == 1),
                    )
                nc.vector.tensor_copy(g1sb_f32[:, c, 128:136], pa[:, 0:8])
                nc.vector.tensor_copy(pa_all[:, c, :], pa[:])

            # a_d hi/lo split for exact bf16-matmul lookups: [128, CPC1, 16]
            nc.vector.tensor_copy(adhl[:, :, 0:8], pa_all[:, :, 8:16])
            adhif = p1.tile([128, CPC1, 8], F32, tag="adhif")
            nc.vector.tensor_copy(adhif[:], adhl[:, :, 0:8])
            nc.vector.tensor_tensor(
                adhl[:, :, 8:16], pa_all[:, :, 8:16], adhif[:], SUB
            )
            nc.sync.dma_start(
                g1_loc[:].rearrange("(c p) w -> p c w", p=128), g1sb[:]
            )

            psA.release()

            # ======== phase 2: AllGather G1 ========
            nc.gpsimd.collective_compute(
                "AllGather", mybir.AluOpType.bypass,
                ins=[g1_loc.opt()], outs=[g1_full.opt()],
                replica_groups=[list(range(NCORES))],
            )
            p1.release()
            g1f = g1_full

            # ======== phase 3+4: layer-1 edge weights + aggregation ========
            l1m = tc.alloc_tile_pool(name="l1m", bufs=3)
            l1g = tc.alloc_tile_pool(name="l1g", bufs=2)
            l1w = tc.alloc_tile_pool(name="l1w", bufs=2)
            psS = tc.alloc_tile_pool(name="psS", bufs=2, space="PSUM")
            psT = tc.alloc_tile_pool(name="psT", bufs=1, space="PSUM")
            psP = tc.alloc_tile_pool(name="psP", bufs=2, space="PSUM")

            # software-pipelined over chunks: while chunk c's attention chain
            # runs on DVE/ScalarE, chunk c-1's aggregation matmuls keep the PE
            # busy (emitted into the chain's dependency gaps).

            def emit_agg_half(st, which):
                c, xg, M8a = st["c"], st["xg"], st["M8a"]
                pT = psT.tile([128, 1024], F32, tag=f"pT{which}")
                st[f"pT{which}"] = pT
                fsl = slice(which * 128, (which + 1) * 128)
                for j in range(T1):
                    M8f = M8a[:, j, :, :].rearrange("p a b -> p (a b)")
                    for half in range(2):
                        nc.tensor.matmul(
                            pT[:, half * 512:(half + 1) * 512],
                            xg[:, j, fsl], M8f[:, half * 512:(half + 1) * 512],
                            start=(j == 0), stop=(j == T1 - 1),
                        )

            def emit_drain_and_group(st):
                c = st["c"]
                nc.scalar.activation(
                    aggnT4[:, 0, c % 4, :, :].rearrange("p a b -> p (a b)"),
                    st["pT0"][:], Copy,
                )
                nc.scalar.activation(
                    aggnT4[:, 1, c % 4, :, :].rearrange("p a b -> p (a b)"),
                    st["pT1"][:], Copy,
                )
                if c % 4 != 3:
                    return
                # phases 5+6 for the completed 4-chunk group (512 nodes)
                g = c // 4
                x2Tg = l1g.tile([128, 16, 512], BF16, tag="x2Tg")
                for hh in range(16):
                    h, os_ = hh // 2, hh % 2
                    px2 = psP.tile([128, 512], F32, tag="pproj")
                    for fs in range(2):
                        rhs = aggnT4[:, fs, :, h, :]   # [128, 4, 128]
                        nc.tensor.matmul(
                            px2[:],
                            w1tb_sb[:, h * 4 + os_ * 2 + fs, :],
                            rhs,
                            start=(fs == 0), stop=(fs == 1),
                        )
                    nc.scalar.activation(x2Tg[:, hh, :], px2[:], Relu)
                h2Tg = l1g.tile([128, 2, 512], BF16, tag="h2Tg")
                for os_ in range(2):
                    ph2 = psP.tile([128, 512], F32, tag="pproj")
                    for f16 in range(16):
                        nc.tensor.matmul(
                            ph2[:], w2tb_sb[:, f16 * 2 + os_, :],
                            x2Tg[:, f16, :],
                            start=(f16 == 0), stop=(f16 == 15),
                        )
                    nc.scalar.activation(h2Tg[:, os_, :], ph2[:], Copy)
                for ci in range(4):
                    cg = g * 4 + ci
                    nsl = slice(ci * 128, (ci + 1) * 128)
                    pa2 = psP.tile([128, 2], F32, tag="pproj")
                    for fs in range(2):
                        nc.tensor.matmul(
                            pa2[:], h2Tg[:, fs, nsl], att2_sb[:, fs, :],
                            start=(fs == 0), stop=(fs == 1),
                        )
                    g2c = l1g.tile([128, 1, GW], BF16, tag="g2c")
                    g2c_f32 = g2c[:].bitcast(F32)
                    for fs in range(2):
                        nc.sync.dma_start(
                            g2c[:, 0, fs * 128:(fs + 1) * 128],
                            h2Tg[:, fs, nsl], transpose=True,
                        )
                    nc.vector.tensor_copy(g2c_f32[:, 0, 128:130], pa2[:])
                    if cg >= CON_CH:
                        nc.vector.tensor_copy(
                            ad2f[:, cg - CON_CH, :], pa2[:, 1:2]
                        )
                    nc.sync.dma_start(
                        g2_loc[:].rearrange("(c p) w -> p c w", p=128)[
                            :, cg:cg + 1, :
                        ],
                        g2c[:],
                    )

            prev = None
            for c in range(CPC1):
                tsl = slice(c * T1, (c + 1) * T1)
                xgt = l1m.tile([128, T1, GW], BF16, tag="xg1")
                nc.gpsimd.dma_gather(
                    xgt[:], g1f[:, 0:GW],
                    idx1_sb[:, c * T1 * 8:(c + 1) * T1 * 8],
                    T1 * 128, T1 * 128, GW, elem_step=GW,
                )
                xg = xgt[:]
                xg_f32 = xg.bitcast(F32)        # [128, T1, 192]
                mt_sb = l1m.tile([128, T1, 128], BF16, tag="mt1", name="mt1s")
                nc.sync.dma_start(mt_sb[:], mT1d[:, tsl, :])
                mm_sb = l1m.tile([128, T1, 128], BF16, tag="mm1", name="mm1s")
                nc.sync.dma_start(mm_sb[:], m1d[:, tsl, :])

                # one packed PSUM bank for the small per-chunk matmul outs
                psmall = psS.tile([128, 2 * T1 * 16 + 16], F32, tag="psmall")
                adps = psmall[:, 0:T1 * 16].rearrange("p (a b) -> p a b", b=16)
                den = psmall[:, T1 * 16:T1 * 16 + 8]
                rcps = psmall[:, T1 * 16 + 16:2 * T1 * 16 + 16].rearrange(
                    "p (a b) -> p a b", b=16
                )
                for j in range(T1):
                    nc.tensor.matmul(
                        adps[:, j, :], mt_sb[:, j, :], adhl[:, c, :],
                        start=True, stop=True,
                    )
                # PE filler while chunk c's chain runs on DVE/ScalarE
                if prev is not None:
                    emit_agg_half(prev, 0)
                ads = l1w.tile([128, T1, 16], F32, tag="ads")
                nc.scalar.activation(ads[:], adps, Copy)
                w1e = l1w.tile([128, T1, 8], F32, tag="w1e")
                # e = a_s + ad_hi + ad_lo
                nc.vector.tensor_tensor(
                    w1e[:], ads[:, :, 0:8], ads[:, :, 8:16], ADD
                )
                nc.vector.tensor_tensor(
                    w1e[:], w1e[:],
                    xg_f32[:, :, 128:136], ADD
                )
                nc.vector.scalar_tensor_tensor(
                    w1e[:], w1e[:], NEG, w1e[:], MUL, MAX
                )
                nc.scalar.activation(w1e[:], w1e[:], Exp)
                w1ebf = l1w.tile([128, T1, 8], BF16, tag="w1ebf")
                nc.scalar.activation(w1ebf[:], w1e[:], Copy)

                # denominators + reciprocal hi/lo
                for j in range(T1):
                    nc.tensor.matmul(
                        den[:], mm_sb[:, j, :], w1ebf[:, j, :],
                        start=(j == 0), stop=(j == T1 - 1),
                    )
                # PE filler while the reciprocal chain runs
                if prev is not None:
                    emit_agg_half(prev, 1)
                    emit_drain_and_group(prev)

                rec = l1w.tile([128, 8], F32, tag="rec")
                nc.vector.tensor_scalar_add(rec[:], den[:], 1e-16)
                nc.vector.reciprocal(rec[:], rec[:])
                rechl = l1w.tile([128, 16], BF16, tag="rechl")
                nc.vector.tensor_copy(rechl[:, 0:8], rec[:])
                rechf = l1w.tile([128, 8], F32, tag="rechf")
                nc.vector.tensor_copy(rechf[:], rechl[:, 0:8])
                nc.vector.tensor_tensor(rechl[:, 8:16], rec[:], rechf[:], SUB)

                # rec per edge + alpha
                for j in range(T1):
                    nc.tensor.matmul(
                        rcps[:, j, :], mt_sb[:, j, :], rechl[:],
                        start=True, stop=True,
                    )
                rcs = l1w.tile([128, T1, 16], F32, tag="rcs")
                nc.scalar.activation(rcs[:], rcps, Copy)
                alpha = l1w.tile([128, T1, 8], BF16, tag="alpha")
                rsum = l1w.tile([128, T1, 8], F32, tag="rsum")
                nc.vector.tensor_tensor(
                    rsum[:], rcs[:, :, 0:8], rcs[:, :, 8:16], ADD
                )
                nc.vector.tensor_tensor(alpha[:], w1e[:], rsum[:], MUL)

                # alpha-scaled interleaved masks for all tiles of this chunk
                M8a = l1g.tile([128, T1, 8, 128], BF16, tag="M8a")
                for j in range(T1):
                    t = c * T1 + j
                    nc.vector.scalar_tensor_tensor(
                        M8a[:, j, :, :],
                        iota128_sb[:].unsqueeze(1).broadcast_to([128, 8, 128]),
                        drel1_sb[:, t:t + 1],
                        alpha[:, j, :].unsqueeze(2).broadcast_to([128, 8, 128]),
                        EQ, MUL,
                    )
                prev = {"c": c, "xg": xg, "M8a": M8a}

            emit_agg_half(prev, 0)
            emit_agg_half(prev, 1)
            emit_drain_and_group(prev)

            psP.release()
            psT.release()
            psS.release()
            l1w.release()
            l1g.release()
            l1m.release()

            # ======== phase 7: AllGather G2 ========
            nc.gpsimd.collective_compute(
                "AllGather", mybir.AluOpType.bypass,
                ins=[g2_loc.opt()], outs=[g2_full.opt()],
                replica_groups=[list(range(NCORES))],
            )
            l1.release()
            g2f = g2_full

            # ======== phase 8+9: layer-2 edge weights + aggregation + W_out ========
            w2m = tc.alloc_tile_pool(name="w2m", bufs=3)
            w2w = tc.alloc_tile_pool(name="w2w", bufs=2)
            psE = tc.alloc_tile_pool(name="psE", bufs=2, space="PSUM")
            psF = tc.alloc_tile_pool(name="psF", bufs=2, space="PSUM")
            # a_d2 hi/lo from the locally saved phase-6 attention values
            nc.vector.tensor_copy(ad2hla[:, :, 0:1], ad2f[:])
            ad2hf = cpool.tile([128, CPC2, 1], F32, name="ad2hf", tag="ad2hf")
            nc.vector.tensor_copy(ad2hf[:], ad2hla[:, :, 0:1])
            nc.vector.tensor_tensor(ad2hla[:, :, 1:2], ad2f[:], ad2hf[:], SUB)
            # software-pipelined like L1: chunk c's chain runs on DVE/ScalarE
            # while chunk c-1's aggregation + output matmuls keep the PE busy
            def emit_l2_agg(st):
                xg2p, M1a = st["xg2"], st["M1a"]
                p30 = psF.tile([128, 128], F32, tag="p30")
                p31 = psF.tile([128, 128], F32, tag="p31")
                st["p30"], st["p31"] = p30, p31
                for j in range(T2):
                    nc.tensor.matmul(
                        p30[:], xg2p[:, j, 0:128], M1a[:, j, :],
                        start=(j == 0), stop=(j == T2 - 1),
                    )
                    nc.tensor.matmul(
                        p31[:], xg2p[:, j, 128:256], M1a[:, j, :],
                        start=(j == 0), stop=(j == T2 - 1),
                    )

            def emit_l2_out(st):
                c2 = st["c"]
                # x3T = relu(agg)  (feature-major: [feat, dst])
                x3T = sb.tile([128, 2, 128], BF16, tag="x3T")
                nc.scalar.activation(x3T[:, 0, :], st["p30"][:], Relu)
                nc.scalar.activation(x3T[:, 1, :], st["p31"][:], Relu)
                pout = psF.tile([128, EMB], F32, tag="pout")
                for fs in range(2):
                    nc.tensor.matmul(
                        pout[:], x3T[:, fs, :], woutt_sb[:, fs, :],
                        start=(fs == 0), stop=(fs == 1),
                    )
                osb = sb.tile([128, EMB], F32, tag="osb")
                nc.scalar.activation(osb[:], pout[:], Copy)
                nc.sync.dma_start(
                    out_dram[c2 * 128:(c2 + 1) * 128, :], osb[:]
                )

            prev2 = None
            for c in range(CPC2):
                tsl = slice(c * T2, (c + 1) * T2)
                xg2t = w2m.tile([128, T2, GW], BF16, tag="xg2")
                nc.gpsimd.dma_gather(
                    xg2t[:], g2f[:, 0:GW],
                    idx2_sb[:, c * T2 * 8:(c + 1) * T2 * 8],
                    T2 * 128, T2 * 128, GW, elem_step=GW,
                )
                xg2 = xg2t[:]
                xg2_f32 = xg2.bitcast(F32)      # [128, T2, 192]
                mt2_sb = w2m.tile([128, T2, 128], BF16, tag="mt2", name="mt2s")
                nc.sync.dma_start(mt2_sb[:], mT2d[:, tsl, :])
                mm2_sb = w2m.tile([128, T2, 128], BF16, tag="mm2", name="mm2s")
                nc.sync.dma_start(mm2_sb[:], m2d[:, tsl, :])

                # packed PSUM bank: [0:12]=a_d lookups, [12:13]=den, [16:28]=rec
                ps2 = psE.tile([128, 4 * T2 + 4], F32, tag="ps2")
                ad2ps = ps2[:, 0:T2 * 2].rearrange("p (a b) -> p a b", b=2)
                den2 = ps2[:, T2 * 2:T2 * 2 + 1]
                rc2ps = ps2[:, T2 * 2 + 2:4 * T2 + 2].rearrange(
                    "p (a b) -> p a b", b=2
                )
                for j in range(T2):
                    nc.tensor.matmul(
                        ad2ps[:, j, :], mt2_sb[:, j, :], ad2hla[:, c, :],
                        start=True, stop=True,
                    )
                # PE filler while chunk c's chain runs
                if prev2 is not None:
                    emit_l2_agg(prev2)
                ad2s = w2w.tile([128, T2, 2], F32, tag="ad2s")
                nc.scalar.activation(ad2s[:], ad2ps, Copy)
                w2e = w2w.tile([128, T2, 1], F32, tag="w2e")
                nc.vector.tensor_tensor(
                    w2e[:], ad2s[:, :, 0:1], ad2s[:, :, 1:2], ADD
                )
                nc.vector.tensor_tensor(
                    w2e[:], w2e[:], xg2_f32[:, 0:T2, 128:129], ADD
                )
                nc.vector.scalar_tensor_tensor(
                    w2e[:], w2e[:], NEG, w2e[:], MUL, MAX
                )
                nc.scalar.activation(w2e[:], w2e[:], Exp)
                w2ebf = w2w.tile([128, T2, 1], BF16, tag="w2ebf")
                nc.scalar.activation(w2ebf[:], w2e[:], Copy)

                for j in range(T2):
                    nc.tensor.matmul(
                        den2[:], mm2_sb[:, j, :], w2ebf[:, j, :],
                        start=(j == 0), stop=(j == T2 - 1),
                    )
                # PE filler while the reciprocal chain runs
                if prev2 is not None:
                    emit_l2_out(prev2)
                rec2 = w2w.tile([128, 1], F32, tag="rec2")
                nc.vector.tensor_scalar(rec2[:], den2[:], 1e-16, None, ADD)
                nc.vector.reciprocal(rec2[:], rec2[:])
                rec2hl = w2w.tile([128, 2], BF16, tag="rec2hl")
                nc.vector.tensor_copy(rec2hl[:, 0:1], rec2[:])
                rec2hf = w2w.tile([128, 1], F32, tag="rec2hf")
                nc.vector.tensor_copy(rec2hf[:], rec2hl[:, 0:1])
                nc.vector.tensor_tensor(rec2hl[:, 1:2], rec2[:], rec2hf[:], SUB)
                for j in range(T2):
                    nc.tensor.matmul(
                        rc2ps[:, j, :], mt2_sb[:, j, :], rec2hl[:],
                        start=True, stop=True,
                    )
                rc2s = w2w.tile([128, T2, 2], F32, tag="rc2s")
                nc.scalar.activation(rc2s[:], rc2ps, Copy)
                alpha2 = w2w.tile([128, T2, 1], F32, tag="alpha2")
                nc.vector.tensor_tensor(
                    alpha2[:], rc2s[:, :, 0:1], rc2s[:, :, 1:2], ADD
                )
                nc.vector.tensor_tensor(alpha2[:], alpha2[:], w2e[:], MUL)

                M1a = w2m.tile([128, T2, 128], BF16, tag="M1a")
                for j in range(T2):
                    t = c * T2 + j
                    nc.vector.scalar_tensor_tensor(
                        M1a[:, j, :], iota128_sb[:], drel2_sb[:, t:t + 1],
                        alpha2[:, j, :].broadcast_to([128, 128]),
                        EQ, MUL,
                    )
                prev2 = {"c": c, "xg2": xg2, "M1a": M1a}

            emit_l2_agg(prev2)
            emit_l2_out(prev2)
            psF.release()
            psE.release()
            w2w.release()
            w2m.release()

    nc.compile()
    return nc


# ================= pjrt execution (axon) with timing =================

_exec_cache = {}


def _run_pjrt(nc, in_maps, key):
    """Mirror of bass2jax.run_bass_via_pjrt with executable caching and
    device-side timing (warmup + timed run when BASS_GAT_TIME=1)."""
    import jax
    from jax.experimental.shard_map import shard_map
    from jax.sharding import Mesh, PartitionSpec
    from concourse import bass2jax, mybir as mb

    global LAST_EXEC_NS
    bass2jax.install_neuronx_cc_hook()

    if key not in _exec_cache:
        partition_name = (
            nc.partition_id_tensor.name if nc.partition_id_tensor else None
        )
        in_names, out_names, out_avals, zero_outs = [], [], [], []
        for alloc in nc.m.functions[0].allocations:
            if not isinstance(alloc, mb.MemoryLocationSet):
                continue
            name = alloc.memorylocations[0].name
            if alloc.kind == "ExternalInput":
                if name != partition_name:
                    in_names.append(name)
            elif alloc.kind == "ExternalOutput":
                shape = tuple(alloc.tensor_shape)
                dtype = mb.dt.np(alloc.dtype)
                out_names.append(name)
                out_avals.append(jax.core.ShapedArray(shape, dtype))
                zero_outs.append(np.zeros(shape, dtype))
        n_params = len(in_names)
        all_in_names = list(in_names) + list(out_names)
        if partition_name is not None:
            all_in_names.append(partition_name)

        def _body(*args):
            operands = list(args)
            if partition_name is not None:
                operands.append(bass2jax.partition_id_tensor())
            outs = bass2jax._bass_exec_p.bind(
                *operands,
                out_avals=tuple(out_avals),
                in_names=tuple(all_in_names),
                out_names=tuple(out_names),
                lowering_input_output_aliases=(),
                sim_require_finite=True,
                sim_require_nnan=True,
                nc=nc,
            )
            return tuple(outs)

        devices = jax.devices()[:NCORES]
        mesh = Mesh(np.asarray(devices), ("core",))
        n_outs = len(out_avals)
        sharded = jax.jit(
            shard_map(
                _body, mesh=mesh,
                in_specs=(PartitionSpec("core"),) * (n_params + n_outs),
                out_specs=(PartitionSpec("core"),) * n_outs,
                check_rep=False,
            ),
            keep_unused=True,
        )
        _exec_cache[key] = (sharded, in_names, out_names, out_avals, zero_outs)
    sharded, in_names, out_names, out_avals, zero_outs = _exec_cache[key]

    import jax
    concat_in = [
        np.concatenate([np.asarray(in_maps[c][n]) for c in range(NCORES)], axis=0)
        for n in in_names
    ]
    concat_zeros = [
        np.zeros((NCORES * z.shape[0], *z.shape[1:]), z.dtype) for z in zero_outs
    ]
    out_arrs = sharded(*concat_in, *concat_zeros)
    jax.block_until_ready(out_arrs)

    if os.environ.get("BASS_GAT_TIME", "0") == "1":
        import time as _time
        args = [jax.device_put(a) for a in concat_in + concat_zeros]
        jax.block_until_ready(args)
        reps = int(os.environ.get("BASS_GAT_REPS", "5"))
        ts = []
        for _ in range(reps):
            t0 = _time.perf_counter()
            o = sharded(*args)
            jax.block_until_ready(o)
            ts.append(_time.perf_counter() - t0)
        LAST_EXEC_NS = int(min(ts) * 1e9)

    return [
        {
            n: np.asarray(out_arrs[i]).reshape(NCORES, *out_avals[i].shape)[c]
            for i, n in enumerate(out_names)
        }
        for c in range(NCORES)
    ]

# ================= entry point =================

def kernel(**inputs) -> np.ndarray:
    inp = {k: np.asarray(v) for k, v in inputs.items()}
    for b in ("b_node", "b_col", "b1", "b2"):
        assert np.abs(inp[b]).max() == 0.0, f"nonzero {b} unsupported"
    b_out = inp["b_out"].astype(np.float32)

    meta = _prep(inp["edges"].astype(np.int64))
    T1, T2 = meta["T1"], meta["T2"]
    wts = _weights_prep(inp)

    key = (T1, T2)
    if key not in _prog_cache:
        _prog_cache[key] = _build_program(T1, T2)
    nc = _prog_cache[key]

    xn = np.tile(inp["constraints_state"].astype(np.float32), (1, 2))  # [4000,128]
    xc = np.tile(inp["columns_state"].astype(np.float32), (1, 2))      # [16000,256]

    in_maps = []
    for m in range(NCORES):
        xct = np.zeros((128, CON_CH * 128), np.float32)
        xcolt = np.zeros((128, 2, COL_CH * 128), np.float32)
        for lc, ch in enumerate(meta["chunks1"][m * CPC1:(m + 1) * CPC1]):
            if lc < CON_CH:
                cols = lc * 128 + np.arange(len(ch))
                xct[:, cols] = xn[ch].T
            else:
                cols = (lc - CON_CH) * 128 + np.arange(len(ch))
                xcv = xc[np.asarray(ch) - NC_NODES]  # [k, 256]
                xcolt[:, 0, cols] = xcv[:, 0:128].T
                xcolt[:, 1, cols] = xcv[:, 128:256].T
        idx1, dr1, m1, mT1 = meta["et1"][m]
        idx2, dr2, m2, mT2 = meta["et2"][m]
        in_maps.append(dict(
            xct=xct, xcolt=xcolt,
            wnodet=wts["wnodet"], wcolt=wts["wcolt"], v1=wts["v1"],
            w1tb=wts["w1tb"], w2tb=wts["w2tb"], att2=wts["att2"],
            woutt=wts["woutt"], iota8i=wts["iota8i"], iota128=wts["iota128"],
            esrc1=idx1, drel1=dr1, m1=m1, mT1=mT1,
            esrc2=idx2, drel2=dr2, m2=m2, mT2=mT2,
        ))

    if os.environ.get("BASS_GAT_NTFF", "0") == "1":
        import ntff_hook
        ntff_hook.install()
        import tempfile
        global LAST_EXEC_NS, LAST_RESULTS
        td = tempfile.mkdtemp(prefix="gat_trace_")
        res = bass_utils.run_bass_kernel_spmd(
            nc, in_maps, core_ids=list(range(NCORES)), trace=True, tmpdir=td,
        )
        LAST_EXEC_NS = res.exec_time_ns
        LAST_RESULTS = res
        print("trace dir:", td)
        results = res.results
    else:
        results = _run_pjrt(nc, in_maps, key)

    out = np.zeros((NCOL, EMB), np.float32)
    for m in range(NCORES):
        o = np.asarray(results[m]["out"]).astype(np.float32)
        for lc, ch in enumerate(meta["chunks2"][m * CPC2:(m + 1) * CPC2]):
            if ch:
                rows = lc * 128 + np.arange(len(ch))
                out[np.asarray(ch) - NC_NODES] = o[rows]
    return out + b_out[None, :]


# revision 41
# speedup vs baseline: 1.1304x; 1.1304x over previous
"""GAT (2-layer, PyG-style) Trainium2 kernel, 8-core SPMD.

Strategy:
  - Nodes assigned to (core, 128-chunk) slots, load-balanced by in-degree;
    L2 chunks are co-located with the L1 column chunks (same membership), so
    layer-2 dst attention values stay core-local.
  - Aggregation in x-space (256-wide) with TRANSPOSED layout: the gathered
    source features are the matmul STATIONARY operand; the moving operand is
    a per-edge alpha-scaled one-hot mask block M8[e, (head, dstpos)] built in
    ONE fused DVE op (scalar_tensor_tensor: (iota==drel)*alpha) per edge
    tile. Output lands feature-major, which is exactly the layout the W1/W2
    projections need, so no transpose roundtrip.
  - Attention: a_src rides the feature gather (f32 cols in the same 768B
    table row); a_dst is expanded per-edge with tiny matmuls against
    host-uploaded static one-hot masks (m = [e,dst], mT = [dst,e]; bf16
    hi/lo splits keep the lookups near-exact); softmax denominators via
    m-matmuls; masks are pre-normalized by 1/den (alpha), so no
    post-scaling of the aggregate.
  - Software pipelining: chunk c's attention chain (DVE/ScalarE/small PE
    lookups) runs while chunk c-1's aggregation matmuls keep the PE busy;
    the layer-1->layer-2 projections (phases 5/6) run per 512-node group
    inside the same loop with small rotating buffers.
  - Cross-core: two AllGathers of the 768B-row node tables (G1, G2).
"""
import os, sys
import numpy as np
import ml_dtypes

sys.path.insert(0, "/opt/trn_rl_repo")
import concourse.bass as bass
import concourse.mybir as mybir
import concourse.tile as tile
import concourse.bacc as bacc
from concourse import bass_utils

F32 = mybir.dt.float32
BF16 = mybir.dt.bfloat16
I16 = mybir.dt.int16
BF = ml_dtypes.bfloat16

# ---------------- problem constants ----------------
NC_NODES = 4000
NCOL = 16000
N = NC_NODES + NCOL
NF, CF = 64, 128
HID = 256
H = 8
EMB = 128
NEG = 0.2

NCORES = 8
CON_CH = 4
COL_CH = 16
CPC1 = CON_CH + COL_CH          # 20
SLOT1 = CPC1 * 128              # 2560
CPC2 = 16
SLOT2 = CPC2 * 128              # 2048
GW = 384                        # bf16 table width (768B stride); f32 view 192
                                # (dma_gather elem size must be a multiple of
                                #  256B: 384*2 = 768B)

_prog_cache = {}
LAST_EXEC_NS = None
LAST_RESULTS = None


# ================= host-side preprocessing =================

def _balance(nodes, deg, n_chunks, cap=128):
    import heapq
    order = nodes[np.argsort(-deg[nodes], kind="stable")]
    loads = np.zeros(n_chunks, dtype=np.int64)
    counts = np.zeros(n_chunks, dtype=np.int64)
    heap = [(0, c) for c in range(n_chunks)]
    heapq.heapify(heap)
    members = [[] for _ in range(n_chunks)]
    for nd in order:
        while True:
            _, c = heapq.heappop(heap)
            if counts[c] < cap:
                break
        members[c].append(int(nd))
        counts[c] += 1
        loads[c] += int(deg[nd])
        if counts[c] < cap:
            heapq.heappush(heap, (loads[c], c))
    return members, loads


def _wrap_idx(idx):
    """dma_gather int16 index layout: [128, n/16]; row p holds idx[s*16+p%16]."""
    idx = np.asarray(idx, dtype=np.int16)
    n = len(idx)
    assert n % 16 == 0
    m = idx.reshape(n // 16, 16).T
    return np.tile(m, (8, 1)).copy()


def _onehots(drel, cpc, T):
    """drel: [cpc, T*128] float (dst position in chunk, or -1 pad).
    Returns m  [128(e), cpc*T, 128(p)]  and mT [128(p), cpc*T, 128(e)]  bf16."""
    d = drel.reshape(cpc, T, 128).astype(np.int32)       # [c, j, e]
    oh = (d[:, :, :, None] == np.arange(128)[None, None, None, :])  # [c,j,e,p]
    m = np.ascontiguousarray(
        oh.transpose(2, 0, 1, 3).reshape(128, cpc * T, 128)
    ).astype(BF)
    mT = np.ascontiguousarray(
        oh.transpose(3, 0, 1, 2).reshape(128, cpc * T, 128)
    ).astype(BF)
    return m, mT


def _prep(edges):
    src1 = np.concatenate([edges[0], np.arange(N)]).astype(np.int64)
    dst1 = np.concatenate([edges[1], np.arange(N)]).astype(np.int64)
    s2 = np.concatenate([edges[1], np.arange(N)]).astype(np.int64)
    d2 = np.concatenate([edges[0], np.arange(N)]).astype(np.int64)
    keep = d2 >= NC_NODES
    src2, dst2 = s2[keep], d2[keep]

    deg1 = np.bincount(dst1, minlength=N)
    deg2 = np.bincount(dst2, minlength=N)

    con_members, con_loads = _balance(np.arange(NC_NODES), deg1, NCORES * CON_CH)
    # column chunks serve BOTH layers (L2 chunks == L1 col chunks); balance on
    # deg1 (the larger layer) and accept the resulting T2
    col_members, _ = _balance(np.arange(NC_NODES, N), deg1, NCORES * COL_CH)
    gslot1 = np.full(N, -1, dtype=np.int64)
    chunks1 = [[] for _ in range(NCORES * CPC1)]
    for g, mem in enumerate(con_members):
        core, lc = g % NCORES, g // NCORES
        chunks1[core * CPC1 + lc] = mem
    for g, mem in enumerate(col_members):
        core, lc = g % NCORES, CON_CH + g // NCORES
        chunks1[core * CPC1 + lc] = mem
    for ci, mem in enumerate(chunks1):
        core, lc = divmod(ci, CPC1)
        for pos, nd in enumerate(mem):
            gslot1[nd] = core * SLOT1 + lc * 128 + pos
    assert (gslot1 >= 0).all()

    # chunks2 = the column chunks of layer 1 (identity co-location)
    chunks2 = [
        chunks1[core * CPC1 + CON_CH + lc]
        for core in range(NCORES) for lc in range(CPC2)
    ]
    # realized per-chunk loads determine the tile counts
    def chunk_load(members_list, deg):
        return max(
            (sum(deg[nd] for nd in mem) for mem in members_list if mem),
            default=0,
        )
    T1 = max(4, int(np.ceil(max(
        chunk_load([chunks1[i] for i in range(len(chunks1))], deg1), 1
    ) / 128)))
    T2 = max(4, int(np.ceil(max(chunk_load(chunks2, deg2), 1) / 128)))

    # table row layout is half-major (for split AllGathers):
    # row = half*(NCORES*HR) + core*HR + loc%HR,  HR = SLOT1//2
    HR = SLOT1 // 2
    def row_of(g):
        core, loc = g // SLOT1, g % SLOT1
        return (loc // HR) * (NCORES * HR) + core * HR + loc % HR

    # dst slot mapping for L2: position within the L1 col-chunk
    def edge_tables(src, dst, cpc, T, chunk_of_node, pos_of_node, remap):
        """Per core: src gather idx, drel, and static one-hot masks."""
        dcore = gslot1[dst] // SLOT1
        order = np.argsort(
            dcore * (cpc * 128) + chunk_of_node[dst] * 128 + pos_of_node[dst],
            kind="stable",
        )
        so, do = src[order], dst[order]
        core_of = dcore[order]
        cm_all, pm_all = chunk_of_node[do], pos_of_node[do]
        res = []
        for mcore in range(NCORES):
            esrc = np.zeros((cpc, T * 128), dtype=np.int64)
            drel = np.full((cpc, T * 128), -1.0, dtype=np.float32)
            sel = core_of == mcore
            sm, cm, pm = so[sel], cm_all[sel], pm_all[sel]
            for lc in range(cpc):
                s = cm == lc
                k = int(s.sum())
                assert k <= T * 128, f"chunk overflow {k} > {T*128}"
                esrc[lc, :k] = remap(gslot1[sm[s]])
                drel[lc, :k] = pm[s]
            idx = _wrap_idx(esrc.reshape(-1))
            m, mT = _onehots(drel, cpc, T)
            drel_dev = np.ascontiguousarray(
                drel.reshape(cpc, T, 128).transpose(2, 0, 1).reshape(128, cpc * T)
            )
            res.append((idx, drel_dev, m, mT))
        return res

    chunk1_of = (gslot1 % SLOT1) // 128          # L1 chunk index per node
    pos_of = gslot1 % 128
    chunk2_of = chunk1_of - CON_CH               # L2 chunk index (col nodes)
    et1 = edge_tables(src1, dst1, CPC1, T1, chunk1_of, pos_of, lambda g: g)
    et2 = edge_tables(src2, dst2, CPC2, T2, chunk2_of, pos_of, lambda g: g)
    return dict(gslot1=gslot1, chunks1=chunks1, chunks2=chunks2,
                T1=T1, T2=T2, et1=et1, et2=et2)


def _weights_prep(inp):
    W1 = inp["W1"].astype(np.float32)       # [2048, 256]
    W2 = inp["W2"].astype(np.float32)       # [256, 2048]
    out = {}
    out["wnodet"] = np.ascontiguousarray(inp["W_node"].T).astype(np.float32)  # [128,256]
    wct = inp["W_col"].T.astype(np.float32)  # [256, 256]
    out["wcolt"] = np.stack([wct[0:128], wct[128:256]], axis=1)  # [128, 2, 256]
    V1 = np.zeros((256, 16), np.float32)
    for h in range(H):
        Wh = W1[h * HID:(h + 1) * HID, :]
        V1[:, h] = Wh.T @ inp["att_src1"][h]
        V1[:, 8 + h] = Wh.T @ inp["att_dst1"][h]
    out["v1"] = np.stack([V1[0:128], V1[128:256]], axis=1)       # [128, 2, 16]
    W1T = W1.T                                                   # [256, 2048]
    w1tb = np.zeros((128, 32, 128), BF)
    for h in range(H):
        for os_ in range(2):
            for fs in range(2):
                w1tb[:, h * 4 + os_ * 2 + fs, :] = W1T[
                    fs * 128:(fs + 1) * 128,
                    h * 256 + os_ * 128: h * 256 + (os_ + 1) * 128,
                ].astype(BF)
    out["w1tb"] = w1tb
    W2T = W2.T                                                   # [2048, 256]
    w2tb = np.zeros((128, 32, 128), BF)
    for f16 in range(16):
        for os_ in range(2):
            w2tb[:, f16 * 2 + os_, :] = W2T[
                f16 * 128:(f16 + 1) * 128, os_ * 128:(os_ + 1) * 128
            ].astype(BF)
    out["w2tb"] = w2tb
    a2 = np.stack([inp["att_src2"][0], inp["att_dst2"][0]], axis=1)  # [256, 2]
    out["att2"] = np.stack([a2[0:128], a2[128:256]], axis=1).astype(BF)  # [128,2,2]
    wot = inp["W_out"].T.astype(np.float32)  # [256, 128]
    out["woutt"] = np.stack([wot[0:128], wot[128:256]], axis=1).astype(BF)  # [128,2,128]
    # iota8i[e, p*8+h] = p  (interleaved one-hot comparison pattern, L1)
    out["iota8i"] = np.broadcast_to(
        (np.arange(1024) // 8).astype(BF), (128, 1024)
    ).copy()
    # iota128[e, p] = p (L2)
    out["iota128"] = np.broadcast_to(
        np.arange(128).astype(BF), (128, 128)
    ).copy()
    return out


# ================= device program =================

def _build_program(T1, T2):
    nc = bacc.Bacc(None, target_bir_lowering=False)
    NT1, NT2 = CPC1 * T1, CPC2 * T2

    xct = nc.dram_tensor("xct", [128, CON_CH * 128], F32, kind="ExternalInput")
    xcolt = nc.dram_tensor("xcolt", [128, 2, COL_CH * 128], F32, kind="ExternalInput")
    wnodet = nc.dram_tensor("wnodet", [128, 256], F32, kind="ExternalInput")
    wcolt = nc.dram_tensor("wcolt", [128, 2, 256], F32, kind="ExternalInput")
    v1 = nc.dram_tensor("v1", [128, 2, 16], F32, kind="ExternalInput")
    w1tb = nc.dram_tensor("w1tb", [128, 32, 128], BF16, kind="ExternalInput")
    w2tb = nc.dram_tensor("w2tb", [128, 32, 128], BF16, kind="ExternalInput")
    att2 = nc.dram_tensor("att2", [128, 2, 2], BF16, kind="ExternalInput")
    woutt = nc.dram_tensor("woutt", [128, 2, 128], BF16, kind="ExternalInput")
    iota8i = nc.dram_tensor("iota8i", [128, 1024], BF16, kind="ExternalInput")
    iota128 = nc.dram_tensor("iota128", [128, 128], BF16, kind="ExternalInput")
    esrc1 = nc.dram_tensor("esrc1", [128, NT1 * 8], I16, kind="ExternalInput")
    drel1 = nc.dram_tensor("drel1", [128, NT1], F32, kind="ExternalInput")
    m1d = nc.dram_tensor("m1", [128, NT1, 128], BF16, kind="ExternalInput")
    mT1d = nc.dram_tensor("mT1", [128, NT1, 128], BF16, kind="ExternalInput")
    esrc2 = nc.dram_tensor("esrc2", [128, NT2 * 8], I16, kind="ExternalInput")
    drel2 = nc.dram_tensor("drel2", [128, NT2], F32, kind="ExternalInput")
    m2d = nc.dram_tensor("m2", [128, NT2, 128], BF16, kind="ExternalInput")
    mT2d = nc.dram_tensor("mT2", [128, NT2, 128], BF16, kind="ExternalInput")
    out_dram = nc.dram_tensor("out", [SLOT2, EMB], F32, kind="ExternalOutput")

    Copy = mybir.ActivationFunctionType.Copy
    Relu = mybir.ActivationFunctionType.Relu
    Exp = mybir.ActivationFunctionType.Exp
    ADD, EQ, MUL, MAX, SUB = (
        mybir.AluOpType.add, mybir.AluOpType.is_equal,
        mybir.AluOpType.mult, mybir.AluOpType.max,
        mybir.AluOpType.subtract,
    )

    with tile.TileContext(nc) as tc:
        with (
            tc.tile_pool(name="const", bufs=1) as cpool,
            tc.tile_pool(name="sb", bufs=3) as sb,
            tc.tile_pool(name="dram", bufs=1, space="DRAM") as dram,
        ):
            def cload(t, shape, dtype):
                nm = t.name + "_sb"
                s = cpool.tile(shape, dtype, name=nm, tag=nm)
                nc.sync.dma_start(s[:], t[:])
                return s

            iota8i_sb = cload(iota8i, [128, 1024], BF16)
            iota128_sb = cload(iota128, [128, 128], BF16)
            wnodet_sb = cload(wnodet, [128, 256], F32)
            wcolt_sb = cload(wcolt, [128, 2, 256], F32)
            v1_sb = cload(v1, [128, 2, 16], F32)
            w1tb_sb = cload(w1tb, [128, 32, 128], BF16)
            w2tb_sb = cload(w2tb, [128, 32, 128], BF16)
            att2_sb = cload(att2, [128, 2, 2], BF16)
            woutt_sb = cload(woutt, [128, 2, 128], BF16)
            idx1_sb = cload(esrc1, [128, NT1 * 8], I16)
            drel1_sb = cload(drel1, [128, NT1], F32)
            idx2_sb = cload(esrc2, [128, NT2 * 8], I16)
            drel2_sb = cload(drel2, [128, NT2], F32)

            g1_loc = dram.tile([SLOT1, GW], BF16)
            g1_full = dram.tile([NCORES * SLOT1, GW], BF16, addr_space="Shared")
            g2_loc = dram.tile([SLOT1, GW], BF16)
            g2_full = dram.tile([NCORES * SLOT1, GW], BF16, addr_space="Shared")

            ad2f = cpool.tile([128, CPC2, 1], F32, name="ad2f", tag="ad2f")
            ad2hla = cpool.tile([128, CPC2, 2], BF16, name="ad2hla", tag="ad2hla")

            # long-lived L1 pool (adhl written in phase 1, read through L1)
            l1 = tc.alloc_tile_pool(name="l1", bufs=1)
            aggnT4 = l1.tile([128, 2, 4, 8, 128], BF16, tag="aggnT4")
            adhl = l1.tile([128, CPC1, 16], BF16, tag="adhl")

            # ======== phase 1: input MLPs ========
            p1 = tc.alloc_tile_pool(name="p1", bufs=1)
            psA = tc.alloc_tile_pool(name="psA", bufs=2, space="PSUM")
            xct_sb = p1.tile([128, CON_CH * 128], F32, tag="xct_sb")
            nc.sync.dma_start(xct_sb[:], xct[:])
            xcolt_sb = p1.tile([128, 2, COL_CH * 128], F32, tag="xcolt_sb")
            nc.sync.dma_start(xcolt_sb[:], xcolt[:])
            xT = p1.tile([128, 2, SLOT1], F32, tag="xT")
            for os_ in range(2):
                p = psA.tile([128, CON_CH * 128], F32, tag="pmlp")
                nc.tensor.matmul(
                    p[:], wnodet_sb[:, os_ * 128:(os_ + 1) * 128], xct_sb[:],
                    start=True, stop=True,
                )
                nc.scalar.activation(xT[:, os_, 0:CON_CH * 128], p[:], Relu)
                for nch in range(4):
                    p2 = psA.tile([128, 512], F32, tag="pmlp2")
                    for fs in range(2):
                        nc.tensor.matmul(
                            p2[:],
                            wcolt_sb[:, fs, os_ * 128:(os_ + 1) * 128],
                            xcolt_sb[:, fs, nch * 512:(nch + 1) * 512],
                            start=(fs == 0), stop=(fs == 1),
                        )
                    nc.scalar.activation(
                        xT[:, os_, CON_CH * 128 + nch * 512: CON_CH * 128 + (nch + 1) * 512],
                        p2[:], Relu,
                    )

            # node-major x + attention values -> G1 rows; keep a_d locally
            g1sb = p1.tile([128, CPC1, GW], BF16, tag="g1sb")
            g1sb_f32 = g1sb[:].bitcast(F32)       # [128, CPC1, 192]
            pa_all = p1.tile([128, CPC1, 16], F32, tag="pa_all")
            for c in range(CPC1):
                nsl = slice(c * 128, (c + 1) * 128)
                px = psA.tile([128, 256], F32, tag="px")
                if c < CON_CH:
                    nc.tensor.matmul(
                        px[:], xct_sb[:, nsl], wnodet_sb[:], start=True, stop=True
                    )
                else:
                    ksl = slice((c - CON_CH) * 128, (c - CON_CH) * 128 + 128)
                    for fs in range(2):
                        nc.tensor.matmul(
                            px[:], xcolt_sb[:, fs, ksl], wcolt_sb[:, fs, :],
                            start=(fs == 0), stop=(fs == 1),
                        )
                pa = psA.tile([128, 16], F32, tag="pa")
                for fs in range(2):
                    nc.tensor.matmul(
                        pa[:], xT[:, fs, nsl], v1_sb[:, fs, :],
                        start=(fs == 0), stop=(fs == 1),
                    )
                nc.scalar.activation(g1sb[:, c, 0:256], px[:], Relu)
                nc.vector.tensor_copy(g1sb_f32[:, c, 128:136], pa[:, 0:8])
                nc.vector.tensor_copy(pa_all[:, c, :], pa[:])

            # a_d hi/lo split for exact bf16-matmul lookups: [128, CPC1, 16]
            nc.vector.tensor_copy(adhl[:, :, 0:8], pa_all[:, :, 8:16])
            adhif = p1.tile([128, CPC1, 8], F32, tag="adhif")
            nc.vector.tensor_copy(adhif[:], adhl[:, :, 0:8])
            nc.vector.tensor_tensor(
                adhl[:, :, 8:16], pa_all[:, :, 8:16], adhif[:], SUB
            )
            nc.sync.dma_start(
                g1_loc[:].rearrange("(c p) w -> p c w", p=128), g1sb[:]
            )

            psA.release()

            # ======== phase 2: AllGather G1 ========
            nc.gpsimd.collective_compute(
                "AllGather", mybir.AluOpType.bypass,
                ins=[g1_loc.opt()], outs=[g1_full.opt()],
                replica_groups=[list(range(NCORES))],
            )
            p1.release()
            g1f = g1_full

            # ======== phase 3+4: layer-1 edge weights + aggregation ========
            l1m = tc.alloc_tile_pool(name="l1m", bufs=3)
            l1g = tc.alloc_tile_pool(name="l1g", bufs=2)
            l1w = tc.alloc_tile_pool(name="l1w", bufs=2)
            psS = tc.alloc_tile_pool(name="psS", bufs=2, space="PSUM")
            psT = tc.alloc_tile_pool(name="psT", bufs=1, space="PSUM")
            psP = tc.alloc_tile_pool(name="psP", bufs=2, space="PSUM")

            # software-pipelined over chunks: while chunk c's attention chain
            # runs on DVE/ScalarE, chunk c-1's aggregation matmuls keep the PE
            # busy (emitted into the chain's dependency gaps).

            def emit_agg_half(st, which):
                c, xg, M8a = st["c"], st["xg"], st["M8a"]
                pT = psT.tile([128, 1024], F32, tag=f"pT{which}")
                st[f"pT{which}"] = pT
                fsl = slice(which * 128, (which + 1) * 128)
                for j in range(T1):
                    M8f = M8a[:, j, :, :].rearrange("p a b -> p (a b)")
                    for half in range(2):
                        nc.tensor.matmul(
                            pT[:, half * 512:(half + 1) * 512],
                            xg[:, j, fsl], M8f[:, half * 512:(half + 1) * 512],
                            start=(j == 0), stop=(j == T1 - 1),
                        )

            def emit_drain_and_group(st):
                c = st["c"]
                nc.scalar.activation(
                    aggnT4[:, 0, c % 4, :, :].rearrange("p a b -> p (a b)"),
                    st["pT0"][:], Copy,
                )
                nc.scalar.activation(
                    aggnT4[:, 1, c % 4, :, :].rearrange("p a b -> p (a b)"),
                    st["pT1"][:], Copy,
                )
                if c % 4 != 3:
                    return
                # phases 5+6 for the completed 4-chunk group (512 nodes)
                g = c // 4
                x2Tg = l1g.tile([128, 16, 512], BF16, tag="x2Tg")
                for hh in range(16):
                    h, os_ = hh // 2, hh % 2
                    px2 = psP.tile([128, 512], F32, tag="pproj")
                    for fs in range(2):
                        rhs = aggnT4[:, fs, :, h, :]   # [128, 4, 128]
                        nc.tensor.matmul(
                            px2[:],
                            w1tb_sb[:, h * 4 + os_ * 2 + fs, :],
                            rhs,
                            start=(fs == 0), stop=(fs == 1),
                        )
                    nc.scalar.activation(x2Tg[:, hh, :], px2[:], Relu)
                h2Tg = l1g.tile([128, 2, 512], BF16, tag="h2Tg")
                for os_ in range(2):
                    ph2 = psP.tile([128, 512], F32, tag="pproj")
                    for f16 in range(16):
                        nc.tensor.matmul(
                            ph2[:], w2tb_sb[:, f16 * 2 + os_, :],
                            x2Tg[:, f16, :],
                            start=(f16 == 0), stop=(f16 == 15),
                        )
                    nc.scalar.activation(h2Tg[:, os_, :], ph2[:], Copy)
                for ci in range(4):
                    cg = g * 4 + ci
                    nsl = slice(ci * 128, (ci + 1) * 128)
                    pa2 = psP.tile([128, 2], F32, tag="pproj")
                    for fs in range(2):
                        nc.tensor.matmul(
                            pa2[:], h2Tg[:, fs, nsl], att2_sb[:, fs, :],
                            start=(fs == 0), stop=(fs == 1),
                        )
                    g2c = l1g.tile([128, 1, GW], BF16, tag="g2c")
                    g2c_f32 = g2c[:].bitcast(F32)
                    for fs in range(2):
                        nc.sync.dma_start(
                            g2c[:, 0, fs * 128:(fs + 1) * 128],
                            h2Tg[:, fs, nsl], transpose=True,
                        )
                    nc.vector.tensor_copy(g2c_f32[:, 0, 128:130], pa2[:])
                    if cg >= CON_CH:
                        nc.vector.tensor_copy(
                            ad2f[:, cg - CON_CH, :], pa2[:, 1:2]
                        )
                    nc.sync.dma_start(
                        g2_loc[:].rearrange("(c p) w -> p c w", p=128)[
                            :, cg:cg + 1, :
                        ],
                        g2c[:],
                    )

            prev = None
            for c in range(CPC1):
                tsl = slice(c * T1, (c + 1) * T1)
                xgt = l1m.tile([128, T1, GW], BF16, tag="xg1")
                nc.gpsimd.dma_gather(
                    xgt[:], g1f[:, 0:GW],
                    idx1_sb[:, c * T1 * 8:(c + 1) * T1 * 8],
                    T1 * 128, T1 * 128, GW, elem_step=GW,
                )
                xg = xgt[:]
                xg_f32 = xg.bitcast(F32)        # [128, T1, 192]
                mt_sb = l1m.tile([128, T1, 128], BF16, tag="mt1", name="mt1s")
                nc.sync.dma_start(mt_sb[:], mT1d[:, tsl, :])
                mm_sb = l1m.tile([128, T1, 128], BF16, tag="mm1", name="mm1s")
                nc.sync.dma_start(mm_sb[:], m1d[:, tsl, :])

                # one packed PSUM bank for the small per-chunk matmul outs
                psmall = psS.tile([128, 2 * T1 * 16 + 16], F32, tag="psmall")
                adps = psmall[:, 0:T1 * 16].rearrange("p (a b) -> p a b", b=16)
                den = psmall[:, T1 * 16:T1 * 16 + 8]
                rcps = psmall[:, T1 * 16 + 16:2 * T1 * 16 + 16].rearrange(
                    "p (a b) -> p a b", b=16
                )
                for j in range(T1):
                    nc.tensor.matmul(
                        adps[:, j, :], mt_sb[:, j, :], adhl[:, c, :],
                        start=True, stop=True,
                    )
                # PE filler while chunk c's chain runs on DVE/ScalarE
                if prev is not None:
                    emit_agg_half(prev, 0)
                ads = l1w.tile([128, T1, 16], F32, tag="ads")
                nc.scalar.activation(ads[:], adps, Copy)
                w1e = l1w.tile([128, T1, 8], F32, tag="w1e")
                # e = a_s + ad_hi + ad_lo
                nc.vector.tensor_tensor(
                    w1e[:], ads[:, :, 0:8], ads[:, :, 8:16], ADD
                )
                nc.vector.tensor_tensor(
                    w1e[:], w1e[:],
                    xg_f32[:, :, 128:136], ADD
                )
                nc.vector.scalar_tensor_tensor(
                    w1e[:], w1e[:], NEG, w1e[:], MUL, MAX
                )
                nc.scalar.activation(w1e[:], w1e[:], Exp)
                w1ebf = l1w.tile([128, T1, 8], BF16, tag="w1ebf")
                nc.scalar.activation(w1ebf[:], w1e[:], Copy)

                # denominators + reciprocal hi/lo
                for j in range(T1):
                    nc.tensor.matmul(
                        den[:], mm_sb[:, j, :], w1ebf[:, j, :],
                        start=(j == 0), stop=(j == T1 - 1),
                    )
                # PE filler while the reciprocal chain runs
                if prev is not None:
                    emit_agg_half(prev, 1)
                    emit_drain_and_group(prev)

                rec = l1w.tile([128, 8], F32, tag="rec")
                nc.vector.tensor_scalar_add(rec[:], den[:], 1e-16)
                nc.vector.reciprocal(rec[:], rec[:])
                rechl = l1w.tile([128, 16], BF16, tag="rechl")
                nc.vector.tensor_copy(rechl[:, 0:8], rec[:])
                rechf = l1w.tile([128, 8], F32, tag="rechf")
                nc.vector.tensor_copy(rechf[:], rechl[:, 0:8])
                nc.vector.tensor_tensor(rechl[:, 8:16], rec[:], rechf[:], SUB)

                # rec per edge + alpha
                for j in range(T1):
                    nc.tensor.matmul(
                        rcps[:, j, :], mt_sb[:, j, :], rechl[:],
                        start=True, stop=True,
                    )
                rcs = l1w.tile([128, T1, 16], F32, tag="rcs")
                nc.scalar.activation(rcs[:], rcps, Copy)
                alpha = l1w.tile([128, T1, 8], BF16, tag="alpha")
                rsum = l1w.tile([128, T1, 8], F32, tag="rsum")
                nc.vector.tensor_tensor(
                    rsum[:], rcs[:, :, 0:8], rcs[:, :, 8:16], ADD
                )
                nc.vector.tensor_tensor(alpha[:], w1e[:], rsum[:], MUL)

                # alpha-scaled interleaved masks for all tiles of this chunk
                M8a = l1g.tile([128, T1, 8, 128], BF16, tag="M8a")
                for j in range(T1):
                    t = c * T1 + j
                    nc.vector.scalar_tensor_tensor(
                        M8a[:, j, :, :],
                        iota128_sb[:].unsqueeze(1).broadcast_to([128, 8, 128]),
                        drel1_sb[:, t:t + 1],
                        alpha[:, j, :].unsqueeze(2).broadcast_to([128, 8, 128]),
                        EQ, MUL,
                    )
                prev = {"c": c, "xg": xg, "M8a": M8a}

            emit_agg_half(prev, 0)
            emit_agg_half(prev, 1)
            emit_drain_and_group(prev)

            psP.release()
            psT.release()
            psS.release()
            l1w.release()
            l1g.release()
            l1m.release()

            # ======== phase 7: AllGather G2 ========
            nc.gpsimd.collective_compute(
                "AllGather", mybir.AluOpType.bypass,
                ins=[g2_loc.opt()], outs=[g2_full.opt()],
                replica_groups=[list(range(NCORES))],
            )
            l1.release()
            g2f = g2_full

            # ======== phase 8+9: layer-2 edge weights + aggregation + W_out ========
            w2m = tc.alloc_tile_pool(name="w2m", bufs=3)
            w2w = tc.alloc_tile_pool(name="w2w", bufs=2)
            psE = tc.alloc_tile_pool(name="psE", bufs=2, space="PSUM")
            psF = tc.alloc_tile_pool(name="psF", bufs=2, space="PSUM")
            # a_d2 hi/lo from the locally saved phase-6 attention values
            nc.vector.tensor_copy(ad2hla[:, :, 0:1], ad2f[:])
            ad2hf = cpool.tile([128, CPC2, 1], F32, name="ad2hf", tag="ad2hf")
            nc.vector.tensor_copy(ad2hf[:], ad2hla[:, :, 0:1])
            nc.vector.tensor_tensor(ad2hla[:, :, 1:2], ad2f[:], ad2hf[:], SUB)
            # software-pipelined like L1: chunk c's chain runs on DVE/ScalarE
            # while chunk c-1's aggregation + output matmuls keep the PE busy
            def emit_l2_agg(st):
                xg2p, M1a = st["xg2"], st["M1a"]
                p30 = psF.tile([128, 128], F32, tag="p30")
                p31 = psF.tile([128, 128], F32, tag="p31")
                st["p30"], st["p31"] = p30, p31
                for j in range(T2):
                    nc.tensor.matmul(
                        p30[:], xg2p[:, j, 0:128], M1a[:, j, :],
                        start=(j == 0), stop=(j == T2 - 1),
                    )
                    nc.tensor.matmul(
                        p31[:], xg2p[:, j, 128:256], M1a[:, j, :],
                        start=(j == 0), stop=(j == T2 - 1),
                    )

            def emit_l2_out(st):
                c2 = st["c"]
                # x3T = relu(agg)  (feature-major: [feat, dst])
                x3T = sb.tile([128, 2, 128], BF16, tag="x3T")
                nc.scalar.activation(x3T[:, 0, :], st["p30"][:], Relu)
                nc.scalar.activation(x3T[:, 1, :], st["p31"][:], Relu)
                pout = psF.tile([128, EMB], F32, tag="pout")
                for fs in range(2):
                    nc.tensor.matmul(
                        pout[:], x3T[:, fs, :], woutt_sb[:, fs, :],
                        start=(fs == 0), stop=(fs == 1),
                    )
                osb = sb.tile([128, EMB], F32, tag="osb")
                nc.scalar.activation(osb[:], pout[:], Copy)
                nc.sync.dma_start(
                    out_dram[c2 * 128:(c2 + 1) * 128, :], osb[:]
                )

            prev2 = None
            for c in range(CPC2):
                tsl = slice(c * T2, (c + 1) * T2)
                xg2t = w2m.tile([128, T2, GW], BF16, tag="xg2")
                nc.gpsimd.dma_gather(
                    xg2t[:], g2f[:, 0:GW],
                    idx2_sb[:, c * T2 * 8:(c + 1) * T2 * 8],
                    T2 * 128, T2 * 128, GW, elem_step=GW,
                )
                xg2 = xg2t[:]
                xg2_f32 = xg2.bitcast(F32)      # [128, T2, 192]
                mt2_sb = w2m.tile([128, T2, 128], BF16, tag="mt2", name="mt2s")
                nc.sync.dma_start(mt2_sb[:], mT2d[:, tsl, :])
                mm2_sb = w2m.tile([128, T2, 128], BF16, tag="mm2", name="mm2s")
                nc.sync.dma_start(mm2_sb[:], m2d[:, tsl, :])

                # packed PSUM bank: [0:12]=a_d lookups, [12:13]=den, [16:28]=rec
                ps2 = psE.tile([128, 4 * T2 + 4], F32, tag="ps2")
                ad2ps = ps2[:, 0:T2 * 2].rearrange("p (a b) -> p a b", b=2)
                den2 = ps2[:, T2 * 2:T2 * 2 + 1]
                rc2ps = ps2[:, T2 * 2 + 2:4 * T2 + 2].rearrange(
                    "p (a b) -> p a b", b=2
                )
                for j in range(T2):
                    nc.tensor.matmul(
                        ad2ps[:, j, :], mt2_sb[:, j, :], ad2hla[:, c, :],
                        start=True, stop=True,
                    )
                # PE filler while chunk c's chain runs
                if prev2 is not None:
                    emit_l2_agg(prev2)
                ad2s = w2w.tile([128, T2, 2], F32, tag="ad2s")
                nc.scalar.activation(ad2s[:], ad2ps, Copy)
                w2e = w2w.tile([128, T2, 1], F32, tag="w2e")
                nc.vector.tensor_tensor(
                    w2e[:], ad2s[:, :, 0:1], ad2s[:, :, 1:2], ADD
                )
                nc.vector.tensor_tensor(
                    w2e[:], w2e[:], xg2_f32[:, 0:T2, 128:129], ADD
                )
                nc.vector.scalar_tensor_tensor(
                    w2e[:], w2e[:], NEG, w2e[:], MUL, MAX
                )
                nc.scalar.activation(w2e[:], w2e[:], Exp)
                w2ebf = w2w.tile([128, T2, 1], BF16, tag="w2ebf")
                nc.scalar.activation(w2ebf[:], w2e[:], Copy)

                for j in range(T2):
                    nc.tensor.matmul(
                        den2[:], mm2_sb[:, j, :], w2ebf[:, j, :],
                        start=(j == 0), stop=(j == T2 - 1),
                    )
                # PE filler while the reciprocal chain runs
                if prev2 is not None:
                    emit_l2_out(prev2)
                rec2 = w2w.tile([128, 1], F32, tag="rec2")
                nc.vector.tensor_scalar(rec2[:], den2[:], 1e-16, None, ADD)
                nc.vector.reciprocal(rec2[:], rec2[:])
                rec2hl = w2w.tile([128, 2], BF16, tag="rec2hl")
                nc.vector.tensor_copy(rec2hl[:, 0:1], rec2[:])
                rec2hf = w2w.tile([128, 1], F32, tag="rec2hf")
                nc.vector.tensor_copy(rec2hf[:], rec2hl[:, 0:1])
                nc.vector.tensor_tensor(rec2hl[:, 1:2], rec2[:], rec2hf[:], SUB)
                for j in range(T2):
                    nc.tensor.matmul(
                        rc2ps[:, j, :], mt2_sb[:, j, :], rec2hl[:],
                        start=True, stop=True,
                    )
                rc2s = w2w.tile([128, T2, 2], F32, tag="rc2s")
                nc.scalar.activation(rc2s[:], rc2ps, Copy)
                alpha2 = w2w.tile([128, T2, 1], F32, tag="alpha2")
                nc.vector.tensor_tensor(
                    alpha2[:], rc2s[:, :, 0:1], rc2s[:, :, 1:2], ADD
                )
                nc.vector.tensor_tensor(alpha2[:], alpha2[:], w2e[:], MUL)

                M1a = w2m.tile([128, T2, 128], BF16, tag="M1a")
                for j in range(T2):
                    t = c * T2 + j
                    nc.vector.scalar_tensor_tensor(
                        M1a[:, j, :], iota128_sb[:], drel2_sb[:, t:t + 1],
                        alpha2[:, j, :].broadcast_to([128, 128]),
                        EQ, MUL,
                    )
                prev2 = {"c": c, "xg2": xg2, "M1a": M1a}

            emit_l2_agg(prev2)
            emit_l2_out(prev2)
            psF.release()
            psE.release()
            w2w.release()
            w2m.release()

    nc.compile()
    return nc


# ================= pjrt execution (axon) with timing =================

_exec_cache = {}


def _run_pjrt(nc, in_maps, key):
    """Mirror of bass2jax.run_bass_via_pjrt with executable caching and
    device-side timing (warmup + timed run when BASS_GAT_TIME=1)."""
    import jax
    from jax.experimental.shard_map import shard_map
    from jax.sharding import Mesh, PartitionSpec
    from concourse import bass2jax, mybir as mb

    global LAST_EXEC_NS
    bass2jax.install_neuronx_cc_hook()

    if key not in _exec_cache:
        partition_name = (
            nc.partition_id_tensor.name if nc.partition_id_tensor else None
        )
        in_names, out_names, out_avals, zero_outs = [], [], [], []
        for alloc in nc.m.functions[0].allocations:
            if not isinstance(alloc, mb.MemoryLocationSet):
                continue
            name = alloc.memorylocations[0].name
            if alloc.kind == "ExternalInput":
                if name != partition_name:
                    in_names.append(name)
            elif alloc.kind == "ExternalOutput":
                shape = tuple(alloc.tensor_shape)
                dtype = mb.dt.np(alloc.dtype)
                out_names.append(name)
                out_avals.append(jax.core.ShapedArray(shape, dtype))
                zero_outs.append(np.zeros(shape, dtype))
        n_params = len(in_names)
        all_in_names = list(in_names) + list(out_names)
        if partition_name is not None:
            all_in_names.append(partition_name)

        def _body(*args):
            operands = list(args)
            if partition_name is not None:
                operands.append(bass2jax.partition_id_tensor())
            outs = bass2jax._bass_exec_p.bind(
                *operands,
                out_avals=tuple(out_avals),
                in_names=tuple(all_in_names),
                out_names=tuple(out_names),
                lowering_input_output_aliases=(),
                sim_require_finite=True,
                sim_require_nnan=True,
                nc=nc,
            )
            return tuple(outs)

        devices = jax.devices()[:NCORES]
        mesh = Mesh(np.asarray(devices), ("core",))
        n_outs = len(out_avals)
        sharded = jax.jit(
            shard_map(
                _body, mesh=mesh,
                in_specs=(PartitionSpec("core"),) * (n_params + n_outs),
                out_specs=(PartitionSpec("core"),) * n_outs,
                check_rep=False,
            ),
            keep_unused=True,
        )
        _exec_cache[key] = (sharded, in_names, out_names, out_avals, zero_outs)
    sharded, in_names, out_names, out_avals, zero_outs = _exec_cache[key]

    import jax
    concat_in = [
        np.concatenate([np.asarray(in_maps[c][n]) for c in range(NCORES)], axis=0)
        for n in in_names
    ]
    concat_zeros = [
        np.zeros((NCORES * z.shape[0], *z.shape[1:]), z.dtype) for z in zero_outs
    ]
    out_arrs = sharded(*concat_in, *concat_zeros)
    jax.block_until_ready(out_arrs)

    if os.environ.get("BASS_GAT_TIME", "0") == "1":
        import time as _time
        args = [jax.device_put(a) for a in concat_in + concat_zeros]
        jax.block_until_ready(args)
        reps = int(os.environ.get("BASS_GAT_REPS", "5"))
        ts = []
        for _ in range(reps):
            t0 = _time.perf_counter()
            o = sharded(*args)
            jax.block_until_ready(o)
            ts.append(_time.perf_counter() - t0)
        LAST_EXEC_NS = int(min(ts) * 1e9)

    return [
        {
            n: np.asarray(out_arrs[i]).reshape(NCORES, *out_avals[i].shape)[c]
            for i, n in enumerate(out_names)
        }
        for c in range(NCORES)
    ]

# ================= entry point =================

def kernel(**inputs) -> np.ndarray:
    inp = {k: np.asarray(v) for k, v in inputs.items()}
    for b in ("b_node", "b_col", "b1", "b2"):
        assert np.abs(inp[b]).max() == 0.0, f"nonzero {b} unsupported"
    b_out = inp["b_out"].astype(np.float32)

    meta = _prep(inp["edges"].astype(np.int64))
    T1, T2 = meta["T1"], meta["T2"]
    wts = _weights_prep(inp)

    key = (T1, T2)
    if key not in _prog_cache:
        _prog_cache[key] = _build_program(T1, T2)
    nc = _prog_cache[key]

    xn = np.tile(inp["constraints_state"].astype(np.float32), (1, 2))  # [4000,128]
    xc = np.tile(inp["columns_state"].astype(np.float32), (1, 2))      # [16000,256]

    in_maps = []
    for m in range(NCORES):
        xct = np.zeros((128, CON_CH * 128), np.float32)
        xcolt = np.zeros((128, 2, COL_CH * 128), np.float32)
        for lc, ch in enumerate(meta["chunks1"][m * CPC1:(m + 1) * CPC1]):
            if lc < CON_CH:
                cols = lc * 128 + np.arange(len(ch))
                xct[:, cols] = xn[ch].T
            else:
                cols = (lc - CON_CH) * 128 + np.arange(len(ch))
                xcv = xc[np.asarray(ch) - NC_NODES]  # [k, 256]
                xcolt[:, 0, cols] = xcv[:, 0:128].T
                xcolt[:, 1, cols] = xcv[:, 128:256].T
        idx1, dr1, m1, mT1 = meta["et1"][m]
        idx2, dr2, m2, mT2 = meta["et2"][m]
        in_maps.append(dict(
            xct=xct, xcolt=xcolt,
            wnodet=wts["wnodet"], wcolt=wts["wcolt"], v1=wts["v1"],
            w1tb=wts["w1tb"], w2tb=wts["w2tb"], att2=wts["att2"],
            woutt=wts["woutt"], iota8i=wts["iota8i"], iota128=wts["iota128"],
            esrc1=idx1, drel1=dr1, m1=m1, mT1=mT1,
            esrc2=idx2, drel2=dr2, m2=m2, mT2=mT2,
        ))

    if os.environ.get("BASS_GAT_NTFF", "0") == "1":
        import ntff_hook
        ntff_hook.install()
        import tempfile
        global LAST_EXEC_NS, LAST_RESULTS
        td = tempfile.mkdtemp(prefix="gat_trace_")
        res = bass_utils.run_bass_kernel_spmd(
            nc, in_maps, core_ids=list(range(NCORES)), trace=True, tmpdir=td,
        )
        LAST_EXEC_NS = res.exec_time_ns
        LAST_RESULTS = res
        print("trace dir:", td)
        results = res.results
    else:
        results = _run_pjrt(nc, in_maps, key)

    out = np.zeros((NCOL, EMB), np.float32)
    for m in range(NCORES):
        o = np.asarray(results[m]["out"]).astype(np.float32)
        for lc, ch in enumerate(meta["chunks2"][m * CPC2:(m + 1) * CPC2]):
            if ch:
                rows = lc * 128 + np.arange(len(ch))
                out[np.asarray(ch) - NC_NODES] = o[rows]
    return out + b_out[None, :]


# revision 43
# speedup vs baseline: 1.1561x; 1.0227x over previous
"""GAT (2-layer, PyG-style) Trainium2 kernel, 8-core SPMD.

Strategy:
  - Nodes assigned to (core, 128-chunk) slots, load-balanced by in-degree;
    L2 chunks are co-located with the L1 column chunks (same membership), so
    layer-2 dst attention values stay core-local.
  - Aggregation in x-space (256-wide) with TRANSPOSED layout: the gathered
    source features are the matmul STATIONARY operand; the moving operand is
    a per-edge alpha-scaled one-hot mask block M8[e, (head, dstpos)] built in
    ONE fused DVE op (scalar_tensor_tensor: (iota==drel)*alpha) per edge
    tile. Output lands feature-major, which is exactly the layout the W1/W2
    projections need, so no transpose roundtrip.
  - Attention: a_src rides the feature gather (f32 cols in the same 768B
    table row); a_dst is expanded per-edge with tiny matmuls against
    host-uploaded static one-hot masks (m = [e,dst], mT = [dst,e]; bf16
    hi/lo splits keep the lookups near-exact); softmax denominators via
    m-matmuls; masks are pre-normalized by 1/den (alpha), so no
    post-scaling of the aggregate.
  - Software pipelining: chunk c's attention chain (DVE/ScalarE/small PE
    lookups) runs while chunk c-1's aggregation matmuls keep the PE busy;
    the layer-1->layer-2 projections (phases 5/6) run per 512-node group
    inside the same loop with small rotating buffers.
  - Cross-core: two AllGathers of the 768B-row node tables (G1, G2).
"""
import os, sys
import numpy as np
import ml_dtypes

sys.path.insert(0, "/opt/trn_rl_repo")
import concourse.bass as bass
import concourse.mybir as mybir
import concourse.tile as tile
import concourse.bacc as bacc
from concourse import bass_utils

F32 = mybir.dt.float32
BF16 = mybir.dt.bfloat16
I16 = mybir.dt.int16
BF = ml_dtypes.bfloat16

# ---------------- problem constants ----------------
NC_NODES = 4000
NCOL = 16000
N = NC_NODES + NCOL
NF, CF = 64, 128
HID = 256
H = 8
EMB = 128
NEG = 0.2

NCORES = 8
CON_CH = 4
COL_CH = 16
CPC1 = CON_CH + COL_CH          # 20
SLOT1 = CPC1 * 128              # 2560
CPC2 = 16
SLOT2 = CPC2 * 128              # 2048
GW = 384                        # bf16 table width (768B stride); f32 view 192
                                # (dma_gather elem size must be a multiple of
                                #  256B: 384*2 = 768B)

_prog_cache = {}
LAST_EXEC_NS = None
LAST_RESULTS = None


# ================= host-side preprocessing =================

def _balance(nodes, deg, n_chunks, cap=128):
    import heapq
    order = nodes[np.argsort(-deg[nodes], kind="stable")]
    loads = np.zeros(n_chunks, dtype=np.int64)
    counts = np.zeros(n_chunks, dtype=np.int64)
    heap = [(0, c) for c in range(n_chunks)]
    heapq.heapify(heap)
    members = [[] for _ in range(n_chunks)]
    for nd in order:
        while True:
            _, c = heapq.heappop(heap)
            if counts[c] < cap:
                break
        members[c].append(int(nd))
        counts[c] += 1
        loads[c] += int(deg[nd])
        if counts[c] < cap:
            heapq.heappush(heap, (loads[c], c))
    return members, loads


def _wrap_idx(idx):
    """dma_gather int16 index layout: [128, n/16]; row p holds idx[s*16+p%16]."""
    idx = np.asarray(idx, dtype=np.int16)
    n = len(idx)
    assert n % 16 == 0
    m = idx.reshape(n // 16, 16).T
    return np.tile(m, (8, 1)).copy()


def _onehots(drel, cpc, T):
    """drel: [cpc, T*128] float (dst position in chunk, or -1 pad).
    Returns m  [128(e), cpc*T, 128(p)]  and mT [128(p), cpc*T, 128(e)]  bf16."""
    d = drel.reshape(cpc, T, 128).astype(np.int32)       # [c, j, e]
    oh = (d[:, :, :, None] == np.arange(128)[None, None, None, :])  # [c,j,e,p]
    m = np.ascontiguousarray(
        oh.transpose(2, 0, 1, 3).reshape(128, cpc * T, 128)
    ).astype(BF)
    mT = np.ascontiguousarray(
        oh.transpose(3, 0, 1, 2).reshape(128, cpc * T, 128)
    ).astype(BF)
    return m, mT


def _prep(edges):
    src1 = np.concatenate([edges[0], np.arange(N)]).astype(np.int64)
    dst1 = np.concatenate([edges[1], np.arange(N)]).astype(np.int64)
    s2 = np.concatenate([edges[1], np.arange(N)]).astype(np.int64)
    d2 = np.concatenate([edges[0], np.arange(N)]).astype(np.int64)
    keep = d2 >= NC_NODES
    src2, dst2 = s2[keep], d2[keep]

    deg1 = np.bincount(dst1, minlength=N)
    deg2 = np.bincount(dst2, minlength=N)

    con_members, con_loads = _balance(np.arange(NC_NODES), deg1, NCORES * CON_CH)
    # column chunks serve BOTH layers (L2 chunks == L1 col chunks); balance on
    # deg1 (the larger layer) and accept the resulting T2
    col_members, _ = _balance(np.arange(NC_NODES, N), deg1, NCORES * COL_CH)
    gslot1 = np.full(N, -1, dtype=np.int64)
    chunks1 = [[] for _ in range(NCORES * CPC1)]
    for g, mem in enumerate(con_members):
        core, lc = g % NCORES, g // NCORES
        chunks1[core * CPC1 + lc] = mem
    for g, mem in enumerate(col_members):
        core, lc = g % NCORES, CON_CH + g // NCORES
        chunks1[core * CPC1 + lc] = mem
    for ci, mem in enumerate(chunks1):
        core, lc = divmod(ci, CPC1)
        for pos, nd in enumerate(mem):
            gslot1[nd] = core * SLOT1 + lc * 128 + pos
    assert (gslot1 >= 0).all()

    # chunks2 = the column chunks of layer 1 (identity co-location)
    chunks2 = [
        chunks1[core * CPC1 + CON_CH + lc]
        for core in range(NCORES) for lc in range(CPC2)
    ]
    # realized per-chunk loads determine the tile counts
    def chunk_load(members_list, deg):
        return max(
            (sum(deg[nd] for nd in mem) for mem in members_list if mem),
            default=0,
        )
    T1 = max(4, int(np.ceil(max(
        chunk_load([chunks1[i] for i in range(len(chunks1))], deg1), 1
    ) / 128)))
    T2 = max(4, int(np.ceil(max(chunk_load(chunks2, deg2), 1) / 128)))

    # table row layout is half-major (for split AllGathers):
    # row = half*(NCORES*HR) + core*HR + loc%HR,  HR = SLOT1//2
    HR = SLOT1 // 2
    def row_of(g):
        core, loc = g // SLOT1, g % SLOT1
        return (loc // HR) * (NCORES * HR) + core * HR + loc % HR

    # dst slot mapping for L2: position within the L1 col-chunk
    def edge_tables(src, dst, cpc, T, chunk_of_node, pos_of_node, remap):
        """Per core: src gather idx, drel, and static one-hot masks."""
        dcore = gslot1[dst] // SLOT1
        order = np.argsort(
            dcore * (cpc * 128) + chunk_of_node[dst] * 128 + pos_of_node[dst],
            kind="stable",
        )
        so, do = src[order], dst[order]
        core_of = dcore[order]
        cm_all, pm_all = chunk_of_node[do], pos_of_node[do]
        res = []
        for mcore in range(NCORES):
            esrc = np.zeros((cpc, T * 128), dtype=np.int64)
            drel = np.full((cpc, T * 128), -1.0, dtype=np.float32)
            sel = core_of == mcore
            sm, cm, pm = so[sel], cm_all[sel], pm_all[sel]
            for lc in range(cpc):
                s = cm == lc
                k = int(s.sum())
                assert k <= T * 128, f"chunk overflow {k} > {T*128}"
                esrc[lc, :k] = remap(gslot1[sm[s]])
                drel[lc, :k] = pm[s]
            idx = _wrap_idx(esrc.reshape(-1))
            m, mT = _onehots(drel, cpc, T)
            drel_dev = np.ascontiguousarray(
                drel.reshape(cpc, T, 128).transpose(2, 0, 1).reshape(128, cpc * T)
            )
            res.append((idx, drel_dev, m, mT))
        return res

    chunk1_of = (gslot1 % SLOT1) // 128          # L1 chunk index per node
    pos_of = gslot1 % 128
    chunk2_of = chunk1_of - CON_CH               # L2 chunk index (col nodes)
    et1 = edge_tables(src1, dst1, CPC1, T1, chunk1_of, pos_of, lambda g: g)
    et2 = edge_tables(src2, dst2, CPC2, T2, chunk2_of, pos_of, lambda g: g)
    return dict(gslot1=gslot1, chunks1=chunks1, chunks2=chunks2,
                T1=T1, T2=T2, et1=et1, et2=et2)


def _weights_prep(inp):
    W1 = inp["W1"].astype(np.float32)       # [2048, 256]
    W2 = inp["W2"].astype(np.float32)       # [256, 2048]
    out = {}
    out["wnodet"] = np.ascontiguousarray(inp["W_node"].T).astype(np.float32)  # [128,256]
    wct = inp["W_col"].T.astype(np.float32)  # [256, 256]
    out["wcolt"] = np.stack([wct[0:128], wct[128:256]], axis=1)  # [128, 2, 256]
    V1 = np.zeros((256, 16), np.float32)
    for h in range(H):
        Wh = W1[h * HID:(h + 1) * HID, :]
        V1[:, h] = Wh.T @ inp["att_src1"][h]
        V1[:, 8 + h] = Wh.T @ inp["att_dst1"][h]
    out["v1"] = np.stack([V1[0:128], V1[128:256]], axis=1)       # [128, 2, 16]
    W1T = W1.T                                                   # [256, 2048]
    w1tb = np.zeros((128, 32, 128), BF)
    for h in range(H):
        for os_ in range(2):
            for fs in range(2):
                w1tb[:, h * 4 + os_ * 2 + fs, :] = W1T[
                    fs * 128:(fs + 1) * 128,
                    h * 256 + os_ * 128: h * 256 + (os_ + 1) * 128,
                ].astype(BF)
    out["w1tb"] = w1tb
    W2T = W2.T                                                   # [2048, 256]
    w2tb = np.zeros((128, 32, 128), BF)
    for f16 in range(16):
        for os_ in range(2):
            w2tb[:, f16 * 2 + os_, :] = W2T[
                f16 * 128:(f16 + 1) * 128, os_ * 128:(os_ + 1) * 128
            ].astype(BF)
    out["w2tb"] = w2tb
    a2 = np.stack([inp["att_src2"][0], inp["att_dst2"][0]], axis=1)  # [256, 2]
    out["att2"] = np.stack([a2[0:128], a2[128:256]], axis=1).astype(BF)  # [128,2,2]
    wot = inp["W_out"].T.astype(np.float32)  # [256, 128]
    out["woutt"] = np.stack([wot[0:128], wot[128:256]], axis=1).astype(BF)  # [128,2,128]
    # iota8i[e, p*8+h] = p  (interleaved one-hot comparison pattern, L1)
    out["iota8i"] = np.broadcast_to(
        (np.arange(1024) // 8).astype(BF), (128, 1024)
    ).copy()
    # iota128[e, p] = p (L2)
    out["iota128"] = np.broadcast_to(
        np.arange(128).astype(BF), (128, 128)
    ).copy()
    return out


# ================= device program =================

def _build_program(T1, T2):
    nc = bacc.Bacc(None, target_bir_lowering=False)
    NT1, NT2 = CPC1 * T1, CPC2 * T2

    xct = nc.dram_tensor("xct", [128, CON_CH * 128], F32, kind="ExternalInput")
    xcolt = nc.dram_tensor("xcolt", [128, 2, COL_CH * 128], F32, kind="ExternalInput")
    wnodet = nc.dram_tensor("wnodet", [128, 256], F32, kind="ExternalInput")
    wcolt = nc.dram_tensor("wcolt", [128, 2, 256], F32, kind="ExternalInput")
    v1 = nc.dram_tensor("v1", [128, 2, 16], F32, kind="ExternalInput")
    w1tb = nc.dram_tensor("w1tb", [128, 32, 128], BF16, kind="ExternalInput")
    w2tb = nc.dram_tensor("w2tb", [128, 32, 128], BF16, kind="ExternalInput")
    att2 = nc.dram_tensor("att2", [128, 2, 2], BF16, kind="ExternalInput")
    woutt = nc.dram_tensor("woutt", [128, 2, 128], BF16, kind="ExternalInput")
    iota8i = nc.dram_tensor("iota8i", [128, 1024], BF16, kind="ExternalInput")
    iota128 = nc.dram_tensor("iota128", [128, 128], BF16, kind="ExternalInput")
    esrc1 = nc.dram_tensor("esrc1", [128, NT1 * 8], I16, kind="ExternalInput")
    drel1 = nc.dram_tensor("drel1", [128, NT1], F32, kind="ExternalInput")
    m1d = nc.dram_tensor("m1", [128, NT1, 128], BF16, kind="ExternalInput")
    mT1d = nc.dram_tensor("mT1", [128, NT1, 128], BF16, kind="ExternalInput")
    esrc2 = nc.dram_tensor("esrc2", [128, NT2 * 8], I16, kind="ExternalInput")
    drel2 = nc.dram_tensor("drel2", [128, NT2], F32, kind="ExternalInput")
    m2d = nc.dram_tensor("m2", [128, NT2, 128], BF16, kind="ExternalInput")
    mT2d = nc.dram_tensor("mT2", [128, NT2, 128], BF16, kind="ExternalInput")
    out_dram = nc.dram_tensor("out", [SLOT2, EMB], F32, kind="ExternalOutput")

    Copy = mybir.ActivationFunctionType.Copy
    Relu = mybir.ActivationFunctionType.Relu
    Exp = mybir.ActivationFunctionType.Exp
    ADD, EQ, MUL, MAX, SUB = (
        mybir.AluOpType.add, mybir.AluOpType.is_equal,
        mybir.AluOpType.mult, mybir.AluOpType.max,
        mybir.AluOpType.subtract,
    )

    with tile.TileContext(nc) as tc:
        with (
            tc.tile_pool(name="const", bufs=1) as cpool,
            tc.tile_pool(name="sb", bufs=3) as sb,
            tc.tile_pool(name="dram", bufs=1, space="DRAM") as dram,
        ):
            def cload(t, shape, dtype):
                nm = t.name + "_sb"
                s = cpool.tile(shape, dtype, name=nm, tag=nm)
                nc.sync.dma_start(s[:], t[:])
                return s

            # phase-1-critical loads first (everything else can trickle in
            # behind them on the DMA queue)
            wnodet_sb = cload(wnodet, [128, 256], F32)
            wcolt_sb = cload(wcolt, [128, 2, 256], F32)
            v1_sb = cload(v1, [128, 2, 16], F32)

            g1_loc = dram.tile([SLOT1, GW], BF16)
            g1_full = dram.tile([NCORES * SLOT1, GW], BF16, addr_space="Shared")
            g2_loc = dram.tile([SLOT1, GW], BF16)
            g2_full = dram.tile([NCORES * SLOT1, GW], BF16, addr_space="Shared")

            ad2f = cpool.tile([128, CPC2, 1], F32, name="ad2f", tag="ad2f")
            ad2hla = cpool.tile([128, CPC2, 2], BF16, name="ad2hla", tag="ad2hla")

            # long-lived L1 pool (adhl written in phase 1, read through L1)
            l1 = tc.alloc_tile_pool(name="l1", bufs=1)
            aggnT4 = l1.tile([128, 2, 4, 8, 128], BF16, tag="aggnT4")
            adhl = l1.tile([128, CPC1, 16], BF16, tag="adhl")

            # ======== phase 1: input MLPs ========
            p1 = tc.alloc_tile_pool(name="p1", bufs=1)
            psA = tc.alloc_tile_pool(name="psA", bufs=2, space="PSUM")
            xct_sb = p1.tile([128, CON_CH * 128], F32, tag="xct_sb")
            nc.sync.dma_start(xct_sb[:], xct[:])
            xcolt_sb = p1.tile([128, 2, COL_CH * 128], F32, tag="xcolt_sb")
            nc.sync.dma_start(xcolt_sb[:], xcolt[:])
            # remaining constants (not needed until later phases)
            iota8i_sb = cload(iota8i, [128, 1024], BF16)
            iota128_sb = cload(iota128, [128, 128], BF16)
            w1tb_sb = cload(w1tb, [128, 32, 128], BF16)
            w2tb_sb = cload(w2tb, [128, 32, 128], BF16)
            att2_sb = cload(att2, [128, 2, 2], BF16)
            woutt_sb = cload(woutt, [128, 2, 128], BF16)
            idx1_sb = cload(esrc1, [128, NT1 * 8], I16)
            drel1_sb = cload(drel1, [128, NT1], F32)
            idx2_sb = cload(esrc2, [128, NT2 * 8], I16)
            drel2_sb = cload(drel2, [128, NT2], F32)
            xT = p1.tile([128, 2, SLOT1], F32, tag="xT")
            for os_ in range(2):
                p = psA.tile([128, CON_CH * 128], F32, tag="pmlp")
                nc.tensor.matmul(
                    p[:], wnodet_sb[:, os_ * 128:(os_ + 1) * 128], xct_sb[:],
                    start=True, stop=True,
                )
                nc.scalar.activation(xT[:, os_, 0:CON_CH * 128], p[:], Relu)
                for nch in range(4):
                    p2 = psA.tile([128, 512], F32, tag="pmlp2")
                    for fs in range(2):
                        nc.tensor.matmul(
                            p2[:],
                            wcolt_sb[:, fs, os_ * 128:(os_ + 1) * 128],
                            xcolt_sb[:, fs, nch * 512:(nch + 1) * 512],
                            start=(fs == 0), stop=(fs == 1),
                        )
                    nc.scalar.activation(
                        xT[:, os_, CON_CH * 128 + nch * 512: CON_CH * 128 + (nch + 1) * 512],
                        p2[:], Relu,
                    )

            # node-major x + attention values -> G1 rows; keep a_d locally
            g1sb = p1.tile([128, CPC1, GW], BF16, tag="g1sb")
            g1sb_f32 = g1sb[:].bitcast(F32)       # [128, CPC1, 192]
            pa_all = p1.tile([128, CPC1, 16], F32, tag="pa_all")
            for c in range(CPC1):
                nsl = slice(c * 128, (c + 1) * 128)
                px = psA.tile([128, 256], F32, tag="px")
                if c < CON_CH:
                    nc.tensor.matmul(
                        px[:], xct_sb[:, nsl], wnodet_sb[:], start=True, stop=True
                    )
                else:
                    ksl = slice((c - CON_CH) * 128, (c - CON_CH) * 128 + 128)
                    for fs in range(2):
                        nc.tensor.matmul(
                            px[:], xcolt_sb[:, fs, ksl], wcolt_sb[:, fs, :],
                            start=(fs == 0), stop=(fs == 1),
                        )
                pa = psA.tile([128, 16], F32, tag="pa")
                for fs in range(2):
                    nc.tensor.matmul(
                        pa[:], xT[:, fs, nsl], v1_sb[:, fs, :],
                        start=(fs == 0), stop=(fs == 1),
                    )
                nc.scalar.activation(g1sb[:, c, 0:256], px[:], Relu)
                nc.vector.tensor_copy(g1sb_f32[:, c, 128:136], pa[:, 0:8])
                nc.vector.tensor_copy(pa_all[:, c, :], pa[:])

            # a_d hi/lo split for exact bf16-matmul lookups: [128, CPC1, 16]
            nc.vector.tensor_copy(adhl[:, :, 0:8], pa_all[:, :, 8:16])
            adhif = p1.tile([128, CPC1, 8], F32, tag="adhif")
            nc.vector.tensor_copy(adhif[:], adhl[:, :, 0:8])
            nc.vector.tensor_tensor(
                adhl[:, :, 8:16], pa_all[:, :, 8:16], adhif[:], SUB
            )
            nc.sync.dma_start(
                g1_loc[:].rearrange("(c p) w -> p c w", p=128), g1sb[:]
            )

            psA.release()

            # ======== phase 2: AllGather G1 ========
            nc.gpsimd.collective_compute(
                "AllGather", mybir.AluOpType.bypass,
                ins=[g1_loc.opt()], outs=[g1_full.opt()],
                replica_groups=[list(range(NCORES))],
            )
            p1.release()
            g1f = g1_full

            # ======== phase 3+4: layer-1 edge weights + aggregation ========
            l1m = tc.alloc_tile_pool(name="l1m", bufs=3)
            l1g = tc.alloc_tile_pool(name="l1g", bufs=2)
            l1k = tc.alloc_tile_pool(name="l1k", bufs=3)
            l1w = tc.alloc_tile_pool(name="l1w", bufs=2)
            psS = tc.alloc_tile_pool(name="psS", bufs=2, space="PSUM")
            psT = tc.alloc_tile_pool(name="psT", bufs=1, space="PSUM")
            psP = tc.alloc_tile_pool(name="psP", bufs=2, space="PSUM")

            # software-pipelined over chunks: while chunk c's attention chain
            # runs on DVE/ScalarE, chunk c-1's aggregation matmuls keep the PE
            # busy (emitted into the chain's dependency gaps).

            def emit_agg_half(st, which):
                c, xg, M8a = st["c"], st["xg"], st["M8a"]
                pT = psT.tile([128, 1024], F32, tag=f"pT{which}")
                st[f"pT{which}"] = pT
                fsl = slice(which * 128, (which + 1) * 128)
                for j in range(T1):
                    M8f = M8a[:, j, :, :].rearrange("p a b -> p (a b)")
                    for half in range(2):
                        nc.tensor.matmul(
                            pT[:, half * 512:(half + 1) * 512],
                            xg[:, j, fsl], M8f[:, half * 512:(half + 1) * 512],
                            start=(j == 0), stop=(j == T1 - 1),
                        )

            def emit_drain_and_group(st):
                c = st["c"]
                nc.scalar.activation(
                    aggnT4[:, 0, c % 4, :, :].rearrange("p a b -> p (a b)"),
                    st["pT0"][:], Copy,
                )
                nc.scalar.activation(
                    aggnT4[:, 1, c % 4, :, :].rearrange("p a b -> p (a b)"),
                    st["pT1"][:], Copy,
                )
                if c % 4 != 3:
                    return
                # phases 5+6 for the completed 4-chunk group (512 nodes)
                g = c // 4
                x2Tg = l1g.tile([128, 16, 512], BF16, tag="x2Tg")
                for hh in range(16):
                    h, os_ = hh // 2, hh % 2
                    px2 = psP.tile([128, 512], F32, tag="pproj")
                    for fs in range(2):
                        rhs = aggnT4[:, fs, :, h, :]   # [128, 4, 128]
                        nc.tensor.matmul(
                            px2[:],
                            w1tb_sb[:, h * 4 + os_ * 2 + fs, :],
                            rhs,
                            start=(fs == 0), stop=(fs == 1),
                        )
                    nc.scalar.activation(x2Tg[:, hh, :], px2[:], Relu)
                h2Tg = l1g.tile([128, 2, 512], BF16, tag="h2Tg")
                for os_ in range(2):
                    ph2 = psP.tile([128, 512], F32, tag="pproj")
                    for f16 in range(16):
                        nc.tensor.matmul(
                            ph2[:], w2tb_sb[:, f16 * 2 + os_, :],
                            x2Tg[:, f16, :],
                            start=(f16 == 0), stop=(f16 == 15),
                        )
                    nc.scalar.activation(h2Tg[:, os_, :], ph2[:], Copy)
                for ci in range(4):
                    cg = g * 4 + ci
                    nsl = slice(ci * 128, (ci + 1) * 128)
                    pa2 = psP.tile([128, 2], F32, tag="pproj")
                    for fs in range(2):
                        nc.tensor.matmul(
                            pa2[:], h2Tg[:, fs, nsl], att2_sb[:, fs, :],
                            start=(fs == 0), stop=(fs == 1),
                        )
                    g2c = l1g.tile([128, 1, GW], BF16, tag="g2c")
                    g2c_f32 = g2c[:].bitcast(F32)
                    for fs in range(2):
                        nc.sync.dma_start(
                            g2c[:, 0, fs * 128:(fs + 1) * 128],
                            h2Tg[:, fs, nsl], transpose=True,
                        )
                    nc.vector.tensor_copy(g2c_f32[:, 0, 128:130], pa2[:])
                    if cg >= CON_CH:
                        nc.vector.tensor_copy(
                            ad2f[:, cg - CON_CH, :], pa2[:, 1:2]
                        )
                    nc.sync.dma_start(
                        g2_loc[:].rearrange("(c p) w -> p c w", p=128)[
                            :, cg:cg + 1, :
                        ],
                        g2c[:],
                    )

            pipe = []
            for c in range(CPC1):
                tsl = slice(c * T1, (c + 1) * T1)
                xgt = l1m.tile([128, T1, GW], BF16, tag="xg1")
                nc.gpsimd.dma_gather(
                    xgt[:], g1f[:, 0:GW],
                    idx1_sb[:, c * T1 * 8:(c + 1) * T1 * 8],
                    T1 * 128, T1 * 128, GW, elem_step=GW,
                )
                xg = xgt[:]
                xg_f32 = xg.bitcast(F32)        # [128, T1, 192]
                mt_sb = l1m.tile([128, T1, 128], BF16, tag="mt1", name="mt1s")
                nc.sync.dma_start(mt_sb[:], mT1d[:, tsl, :])
                mm_sb = l1m.tile([128, T1, 128], BF16, tag="mm1", name="mm1s")
                nc.sync.dma_start(mm_sb[:], m1d[:, tsl, :])

                # one packed PSUM bank for the small per-chunk matmul outs
                psmall = psS.tile([128, 2 * T1 * 16 + 16], F32, tag="psmall")
                adps = psmall[:, 0:T1 * 16].rearrange("p (a b) -> p a b", b=16)
                den = psmall[:, T1 * 16:T1 * 16 + 8]
                rcps = psmall[:, T1 * 16 + 16:2 * T1 * 16 + 16].rearrange(
                    "p (a b) -> p a b", b=16
                )
                for j in range(T1):
                    nc.tensor.matmul(
                        adps[:, j, :], mt_sb[:, j, :], adhl[:, c, :],
                        start=True, stop=True,
                    )
                # PE filler while chunk c's chain runs on DVE/ScalarE
                if len(pipe) == 2:
                    emit_agg_half(pipe[0], 0)
                ads = l1w.tile([128, T1, 16], F32, tag="ads")
                nc.scalar.activation(ads[:], adps, Copy)
                w1e = l1w.tile([128, T1, 8], F32, tag="w1e")
                # e = a_s + ad_hi + ad_lo
                nc.vector.tensor_tensor(
                    w1e[:], ads[:, :, 0:8], ads[:, :, 8:16], ADD
                )
                nc.vector.tensor_tensor(
                    w1e[:], w1e[:],
                    xg_f32[:, :, 128:136], ADD
                )
                nc.vector.scalar_tensor_tensor(
                    w1e[:], w1e[:], NEG, w1e[:], MUL, MAX
                )
                nc.scalar.activation(w1e[:], w1e[:], Exp)
                w1ebf = l1w.tile([128, T1, 8], BF16, tag="w1ebf")
                nc.scalar.activation(w1ebf[:], w1e[:], Copy)

                # denominators + reciprocal hi/lo
                for j in range(T1):
                    nc.tensor.matmul(
                        den[:], mm_sb[:, j, :], w1ebf[:, j, :],
                        start=(j == 0), stop=(j == T1 - 1),
                    )
                # PE filler while the reciprocal chain runs
                if len(pipe) == 2:
                    st = pipe.pop(0)
                    emit_agg_half(st, 1)
                    emit_drain_and_group(st)

                rec = l1w.tile([128, 8], F32, tag="rec")
                nc.vector.tensor_scalar_add(rec[:], den[:], 1e-16)
                nc.vector.reciprocal(rec[:], rec[:])
                rechl = l1w.tile([128, 16], BF16, tag="rechl")
                nc.vector.tensor_copy(rechl[:, 0:8], rec[:])
                rechf = l1w.tile([128, 8], F32, tag="rechf")
                nc.vector.tensor_copy(rechf[:], rechl[:, 0:8])
                nc.vector.tensor_tensor(rechl[:, 8:16], rec[:], rechf[:], SUB)

                # rec per edge + alpha
                for j in range(T1):
                    nc.tensor.matmul(
                        rcps[:, j, :], mt_sb[:, j, :], rechl[:],
                        start=True, stop=True,
                    )
                rcs = l1w.tile([128, T1, 16], F32, tag="rcs")
                nc.scalar.activation(rcs[:], rcps, Copy)
                alpha = l1w.tile([128, T1, 8], BF16, tag="alpha")
                rsum = l1w.tile([128, T1, 8], F32, tag="rsum")
                nc.vector.tensor_tensor(
                    rsum[:], rcs[:, :, 0:8], rcs[:, :, 8:16], ADD
                )
                nc.vector.tensor_tensor(alpha[:], w1e[:], rsum[:], MUL)

                # alpha-scaled interleaved masks for all tiles of this chunk
                M8a = l1k.tile([128, T1, 8, 128], BF16, tag="M8a")
                for j in range(T1):
                    t = c * T1 + j
                    nc.vector.scalar_tensor_tensor(
                        M8a[:, j, :, :],
                        iota128_sb[:].unsqueeze(1).broadcast_to([128, 8, 128]),
                        drel1_sb[:, t:t + 1],
                        alpha[:, j, :].unsqueeze(2).broadcast_to([128, 8, 128]),
                        EQ, MUL,
                    )
                pipe.append({"c": c, "xg": xg, "M8a": M8a})

            for st in pipe:
                emit_agg_half(st, 0)
                emit_agg_half(st, 1)
                emit_drain_and_group(st)

            psP.release()
            psT.release()
            psS.release()
            l1w.release()
            l1k.release()
            l1g.release()
            l1m.release()

            # ======== phase 7: AllGather G2 ========
            nc.gpsimd.collective_compute(
                "AllGather", mybir.AluOpType.bypass,
                ins=[g2_loc.opt()], outs=[g2_full.opt()],
                replica_groups=[list(range(NCORES))],
            )
            l1.release()
            g2f = g2_full

            # ======== phase 8+9: layer-2 edge weights + aggregation + W_out ========
            w2m = tc.alloc_tile_pool(name="w2m", bufs=3)
            w2w = tc.alloc_tile_pool(name="w2w", bufs=2)
            psE = tc.alloc_tile_pool(name="psE", bufs=2, space="PSUM")
            psF = tc.alloc_tile_pool(name="psF", bufs=2, space="PSUM")
            # a_d2 hi/lo from the locally saved phase-6 attention values
            nc.vector.tensor_copy(ad2hla[:, :, 0:1], ad2f[:])
            ad2hf = cpool.tile([128, CPC2, 1], F32, name="ad2hf", tag="ad2hf")
            nc.vector.tensor_copy(ad2hf[:], ad2hla[:, :, 0:1])
            nc.vector.tensor_tensor(ad2hla[:, :, 1:2], ad2f[:], ad2hf[:], SUB)
            # software-pipelined like L1: chunk c's chain runs on DVE/ScalarE
            # while chunk c-1's aggregation + output matmuls keep the PE busy
            def emit_l2_agg(st):
                xg2p, M1a = st["xg2"], st["M1a"]
                p30 = psF.tile([128, 128], F32, tag="p30")
                p31 = psF.tile([128, 128], F32, tag="p31")
                st["p30"], st["p31"] = p30, p31
                for j in range(T2):
                    nc.tensor.matmul(
                        p30[:], xg2p[:, j, 0:128], M1a[:, j, :],
                        start=(j == 0), stop=(j == T2 - 1),
                    )
                    nc.tensor.matmul(
                        p31[:], xg2p[:, j, 128:256], M1a[:, j, :],
                        start=(j == 0), stop=(j == T2 - 1),
                    )

            def emit_l2_out(st):
                c2 = st["c"]
                # x3T = relu(agg)  (feature-major: [feat, dst])
                x3T = sb.tile([128, 2, 128], BF16, tag="x3T")
                nc.scalar.activation(x3T[:, 0, :], st["p30"][:], Relu)
                nc.scalar.activation(x3T[:, 1, :], st["p31"][:], Relu)
                pout = psF.tile([128, EMB], F32, tag="pout")
                for fs in range(2):
                    nc.tensor.matmul(
                        pout[:], x3T[:, fs, :], woutt_sb[:, fs, :],
                        start=(fs == 0), stop=(fs == 1),
                    )
                osb = sb.tile([128, EMB], F32, tag="osb")
                nc.scalar.activation(osb[:], pout[:], Copy)
                nc.sync.dma_start(
                    out_dram[c2 * 128:(c2 + 1) * 128, :], osb[:]
                )

            prev2 = None
            for c in range(CPC2):
                tsl = slice(c * T2, (c + 1) * T2)
                xg2t = w2m.tile([128, T2, GW], BF16, tag="xg2")
                nc.gpsimd.dma_gather(
                    xg2t[:], g2f[:, 0:GW],
                    idx2_sb[:, c * T2 * 8:(c + 1) * T2 * 8],
                    T2 * 128, T2 * 128, GW, elem_step=GW,
                )
                xg2 = xg2t[:]
                xg2_f32 = xg2.bitcast(F32)      # [128, T2, 192]
                mt2_sb = w2m.tile([128, T2, 128], BF16, tag="mt2", name="mt2s")
                nc.sync.dma_start(mt2_sb[:], mT2d[:, tsl, :])
                mm2_sb = w2m.tile([128, T2, 128], BF16, tag="mm2", name="mm2s")
                nc.sync.dma_start(mm2_sb[:], m2d[:, tsl, :])

                # packed PSUM bank: [0:12]=a_d lookups, [12:13]=den, [16:28]=rec
                ps2 = psE.tile([128, 4 * T2 + 4], F32, tag="ps2")
                ad2ps = ps2[:, 0:T2 * 2].rearrange("p (a b) -> p a b", b=2)
                den2 = ps2[:, T2 * 2:T2 * 2 + 1]
                rc2ps = ps2[:, T2 * 2 + 2:4 * T2 + 2].rearrange(
                    "p (a b) -> p a b", b=2
                )
                for j in range(T2):
                    nc.tensor.matmul(
                        ad2ps[:, j, :], mt2_sb[:, j, :], ad2hla[:, c, :],
                        start=True, stop=True,
                    )
                # PE filler while chunk c's chain runs
                if prev2 is not None:
                    emit_l2_agg(prev2)
                ad2s = w2w.tile([128, T2, 2], F32, tag="ad2s")
                nc.scalar.activation(ad2s[:], ad2ps, Copy)
                w2e = w2w.tile([128, T2, 1], F32, tag="w2e")
                nc.vector.tensor_tensor(
                    w2e[:], ad2s[:, :, 0:1], ad2s[:, :, 1:2], ADD
                )
                nc.vector.tensor_tensor(
                    w2e[:], w2e[:], xg2_f32[:, 0:T2, 128:129], ADD
                )
                nc.vector.scalar_tensor_tensor(
                    w2e[:], w2e[:], NEG, w2e[:], MUL, MAX
                )
                nc.scalar.activation(w2e[:], w2e[:], Exp)
                w2ebf = w2w.tile([128, T2, 1], BF16, tag="w2ebf")
                nc.scalar.activation(w2ebf[:], w2e[:], Copy)

                for j in range(T2):
                    nc.tensor.matmul(
                        den2[:], mm2_sb[:, j, :], w2ebf[:, j, :],
                        start=(j == 0), stop=(j == T2 - 1),
                    )
                # PE filler while the reciprocal chain runs
                if prev2 is not None:
                    emit_l2_out(prev2)
                rec2 = w2w.tile([128, 1], F32, tag="rec2")
                nc.vector.tensor_scalar(rec2[:], den2[:], 1e-16, None, ADD)
                nc.vector.reciprocal(rec2[:], rec2[:])
                rec2hl = w2w.tile([128, 2], BF16, tag="rec2hl")
                nc.vector.tensor_copy(rec2hl[:, 0:1], rec2[:])
                rec2hf = w2w.tile([128, 1], F32, tag="rec2hf")
                nc.vector.tensor_copy(rec2hf[:], rec2hl[:, 0:1])
                nc.vector.tensor_tensor(rec2hl[:, 1:2], rec2[:], rec2hf[:], SUB)
                for j in range(T2):
                    nc.tensor.matmul(
                        rc2ps[:, j, :], mt2_sb[:, j, :], rec2hl[:],
                        start=True, stop=True,
                    )
                rc2s = w2w.tile([128, T2, 2], F32, tag="rc2s")
                nc.scalar.activation(rc2s[:], rc2ps, Copy)
                alpha2 = w2w.tile([128, T2, 1], F32, tag="alpha2")
                nc.vector.tensor_tensor(
                    alpha2[:], rc2s[:, :, 0:1], rc2s[:, :, 1:2], ADD
                )
                nc.vector.tensor_tensor(alpha2[:], alpha2[:], w2e[:], MUL)

                M1a = w2m.tile([128, T2, 128], BF16, tag="M1a")
                for j in range(T2):
                    t = c * T2 + j
                    nc.vector.scalar_tensor_tensor(
                        M1a[:, j, :], iota128_sb[:], drel2_sb[:, t:t + 1],
                        alpha2[:, j, :].broadcast_to([128, 128]),
                        EQ, MUL,
                    )
                prev2 = {"c": c, "xg2": xg2, "M1a": M1a}

            emit_l2_agg(prev2)
            emit_l2_out(prev2)
            psF.release()
            psE.release()
            w2w.release()
            w2m.release()

    nc.compile()
    return nc


# ================= pjrt execution (axon) with timing =================

_exec_cache = {}


def _run_pjrt(nc, in_maps, key):
    """Mirror of bass2jax.run_bass_via_pjrt with executable caching and
    device-side timing (warmup + timed run when BASS_GAT_TIME=1)."""
    import jax
    from jax.experimental.shard_map import shard_map
    from jax.sharding import Mesh, PartitionSpec
    from concourse import bass2jax, mybir as mb

    global LAST_EXEC_NS
    bass2jax.install_neuronx_cc_hook()

    if key not in _exec_cache:
        partition_name = (
            nc.partition_id_tensor.name if nc.partition_id_tensor else None
        )
        in_names, out_names, out_avals, zero_outs = [], [], [], []
        for alloc in nc.m.functions[0].allocations:
            if not isinstance(alloc, mb.MemoryLocationSet):
                continue
            name = alloc.memorylocations[0].name
            if alloc.kind == "ExternalInput":
                if name != partition_name:
                    in_names.append(name)
            elif alloc.kind == "ExternalOutput":
                shape = tuple(alloc.tensor_shape)
                dtype = mb.dt.np(alloc.dtype)
                out_names.append(name)
                out_avals.append(jax.core.ShapedArray(shape, dtype))
                zero_outs.append(np.zeros(shape, dtype))
        n_params = len(in_names)
        all_in_names = list(in_names) + list(out_names)
        if partition_name is not None:
            all_in_names.append(partition_name)

        def _body(*args):
            operands = list(args)
            if partition_name is not None:
                operands.append(bass2jax.partition_id_tensor())
            outs = bass2jax._bass_exec_p.bind(
                *operands,
                out_avals=tuple(out_avals),
                in_names=tuple(all_in_names),
                out_names=tuple(out_names),
                lowering_input_output_aliases=(),
                sim_require_finite=True,
                sim_require_nnan=True,
                nc=nc,
            )
            return tuple(outs)

        devices = jax.devices()[:NCORES]
        mesh = Mesh(np.asarray(devices), ("core",))
        n_outs = len(out_avals)
        sharded = jax.jit(
            shard_map(
                _body, mesh=mesh,
                in_specs=(PartitionSpec("core"),) * (n_params + n_outs),
                out_specs=(PartitionSpec("core"),) * n_outs,
                check_rep=False,
            ),
            keep_unused=True,
        )
        _exec_cache[key] = (sharded, in_names, out_names, out_avals, zero_outs)
    sharded, in_names, out_names, out_avals, zero_outs = _exec_cache[key]

    import jax
    concat_in = [
        np.concatenate([np.asarray(in_maps[c][n]) for c in range(NCORES)], axis=0)
        for n in in_names
    ]
    concat_zeros = [
        np.zeros((NCORES * z.shape[0], *z.shape[1:]), z.dtype) for z in zero_outs
    ]
    out_arrs = sharded(*concat_in, *concat_zeros)
    jax.block_until_ready(out_arrs)

    if os.environ.get("BASS_GAT_TIME", "0") == "1":
        import time as _time
        args = [jax.device_put(a) for a in concat_in + concat_zeros]
        jax.block_until_ready(args)
        reps = int(os.environ.get("BASS_GAT_REPS", "5"))
        ts = []
        for _ in range(reps):
            t0 = _time.perf_counter()
            o = sharded(*args)
            jax.block_until_ready(o)
            ts.append(_time.perf_counter() - t0)
        LAST_EXEC_NS = int(min(ts) * 1e9)

    return [
        {
            n: np.asarray(out_arrs[i]).reshape(NCORES, *out_avals[i].shape)[c]
            for i, n in enumerate(out_names)
        }
        for c in range(NCORES)
    ]

# ================= entry point =================

def kernel(**inputs) -> np.ndarray:
    inp = {k: np.asarray(v) for k, v in inputs.items()}
    for b in ("b_node", "b_col", "b1", "b2"):
        assert np.abs(inp[b]).max() == 0.0, f"nonzero {b} unsupported"
    b_out = inp["b_out"].astype(np.float32)

    meta = _prep(inp["edges"].astype(np.int64))
    T1, T2 = meta["T1"], meta["T2"]
    wts = _weights_prep(inp)

    key = (T1, T2)
    if key not in _prog_cache:
        _prog_cache[key] = _build_program(T1, T2)
    nc = _prog_cache[key]

    xn = np.tile(inp["constraints_state"].astype(np.float32), (1, 2))  # [4000,128]
    xc = np.tile(inp["columns_state"].astype(np.float32), (1, 2))      # [16000,256]

    in_maps = []
    for m in range(NCORES):
        xct = np.zeros((128, CON_CH * 128), np.float32)
        xcolt = np.zeros((128, 2, COL_CH * 128), np.float32)
        for lc, ch in enumerate(meta["chunks1"][m * CPC1:(m + 1) * CPC1]):
            if lc < CON_CH:
                cols = lc * 128 + np.arange(len(ch))
                xct[:, cols] = xn[ch].T
            else:
                cols = (lc - CON_CH) * 128 + np.arange(len(ch))
                xcv = xc[np.asarray(ch) - NC_NODES]  # [k, 256]
                xcolt[:, 0, cols] = xcv[:, 0:128].T
                xcolt[:, 1, cols] = xcv[:, 128:256].T
        idx1, dr1, m1, mT1 = meta["et1"][m]
        idx2, dr2, m2, mT2 = meta["et2"][m]
        in_maps.append(dict(
            xct=xct, xcolt=xcolt,
            wnodet=wts["wnodet"], wcolt=wts["wcolt"], v1=wts["v1"],
            w1tb=wts["w1tb"], w2tb=wts["w2tb"], att2=wts["att2"],
            woutt=wts["woutt"], iota8i=wts["iota8i"], iota128=wts["iota128"],
            esrc1=idx1, drel1=dr1, m1=m1, mT1=mT1,
            esrc2=idx2, drel2=dr2, m2=m2, mT2=mT2,
        ))

    if os.environ.get("BASS_GAT_NTFF", "0") == "1":
        import ntff_hook
        ntff_hook.install()
        import tempfile
        global LAST_EXEC_NS, LAST_RESULTS
        td = tempfile.mkdtemp(prefix="gat_trace_")
        res = bass_utils.run_bass_kernel_spmd(
            nc, in_maps, core_ids=list(range(NCORES)), trace=True, tmpdir=td,
        )
        LAST_EXEC_NS = res.exec_time_ns
        LAST_RESULTS = res
        print("trace dir:", td)
        results = res.results
    else:
        results = _run_pjrt(nc, in_maps, key)

    out = np.zeros((NCOL, EMB), np.float32)
    for m in range(NCORES):
        o = np.asarray(results[m]["out"]).astype(np.float32)
        for lc, ch in enumerate(meta["chunks2"][m * CPC2:(m + 1) * CPC2]):
            if ch:
                rows = lc * 128 + np.arange(len(ch))
                out[np.asarray(ch) - NC_NODES] = o[rows]
    return out + b_out[None, :]


# revision 44
# speedup vs baseline: 1.1915x; 1.0306x over previous
"""GAT (2-layer, PyG-style) Trainium2 kernel, 8-core SPMD.

Strategy:
  - Nodes assigned to (core, 128-chunk) slots, load-balanced by in-degree;
    L2 chunks are co-located with the L1 column chunks (same membership), so
    layer-2 dst attention values stay core-local.
  - Aggregation in x-space (256-wide) with TRANSPOSED layout: the gathered
    source features are the matmul STATIONARY operand; the moving operand is
    a per-edge alpha-scaled one-hot mask block M8[e, (head, dstpos)] built in
    ONE fused DVE op (scalar_tensor_tensor: (iota==drel)*alpha) per edge
    tile. Output lands feature-major, which is exactly the layout the W1/W2
    projections need, so no transpose roundtrip.
  - Attention: a_src rides the feature gather (f32 cols in the same 768B
    table row); a_dst is expanded per-edge with tiny matmuls against
    host-uploaded static one-hot masks (m = [e,dst], mT = [dst,e]; bf16
    hi/lo splits keep the lookups near-exact); softmax denominators via
    m-matmuls; masks are pre-normalized by 1/den (alpha), so no
    post-scaling of the aggregate.
  - Software pipelining: chunk c's attention chain (DVE/ScalarE/small PE
    lookups) runs while chunk c-1's aggregation matmuls keep the PE busy;
    the layer-1->layer-2 projections (phases 5/6) run per 512-node group
    inside the same loop with small rotating buffers.
  - Cross-core: two AllGathers of the 768B-row node tables (G1, G2).
"""
import os, sys
import numpy as np
import ml_dtypes

sys.path.insert(0, "/opt/trn_rl_repo")
import concourse.bass as bass
import concourse.mybir as mybir
import concourse.tile as tile
import concourse.bacc as bacc
from concourse import bass_utils

F32 = mybir.dt.float32
BF16 = mybir.dt.bfloat16
I16 = mybir.dt.int16
BF = ml_dtypes.bfloat16

# ---------------- problem constants ----------------
NC_NODES = 4000
NCOL = 16000
N = NC_NODES + NCOL
NF, CF = 64, 128
HID = 256
H = 8
EMB = 128
NEG = 0.2

NCORES = 8
CON_CH = 4
COL_CH = 16
CPC1 = CON_CH + COL_CH          # 20
SLOT1 = CPC1 * 128              # 2560
CPC2 = 16
SLOT2 = CPC2 * 128              # 2048
GW = 384                        # bf16 table width (768B stride); f32 view 192
                                # (dma_gather elem size must be a multiple of
                                #  256B: 384*2 = 768B)

_prog_cache = {}
LAST_EXEC_NS = None
LAST_RESULTS = None


# ================= host-side preprocessing =================

def _balance(nodes, deg, n_chunks, cap=128):
    import heapq
    order = nodes[np.argsort(-deg[nodes], kind="stable")]
    loads = np.zeros(n_chunks, dtype=np.int64)
    counts = np.zeros(n_chunks, dtype=np.int64)
    heap = [(0, c) for c in range(n_chunks)]
    heapq.heapify(heap)
    members = [[] for _ in range(n_chunks)]
    for nd in order:
        while True:
            _, c = heapq.heappop(heap)
            if counts[c] < cap:
                break
        members[c].append(int(nd))
        counts[c] += 1
        loads[c] += int(deg[nd])
        if counts[c] < cap:
            heapq.heappush(heap, (loads[c], c))
    return members, loads


def _wrap_idx(idx):
    """dma_gather int16 index layout: [128, n/16]; row p holds idx[s*16+p%16]."""
    idx = np.asarray(idx, dtype=np.int16)
    n = len(idx)
    assert n % 16 == 0
    m = idx.reshape(n // 16, 16).T
    return np.tile(m, (8, 1)).copy()


def _onehots(drel, cpc, T):
    """drel: [cpc, T*128] float (dst position in chunk, or -1 pad).
    Returns m  [128(e), cpc*T, 128(p)]  and mT [128(p), cpc*T, 128(e)]  bf16."""
    d = drel.reshape(cpc, T, 128).astype(np.int32)       # [c, j, e]
    oh = (d[:, :, :, None] == np.arange(128)[None, None, None, :])  # [c,j,e,p]
    m = np.ascontiguousarray(
        oh.transpose(2, 0, 1, 3).reshape(128, cpc * T, 128)
    ).astype(BF)
    mT = np.ascontiguousarray(
        oh.transpose(3, 0, 1, 2).reshape(128, cpc * T, 128)
    ).astype(BF)
    return m, mT


def _prep(edges):
    src1 = np.concatenate([edges[0], np.arange(N)]).astype(np.int64)
    dst1 = np.concatenate([edges[1], np.arange(N)]).astype(np.int64)
    s2 = np.concatenate([edges[1], np.arange(N)]).astype(np.int64)
    d2 = np.concatenate([edges[0], np.arange(N)]).astype(np.int64)
    keep = d2 >= NC_NODES
    src2, dst2 = s2[keep], d2[keep]

    deg1 = np.bincount(dst1, minlength=N)
    deg2 = np.bincount(dst2, minlength=N)

    con_members, con_loads = _balance(np.arange(NC_NODES), deg1, NCORES * CON_CH)
    # column chunks serve BOTH layers (L2 chunks == L1 col chunks); balance on
    # deg1 (the larger layer) and accept the resulting T2
    col_members, _ = _balance(np.arange(NC_NODES, N), deg1, NCORES * COL_CH)
    gslot1 = np.full(N, -1, dtype=np.int64)
    chunks1 = [[] for _ in range(NCORES * CPC1)]
    for g, mem in enumerate(con_members):
        core, lc = g % NCORES, g // NCORES
        chunks1[core * CPC1 + lc] = mem
    for g, mem in enumerate(col_members):
        core, lc = g % NCORES, CON_CH + g // NCORES
        chunks1[core * CPC1 + lc] = mem
    for ci, mem in enumerate(chunks1):
        core, lc = divmod(ci, CPC1)
        for pos, nd in enumerate(mem):
            gslot1[nd] = core * SLOT1 + lc * 128 + pos
    assert (gslot1 >= 0).all()

    # chunks2 = the column chunks of layer 1 (identity co-location)
    chunks2 = [
        chunks1[core * CPC1 + CON_CH + lc]
        for core in range(NCORES) for lc in range(CPC2)
    ]
    # realized per-chunk loads determine the tile counts
    def chunk_load(members_list, deg):
        return max(
            (sum(deg[nd] for nd in mem) for mem in members_list if mem),
            default=0,
        )
    T1 = max(4, int(np.ceil(max(
        chunk_load([chunks1[i] for i in range(len(chunks1))], deg1), 1
    ) / 128)))
    T2 = max(4, int(np.ceil(max(chunk_load(chunks2, deg2), 1) / 128)))

    # table row layout is half-major (for split AllGathers):
    # row = half*(NCORES*HR) + core*HR + loc%HR,  HR = SLOT1//2
    HR = SLOT1 // 2
    def row_of(g):
        core, loc = g // SLOT1, g % SLOT1
        return (loc // HR) * (NCORES * HR) + core * HR + loc % HR

    # dst slot mapping for L2: position within the L1 col-chunk
    def edge_tables(src, dst, cpc, T, chunk_of_node, pos_of_node, remap):
        """Per core: src gather idx, drel, and static one-hot masks."""
        dcore = gslot1[dst] // SLOT1
        order = np.argsort(
            dcore * (cpc * 128) + chunk_of_node[dst] * 128 + pos_of_node[dst],
            kind="stable",
        )
        so, do = src[order], dst[order]
        core_of = dcore[order]
        cm_all, pm_all = chunk_of_node[do], pos_of_node[do]
        res = []
        for mcore in range(NCORES):
            esrc = np.zeros((cpc, T * 128), dtype=np.int64)
            drel = np.full((cpc, T * 128), -1.0, dtype=np.float32)
            sel = core_of == mcore
            sm, cm, pm = so[sel], cm_all[sel], pm_all[sel]
            for lc in range(cpc):
                s = cm == lc
                k = int(s.sum())
                assert k <= T * 128, f"chunk overflow {k} > {T*128}"
                esrc[lc, :k] = remap(gslot1[sm[s]])
                drel[lc, :k] = pm[s]
            idx = _wrap_idx(esrc.reshape(-1))
            m, mT = _onehots(drel, cpc, T)
            drel_dev = np.ascontiguousarray(
                drel.reshape(cpc, T, 128).transpose(2, 0, 1).reshape(128, cpc * T)
            )
            res.append((idx, drel_dev, m, mT))
        return res

    chunk1_of = (gslot1 % SLOT1) // 128          # L1 chunk index per node
    pos_of = gslot1 % 128
    chunk2_of = chunk1_of - CON_CH               # L2 chunk index (col nodes)
    et1 = edge_tables(src1, dst1, CPC1, T1, chunk1_of, pos_of, lambda g: g)
    et2 = edge_tables(src2, dst2, CPC2, T2, chunk2_of, pos_of, lambda g: g)
    return dict(gslot1=gslot1, chunks1=chunks1, chunks2=chunks2,
                T1=T1, T2=T2, et1=et1, et2=et2)


def _weights_prep(inp):
    W1 = inp["W1"].astype(np.float32)       # [2048, 256]
    W2 = inp["W2"].astype(np.float32)       # [256, 2048]
    out = {}
    out["wnodet"] = np.ascontiguousarray(inp["W_node"].T).astype(np.float32)  # [128,256]
    wct = inp["W_col"].T.astype(np.float32)  # [256, 256]
    out["wcolt"] = np.stack([wct[0:128], wct[128:256]], axis=1)  # [128, 2, 256]
    V1 = np.zeros((256, 16), np.float32)
    for h in range(H):
        Wh = W1[h * HID:(h + 1) * HID, :]
        V1[:, h] = Wh.T @ inp["att_src1"][h]
        V1[:, 8 + h] = Wh.T @ inp["att_dst1"][h]
    out["v1"] = np.stack([V1[0:128], V1[128:256]], axis=1)       # [128, 2, 16]
    W1T = W1.T                                                   # [256, 2048]
    w1tb = np.zeros((128, 32, 128), BF)
    for h in range(H):
        for os_ in range(2):
            for fs in range(2):
                w1tb[:, h * 4 + os_ * 2 + fs, :] = W1T[
                    fs * 128:(fs + 1) * 128,
                    h * 256 + os_ * 128: h * 256 + (os_ + 1) * 128,
                ].astype(BF)
    out["w1tb"] = w1tb
    W2T = W2.T                                                   # [2048, 256]
    w2tb = np.zeros((128, 32, 128), BF)
    for f16 in range(16):
        for os_ in range(2):
            w2tb[:, f16 * 2 + os_, :] = W2T[
                f16 * 128:(f16 + 1) * 128, os_ * 128:(os_ + 1) * 128
            ].astype(BF)
    out["w2tb"] = w2tb
    a2 = np.stack([inp["att_src2"][0], inp["att_dst2"][0]], axis=1)  # [256, 2]
    out["att2"] = np.stack([a2[0:128], a2[128:256]], axis=1).astype(BF)  # [128,2,2]
    wot = inp["W_out"].T.astype(np.float32)  # [256, 128]
    out["woutt"] = np.stack([wot[0:128], wot[128:256]], axis=1).astype(BF)  # [128,2,128]
    # iota8i[e, p*8+h] = p  (interleaved one-hot comparison pattern, L1)
    out["iota8i"] = np.broadcast_to(
        (np.arange(1024) // 8).astype(BF), (128, 1024)
    ).copy()
    # iota128[e, p] = p (L2)
    out["iota128"] = np.broadcast_to(
        np.arange(128).astype(BF), (128, 128)
    ).copy()
    return out


# ================= device program =================

def _build_program(T1, T2):
    nc = bacc.Bacc(None, target_bir_lowering=False)
    NT1, NT2 = CPC1 * T1, CPC2 * T2

    xct = nc.dram_tensor("xct", [128, CON_CH * 128], F32, kind="ExternalInput")
    xcolt = nc.dram_tensor("xcolt", [128, 2, COL_CH * 128], F32, kind="ExternalInput")
    wnodet = nc.dram_tensor("wnodet", [128, 256], F32, kind="ExternalInput")
    wcolt = nc.dram_tensor("wcolt", [128, 2, 256], F32, kind="ExternalInput")
    v1 = nc.dram_tensor("v1", [128, 2, 16], F32, kind="ExternalInput")
    w1tb = nc.dram_tensor("w1tb", [128, 32, 128], BF16, kind="ExternalInput")
    w2tb = nc.dram_tensor("w2tb", [128, 32, 128], BF16, kind="ExternalInput")
    att2 = nc.dram_tensor("att2", [128, 2, 2], BF16, kind="ExternalInput")
    woutt = nc.dram_tensor("woutt", [128, 2, 128], BF16, kind="ExternalInput")
    iota8i = nc.dram_tensor("iota8i", [128, 1024], BF16, kind="ExternalInput")
    iota128 = nc.dram_tensor("iota128", [128, 128], BF16, kind="ExternalInput")
    esrc1 = nc.dram_tensor("esrc1", [128, NT1 * 8], I16, kind="ExternalInput")
    drel1 = nc.dram_tensor("drel1", [128, NT1], F32, kind="ExternalInput")
    m1d = nc.dram_tensor("m1", [128, NT1, 128], BF16, kind="ExternalInput")
    mT1d = nc.dram_tensor("mT1", [128, NT1, 128], BF16, kind="ExternalInput")
    esrc2 = nc.dram_tensor("esrc2", [128, NT2 * 8], I16, kind="ExternalInput")
    drel2 = nc.dram_tensor("drel2", [128, NT2], F32, kind="ExternalInput")
    m2d = nc.dram_tensor("m2", [128, NT2, 128], BF16, kind="ExternalInput")
    mT2d = nc.dram_tensor("mT2", [128, NT2, 128], BF16, kind="ExternalInput")
    out_dram = nc.dram_tensor("out", [SLOT2, EMB], F32, kind="ExternalOutput")

    Copy = mybir.ActivationFunctionType.Copy
    Relu = mybir.ActivationFunctionType.Relu
    Exp = mybir.ActivationFunctionType.Exp
    ADD, EQ, MUL, MAX, SUB = (
        mybir.AluOpType.add, mybir.AluOpType.is_equal,
        mybir.AluOpType.mult, mybir.AluOpType.max,
        mybir.AluOpType.subtract,
    )

    with tile.TileContext(nc) as tc:
        with (
            tc.tile_pool(name="const", bufs=1) as cpool,
            tc.tile_pool(name="sb", bufs=3) as sb,
            tc.tile_pool(name="dram", bufs=1, space="DRAM") as dram,
        ):
            def cload(t, shape, dtype):
                nm = t.name + "_sb"
                s = cpool.tile(shape, dtype, name=nm, tag=nm)
                nc.sync.dma_start(s[:], t[:])
                return s

            # phase-1-critical loads first (everything else can trickle in
            # behind them on the DMA queue)
            wnodet_sb = cload(wnodet, [128, 256], F32)
            wcolt_sb = cload(wcolt, [128, 2, 256], F32)
            v1_sb = cload(v1, [128, 2, 16], F32)

            g1_loc = dram.tile([SLOT1, GW], BF16)
            g1_full = dram.tile([NCORES * SLOT1, GW], BF16, addr_space="Shared")
            g2_loc = dram.tile([SLOT1, GW], BF16)
            g2_full = dram.tile([NCORES * SLOT1, GW], BF16, addr_space="Shared")

            ad2f = cpool.tile([128, CPC2, 1], F32, name="ad2f", tag="ad2f")
            ad2hla = cpool.tile([128, CPC2, 2], BF16, name="ad2hla", tag="ad2hla")

            # long-lived L1 pool (adhl written in phase 1, read through L1)
            l1 = tc.alloc_tile_pool(name="l1", bufs=1)
            aggnT4 = l1.tile([128, 2, 4, 8, 128], BF16, tag="aggnT4")
            adhl = l1.tile([128, CPC1, 16], BF16, tag="adhl")

            # ======== phase 1: input MLPs ========
            p1 = tc.alloc_tile_pool(name="p1", bufs=1)
            psA = tc.alloc_tile_pool(name="psA", bufs=2, space="PSUM")
            xct_sb = p1.tile([128, CON_CH * 128], F32, tag="xct_sb")
            nc.sync.dma_start(xct_sb[:], xct[:])
            xcolt_sb = p1.tile([128, 2, COL_CH * 128], F32, tag="xcolt_sb")
            nc.sync.dma_start(xcolt_sb[:], xcolt[:])
            # remaining constants (not needed until later phases)
            iota8i_sb = cload(iota8i, [128, 1024], BF16)
            iota128_sb = cload(iota128, [128, 128], BF16)
            w1tb_sb = cload(w1tb, [128, 32, 128], BF16)
            w2tb_sb = cload(w2tb, [128, 32, 128], BF16)
            att2_sb = cload(att2, [128, 2, 2], BF16)
            woutt_sb = cload(woutt, [128, 2, 128], BF16)
            idx1_sb = cload(esrc1, [128, NT1 * 8], I16)
            drel1_sb = cload(drel1, [128, NT1], F32)
            idx2_sb = cload(esrc2, [128, NT2 * 8], I16)
            drel2_sb = cload(drel2, [128, NT2], F32)
            xT = p1.tile([128, 2, SLOT1], F32, tag="xT")
            for os_ in range(2):
                p = psA.tile([128, CON_CH * 128], F32, tag="pmlp")
                nc.tensor.matmul(
                    p[:], wnodet_sb[:, os_ * 128:(os_ + 1) * 128], xct_sb[:],
                    start=True, stop=True,
                )
                nc.scalar.activation(xT[:, os_, 0:CON_CH * 128], p[:], Relu)
                for nch in range(4):
                    p2 = psA.tile([128, 512], F32, tag="pmlp2")
                    for fs in range(2):
                        nc.tensor.matmul(
                            p2[:],
                            wcolt_sb[:, fs, os_ * 128:(os_ + 1) * 128],
                            xcolt_sb[:, fs, nch * 512:(nch + 1) * 512],
                            start=(fs == 0), stop=(fs == 1),
                        )
                    nc.scalar.activation(
                        xT[:, os_, CON_CH * 128 + nch * 512: CON_CH * 128 + (nch + 1) * 512],
                        p2[:], Relu,
                    )

            # node-major x + attention values -> G1 rows; keep a_d locally
            g1sb = p1.tile([128, CPC1, GW], BF16, tag="g1sb")
            g1sb_f32 = g1sb[:].bitcast(F32)       # [128, CPC1, 192]
            pa_all = p1.tile([128, CPC1, 16], F32, tag="pa_all")
            for c in range(CPC1):
                nsl = slice(c * 128, (c + 1) * 128)
                px = psA.tile([128, 256], F32, tag="px")
                if c < CON_CH:
                    nc.tensor.matmul(
                        px[:], xct_sb[:, nsl], wnodet_sb[:], start=True, stop=True
                    )
                else:
                    ksl = slice((c - CON_CH) * 128, (c - CON_CH) * 128 + 128)
                    for fs in range(2):
                        nc.tensor.matmul(
                            px[:], xcolt_sb[:, fs, ksl], wcolt_sb[:, fs, :],
                            start=(fs == 0), stop=(fs == 1),
                        )
                pa = psA.tile([128, 16], F32, tag="pa")
                for fs in range(2):
                    nc.tensor.matmul(
                        pa[:], xT[:, fs, nsl], v1_sb[:, fs, :],
                        start=(fs == 0), stop=(fs == 1),
                    )
                nc.scalar.activation(g1sb[:, c, 0:256], px[:], Relu)
                nc.vector.tensor_copy(g1sb_f32[:, c, 128:136], pa[:, 0:8])
                nc.vector.tensor_copy(pa_all[:, c, :], pa[:])

            # a_d hi/lo split for exact bf16-matmul lookups: [128, CPC1, 16]
            nc.vector.tensor_copy(adhl[:, :, 0:8], pa_all[:, :, 8:16])
            adhif = p1.tile([128, CPC1, 8], F32, tag="adhif")
            nc.vector.tensor_copy(adhif[:], adhl[:, :, 0:8])
            nc.vector.tensor_tensor(
                adhl[:, :, 8:16], pa_all[:, :, 8:16], adhif[:], SUB
            )
            nc.sync.dma_start(
                g1_loc[:].rearrange("(c p) w -> p c w", p=128), g1sb[:]
            )

            psA.release()

            # ======== phase 2: AllGather G1 ========
            nc.gpsimd.collective_compute(
                "AllGather", mybir.AluOpType.bypass,
                ins=[g1_loc.opt()], outs=[g1_full.opt()],
                replica_groups=[list(range(NCORES))],
            )
            p1.release()
            g1f = g1_full

            # ======== phase 3+4: layer-1 edge weights + aggregation ========
            l1m = tc.alloc_tile_pool(name="l1m", bufs=4)
            l1g = tc.alloc_tile_pool(name="l1g", bufs=2)
            l1k = tc.alloc_tile_pool(name="l1k", bufs=3)
            l1w = tc.alloc_tile_pool(name="l1w", bufs=3)
            psS = tc.alloc_tile_pool(name="psS", bufs=2, space="PSUM")
            psT = tc.alloc_tile_pool(name="psT", bufs=1, space="PSUM")
            psP = tc.alloc_tile_pool(name="psP", bufs=2, space="PSUM")

            # software-pipelined over chunks: while chunk c's attention chain
            # runs on DVE/ScalarE, chunk c-1's aggregation matmuls keep the PE
            # busy (emitted into the chain's dependency gaps).

            def emit_agg_half(st, which):
                c, xg, M8a = st["c"], st["xg"], st["M8a"]
                pT = psT.tile([128, 1024], F32, tag=f"pT{which}")
                st[f"pT{which}"] = pT
                fsl = slice(which * 128, (which + 1) * 128)
                for j in range(T1):
                    M8f = M8a[:, j, :, :].rearrange("p a b -> p (a b)")
                    for half in range(2):
                        nc.tensor.matmul(
                            pT[:, half * 512:(half + 1) * 512],
                            xg[:, j, fsl], M8f[:, half * 512:(half + 1) * 512],
                            start=(j == 0), stop=(j == T1 - 1),
                        )

            def emit_drain_and_group(st):
                c = st["c"]
                nc.scalar.activation(
                    aggnT4[:, 0, c % 4, :, :].rearrange("p a b -> p (a b)"),
                    st["pT0"][:], Copy,
                )
                nc.scalar.activation(
                    aggnT4[:, 1, c % 4, :, :].rearrange("p a b -> p (a b)"),
                    st["pT1"][:], Copy,
                )
                if c % 4 != 3:
                    return
                # phases 5+6 for the completed 4-chunk group (512 nodes)
                g = c // 4
                x2Tg = l1g.tile([128, 16, 512], BF16, tag="x2Tg")
                for hh in range(16):
                    h, os_ = hh // 2, hh % 2
                    px2 = psP.tile([128, 512], F32, tag="pproj")
                    for fs in range(2):
                        rhs = aggnT4[:, fs, :, h, :]   # [128, 4, 128]
                        nc.tensor.matmul(
                            px2[:],
                            w1tb_sb[:, h * 4 + os_ * 2 + fs, :],
                            rhs,
                            start=(fs == 0), stop=(fs == 1),
                        )
                    nc.scalar.activation(x2Tg[:, hh, :], px2[:], Relu)
                h2Tg = l1g.tile([128, 2, 512], BF16, tag="h2Tg")
                for os_ in range(2):
                    ph2 = psP.tile([128, 512], F32, tag="pproj")
                    for f16 in range(16):
                        nc.tensor.matmul(
                            ph2[:], w2tb_sb[:, f16 * 2 + os_, :],
                            x2Tg[:, f16, :],
                            start=(f16 == 0), stop=(f16 == 15),
                        )
                    nc.scalar.activation(h2Tg[:, os_, :], ph2[:], Copy)
                for ci in range(4):
                    cg = g * 4 + ci
                    nsl = slice(ci * 128, (ci + 1) * 128)
                    pa2 = psP.tile([128, 2], F32, tag="pproj")
                    for fs in range(2):
                        nc.tensor.matmul(
                            pa2[:], h2Tg[:, fs, nsl], att2_sb[:, fs, :],
                            start=(fs == 0), stop=(fs == 1),
                        )
                    g2c = l1g.tile([128, 1, GW], BF16, tag="g2c")
                    g2c_f32 = g2c[:].bitcast(F32)
                    for fs in range(2):
                        nc.sync.dma_start(
                            g2c[:, 0, fs * 128:(fs + 1) * 128],
                            h2Tg[:, fs, nsl], transpose=True,
                        )
                    nc.vector.tensor_copy(g2c_f32[:, 0, 128:130], pa2[:])
                    if cg >= CON_CH:
                        nc.vector.tensor_copy(
                            ad2f[:, cg - CON_CH, :], pa2[:, 1:2]
                        )
                    nc.sync.dma_start(
                        g2_loc[:].rearrange("(c p) w -> p c w", p=128)[
                            :, cg:cg + 1, :
                        ],
                        g2c[:],
                    )

            pipe = []
            for c in range(CPC1):
                tsl = slice(c * T1, (c + 1) * T1)
                xgt = l1m.tile([128, T1, GW], BF16, tag="xg1")
                nc.gpsimd.dma_gather(
                    xgt[:], g1f[:, 0:GW],
                    idx1_sb[:, c * T1 * 8:(c + 1) * T1 * 8],
                    T1 * 128, T1 * 128, GW, elem_step=GW,
                )
                xg = xgt[:]
                xg_f32 = xg.bitcast(F32)        # [128, T1, 192]
                mt_sb = l1m.tile([128, T1, 128], BF16, tag="mt1", name="mt1s")
                nc.sync.dma_start(mt_sb[:], mT1d[:, tsl, :])
                mm_sb = l1m.tile([128, T1, 128], BF16, tag="mm1", name="mm1s")
                nc.sync.dma_start(mm_sb[:], m1d[:, tsl, :])

                # one packed PSUM bank for the small per-chunk matmul outs
                psmall = psS.tile([128, 2 * T1 * 16 + 16], F32, tag="psmall")
                adps = psmall[:, 0:T1 * 16].rearrange("p (a b) -> p a b", b=16)
                den = psmall[:, T1 * 16:T1 * 16 + 8]
                rcps = psmall[:, T1 * 16 + 16:2 * T1 * 16 + 16].rearrange(
                    "p (a b) -> p a b", b=16
                )
                for j in range(T1):
                    nc.tensor.matmul(
                        adps[:, j, :], mt_sb[:, j, :], adhl[:, c, :],
                        start=True, stop=True,
                    )
                # PE filler while chunk c's chain runs on DVE/ScalarE
                if len(pipe) == 2:
                    emit_agg_half(pipe[0], 0)
                ads = l1w.tile([128, T1, 16], F32, tag="ads")
                nc.scalar.activation(ads[:], adps, Copy)
                w1e = l1w.tile([128, T1, 8], F32, tag="w1e")
                # e = a_s + ad_hi + ad_lo
                nc.vector.tensor_tensor(
                    w1e[:], ads[:, :, 0:8], ads[:, :, 8:16], ADD
                )
                nc.vector.tensor_tensor(
                    w1e[:], w1e[:],
                    xg_f32[:, :, 128:136], ADD
                )
                nc.vector.scalar_tensor_tensor(
                    w1e[:], w1e[:], NEG, w1e[:], MUL, MAX
                )
                nc.scalar.activation(w1e[:], w1e[:], Exp)
                w1ebf = l1w.tile([128, T1, 8], BF16, tag="w1ebf")
                nc.scalar.activation(w1ebf[:], w1e[:], Copy)

                # denominators + reciprocal hi/lo
                for j in range(T1):
                    nc.tensor.matmul(
                        den[:], mm_sb[:, j, :], w1ebf[:, j, :],
                        start=(j == 0), stop=(j == T1 - 1),
                    )
                # PE filler while the reciprocal chain runs
                if len(pipe) == 2:
                    st = pipe.pop(0)
                    emit_agg_half(st, 1)
                    emit_drain_and_group(st)

                rec = l1w.tile([128, 8], F32, tag="rec")
                nc.vector.tensor_scalar_add(rec[:], den[:], 1e-16)
                nc.vector.reciprocal(rec[:], rec[:])
                rechl = l1w.tile([128, 16], BF16, tag="rechl")
                nc.vector.tensor_copy(rechl[:, 0:8], rec[:])
                rechf = l1w.tile([128, 8], F32, tag="rechf")
                nc.vector.tensor_copy(rechf[:], rechl[:, 0:8])
                nc.vector.tensor_tensor(rechl[:, 8:16], rec[:], rechf[:], SUB)

                # rec per edge + alpha
                for j in range(T1):
                    nc.tensor.matmul(
                        rcps[:, j, :], mt_sb[:, j, :], rechl[:],
                        start=True, stop=True,
                    )
                rcs = l1w.tile([128, T1, 16], F32, tag="rcs")
                nc.scalar.activation(rcs[:], rcps, Copy)
                alpha = l1w.tile([128, T1, 8], BF16, tag="alpha")
                rsum = l1w.tile([128, T1, 8], F32, tag="rsum")
                nc.vector.tensor_tensor(
                    rsum[:], rcs[:, :, 0:8], rcs[:, :, 8:16], ADD
                )
                nc.vector.tensor_tensor(alpha[:], w1e[:], rsum[:], MUL)

                # alpha-scaled interleaved masks for all tiles of this chunk
                M8a = l1k.tile([128, T1, 8, 128], BF16, tag="M8a")
                for j in range(T1):
                    t = c * T1 + j
                    nc.vector.scalar_tensor_tensor(
                        M8a[:, j, :, :],
                        iota128_sb[:].unsqueeze(1).broadcast_to([128, 8, 128]),
                        drel1_sb[:, t:t + 1],
                        alpha[:, j, :].unsqueeze(2).broadcast_to([128, 8, 128]),
                        EQ, MUL,
                    )
                pipe.append({"c": c, "xg": xg, "M8a": M8a})

            for st in pipe:
                emit_agg_half(st, 0)
                emit_agg_half(st, 1)
                emit_drain_and_group(st)

            psP.release()
            psT.release()
            psS.release()
            l1w.release()
            l1k.release()
            l1g.release()
            l1m.release()

            # ======== phase 7: AllGather G2 ========
            nc.gpsimd.collective_compute(
                "AllGather", mybir.AluOpType.bypass,
                ins=[g2_loc.opt()], outs=[g2_full.opt()],
                replica_groups=[list(range(NCORES))],
            )
            l1.release()
            g2f = g2_full

            # ======== phase 8+9: layer-2 edge weights + aggregation + W_out ========
            w2m = tc.alloc_tile_pool(name="w2m", bufs=4)
            w2w = tc.alloc_tile_pool(name="w2w", bufs=2)
            psE = tc.alloc_tile_pool(name="psE", bufs=2, space="PSUM")
            psF = tc.alloc_tile_pool(name="psF", bufs=2, space="PSUM")
            # a_d2 hi/lo from the locally saved phase-6 attention values
            nc.vector.tensor_copy(ad2hla[:, :, 0:1], ad2f[:])
            ad2hf = cpool.tile([128, CPC2, 1], F32, name="ad2hf", tag="ad2hf")
            nc.vector.tensor_copy(ad2hf[:], ad2hla[:, :, 0:1])
            nc.vector.tensor_tensor(ad2hla[:, :, 1:2], ad2f[:], ad2hf[:], SUB)
            # software-pipelined like L1: chunk c's chain runs on DVE/ScalarE
            # while chunk c-1's aggregation + output matmuls keep the PE busy
            def emit_l2_agg(st):
                xg2p, M1a = st["xg2"], st["M1a"]
                p30 = psF.tile([128, 128], F32, tag="p30")
                p31 = psF.tile([128, 128], F32, tag="p31")
                st["p30"], st["p31"] = p30, p31
                for j in range(T2):
                    nc.tensor.matmul(
                        p30[:], xg2p[:, j, 0:128], M1a[:, j, :],
                        start=(j == 0), stop=(j == T2 - 1),
                    )
                    nc.tensor.matmul(
                        p31[:], xg2p[:, j, 128:256], M1a[:, j, :],
                        start=(j == 0), stop=(j == T2 - 1),
                    )

            def emit_l2_out(st):
                c2 = st["c"]
                # x3T = relu(agg)  (feature-major: [feat, dst])
                x3T = sb.tile([128, 2, 128], BF16, tag="x3T")
                nc.scalar.activation(x3T[:, 0, :], st["p30"][:], Relu)
                nc.scalar.activation(x3T[:, 1, :], st["p31"][:], Relu)
                pout = psF.tile([128, EMB], F32, tag="pout")
                for fs in range(2):
                    nc.tensor.matmul(
                        pout[:], x3T[:, fs, :], woutt_sb[:, fs, :],
                        start=(fs == 0), stop=(fs == 1),
                    )
                osb = sb.tile([128, EMB], F32, tag="osb")
                nc.scalar.activation(osb[:], pout[:], Copy)
                nc.sync.dma_start(
                    out_dram[c2 * 128:(c2 + 1) * 128, :], osb[:]
                )

            prev2 = None
            for c in range(CPC2):
                tsl = slice(c * T2, (c + 1) * T2)
                xg2t = w2m.tile([128, T2, GW], BF16, tag="xg2")
                nc.gpsimd.dma_gather(
                    xg2t[:], g2f[:, 0:GW],
                    idx2_sb[:, c * T2 * 8:(c + 1) * T2 * 8],
                    T2 * 128, T2 * 128, GW, elem_step=GW,
                )
                xg2 = xg2t[:]
                xg2_f32 = xg2.bitcast(F32)      # [128, T2, 192]
                mt2_sb = w2m.tile([128, T2, 128], BF16, tag="mt2", name="mt2s")
                nc.sync.dma_start(mt2_sb[:], mT2d[:, tsl, :])
                mm2_sb = w2m.tile([128, T2, 128], BF16, tag="mm2", name="mm2s")
                nc.sync.dma_start(mm2_sb[:], m2d[:, tsl, :])

                # packed PSUM bank: [0:12]=a_d lookups, [12:13]=den, [16:28]=rec
                ps2 = psE.tile([128, 4 * T2 + 4], F32, tag="ps2")
                ad2ps = ps2[:, 0:T2 * 2].rearrange("p (a b) -> p a b", b=2)
                den2 = ps2[:, T2 * 2:T2 * 2 + 1]
                rc2ps = ps2[:, T2 * 2 + 2:4 * T2 + 2].rearrange(
                    "p (a b) -> p a b", b=2
                )
                for j in range(T2):
                    nc.tensor.matmul(
                        ad2ps[:, j, :], mt2_sb[:, j, :], ad2hla[:, c, :],
                        start=True, stop=True,
                    )
                # PE filler while chunk c's chain runs
                if prev2 is not None:
                    emit_l2_agg(prev2)
                ad2s = w2w.tile([128, T2, 2], F32, tag="ad2s")
                nc.scalar.activation(ad2s[:], ad2ps, Copy)
                w2e = w2w.tile([128, T2, 1], F32, tag="w2e")
                nc.vector.tensor_tensor(
                    w2e[:], ad2s[:, :, 0:1], ad2s[:, :, 1:2], ADD
                )
                nc.vector.tensor_tensor(
                    w2e[:], w2e[:], xg2_f32[:, 0:T2, 128:129], ADD
                )
                nc.vector.scalar_tensor_tensor(
                    w2e[:], w2e[:], NEG, w2e[:], MUL, MAX
                )
                nc.scalar.activation(w2e[:], w2e[:], Exp)
                w2ebf = w2w.tile([128, T2, 1], BF16, tag="w2ebf")
                nc.scalar.activation(w2ebf[:], w2e[:], Copy)

                for j in range(T2):
                    nc.tensor.matmul(
                        den2[:], mm2_sb[:, j, :], w2ebf[:, j, :],
                        start=(j == 0), stop=(j == T2 - 1),
                    )
                # PE filler while the reciprocal chain runs
                if prev2 is not None:
                    emit_l2_out(prev2)
                rec2 = w2w.tile([128, 1], F32, tag="rec2")
                nc.vector.tensor_scalar(rec2[:], den2[:], 1e-16, None, ADD)
                nc.vector.reciprocal(rec2[:], rec2[:])
                rec2hl = w2w.tile([128, 2], BF16, tag="rec2hl")
                nc.vector.tensor_copy(rec2hl[:, 0:1], rec2[:])
                rec2hf = w2w.tile([128, 1], F32, tag="rec2hf")
                nc.vector.tensor_copy(rec2hf[:], rec2hl[:, 0:1])
                nc.vector.tensor_tensor(rec2hl[:, 1:2], rec2[:], rec2hf[:], SUB)
                for j in range(T2):
                    nc.tensor.matmul(
                        rc2ps[:, j, :], mt2_sb[:, j, :], rec2hl[:],
                        start=True, stop=True,
                    )
                rc2s = w2w.tile([128, T2, 2], F32, tag="rc2s")
                nc.scalar.activation(rc2s[:], rc2ps, Copy)
                alpha2 = w2w.tile([128, T2, 1], F32, tag="alpha2")
                nc.vector.tensor_tensor(
                    alpha2[:], rc2s[:, :, 0:1], rc2s[:, :, 1:2], ADD
                )
                nc.vector.tensor_tensor(alpha2[:], alpha2[:], w2e[:], MUL)

                M1a = w2m.tile([128, T2, 128], BF16, tag="M1a")
                for j in range(T2):
                    t = c * T2 + j
                    nc.vector.scalar_tensor_tensor(
                        M1a[:, j, :], iota128_sb[:], drel2_sb[:, t:t + 1],
                        alpha2[:, j, :].broadcast_to([128, 128]),
                        EQ, MUL,
                    )
                prev2 = {"c": c, "xg2": xg2, "M1a": M1a}

            emit_l2_agg(prev2)
            emit_l2_out(prev2)
            psF.release()
            psE.release()
            w2w.release()
            w2m.release()

    nc.compile()
    return nc


# ================= pjrt execution (axon) with timing =================

_exec_cache = {}


def _run_pjrt(nc, in_maps, key):
    """Mirror of bass2jax.run_bass_via_pjrt with executable caching and
    device-side timing (warmup + timed run when BASS_GAT_TIME=1)."""
    import jax
    from jax.experimental.shard_map import shard_map
    from jax.sharding import Mesh, PartitionSpec
    from concourse import bass2jax, mybir as mb

    global LAST_EXEC_NS
    bass2jax.install_neuronx_cc_hook()

    if key not in _exec_cache:
        partition_name = (
            nc.partition_id_tensor.name if nc.partition_id_tensor else None
        )
        in_names, out_names, out_avals, zero_outs = [], [], [], []
        for alloc in nc.m.functions[0].allocations:
            if not isinstance(alloc, mb.MemoryLocationSet):
                continue
            name = alloc.memorylocations[0].name
            if alloc.kind == "ExternalInput":
                if name != partition_name:
                    in_names.append(name)
            elif alloc.kind == "ExternalOutput":
                shape = tuple(alloc.tensor_shape)
                dtype = mb.dt.np(alloc.dtype)
                out_names.append(name)
                out_avals.append(jax.core.ShapedArray(shape, dtype))
                zero_outs.append(np.zeros(shape, dtype))
        n_params = len(in_names)
        all_in_names = list(in_names) + list(out_names)
        if partition_name is not None:
            all_in_names.append(partition_name)

        def _body(*args):
            operands = list(args)
            if partition_name is not None:
                operands.append(bass2jax.partition_id_tensor())
            outs = bass2jax._bass_exec_p.bind(
                *operands,
                out_avals=tuple(out_avals),
                in_names=tuple(all_in_names),
                out_names=tuple(out_names),
                lowering_input_output_aliases=(),
                sim_require_finite=True,
                sim_require_nnan=True,
                nc=nc,
            )
            return tuple(outs)

        devices = jax.devices()[:NCORES]
        mesh = Mesh(np.asarray(devices), ("core",))
        n_outs = len(out_avals)
        sharded = jax.jit(
            shard_map(
                _body, mesh=mesh,
                in_specs=(PartitionSpec("core"),) * (n_params + n_outs),
                out_specs=(PartitionSpec("core"),) * n_outs,
                check_rep=False,
            ),
            keep_unused=True,
        )
        _exec_cache[key] = (sharded, in_names, out_names, out_avals, zero_outs)
    sharded, in_names, out_names, out_avals, zero_outs = _exec_cache[key]

    import jax
    concat_in = [
        np.concatenate([np.asarray(in_maps[c][n]) for c in range(NCORES)], axis=0)
        for n in in_names
    ]
    concat_zeros = [
        np.zeros((NCORES * z.shape[0], *z.shape[1:]), z.dtype) for z in zero_outs
    ]
    out_arrs = sharded(*concat_in, *concat_zeros)
    jax.block_until_ready(out_arrs)

    if os.environ.get("BASS_GAT_TIME", "0") == "1":
        import time as _time
        args = [jax.device_put(a) for a in concat_in + concat_zeros]
        jax.block_until_ready(args)
        reps = int(os.environ.get("BASS_GAT_REPS", "5"))
        ts = []
        for _ in range(reps):
            t0 = _time.perf_counter()
            o = sharded(*args)
            jax.block_until_ready(o)
            ts.append(_time.perf_counter() - t0)
        LAST_EXEC_NS = int(min(ts) * 1e9)

    return [
        {
            n: np.asarray(out_arrs[i]).reshape(NCORES, *out_avals[i].shape)[c]
            for i, n in enumerate(out_names)
        }
        for c in range(NCORES)
    ]

# ================= entry point =================

def kernel(**inputs) -> np.ndarray:
    inp = {k: np.asarray(v) for k, v in inputs.items()}
    for b in ("b_node", "b_col", "b1", "b2"):
        assert np.abs(inp[b]).max() == 0.0, f"nonzero {b} unsupported"
    b_out = inp["b_out"].astype(np.float32)

    meta = _prep(inp["edges"].astype(np.int64))
    T1, T2 = meta["T1"], meta["T2"]
    wts = _weights_prep(inp)

    key = (T1, T2)
    if key not in _prog_cache:
        _prog_cache[key] = _build_program(T1, T2)
    nc = _prog_cache[key]

    xn = np.tile(inp["constraints_state"].astype(np.float32), (1, 2))  # [4000,128]
    xc = np.tile(inp["columns_state"].astype(np.float32), (1, 2))      # [16000,256]

    in_maps = []
    for m in range(NCORES):
        xct = np.zeros((128, CON_CH * 128), np.float32)
        xcolt = np.zeros((128, 2, COL_CH * 128), np.float32)
        for lc, ch in enumerate(meta["chunks1"][m * CPC1:(m + 1) * CPC1]):
            if lc < CON_CH:
                cols = lc * 128 + np.arange(len(ch))
                xct[:, cols] = xn[ch].T
            else:
                cols = (lc - CON_CH) * 128 + np.arange(len(ch))
                xcv = xc[np.asarray(ch) - NC_NODES]  # [k, 256]
                xcolt[:, 0, cols] = xcv[:, 0:128].T
                xcolt[:, 1, cols] = xcv[:, 128:256].T
        idx1, dr1, m1, mT1 = meta["et1"][m]
        idx2, dr2, m2, mT2 = meta["et2"][m]
        in_maps.append(dict(
            xct=xct, xcolt=xcolt,
            wnodet=wts["wnodet"], wcolt=wts["wcolt"], v1=wts["v1"],
            w1tb=wts["w1tb"], w2tb=wts["w2tb"], att2=wts["att2"],
            woutt=wts["woutt"], iota8i=wts["iota8i"], iota128=wts["iota128"],
            esrc1=idx1, drel1=dr1, m1=m1, mT1=mT1,
            esrc2=idx2, drel2=dr2, m2=m2, mT2=mT2,
        ))

    if os.environ.get("BASS_GAT_NTFF", "0") == "1":
        import ntff_hook
        ntff_hook.install()
        import tempfile
        global LAST_EXEC_NS, LAST_RESULTS
        td = tempfile.mkdtemp(prefix="gat_trace_")
        res = bass_utils.run_bass_kernel_spmd(
            nc, in_maps, core_ids=list(range(NCORES)), trace=True, tmpdir=td,
        )
        LAST_EXEC_NS = res.exec_time_ns
        LAST_RESULTS = res
        print("trace dir:", td)
        results = res.results
    else:
        results = _run_pjrt(nc, in_maps, key)

    out = np.zeros((NCOL, EMB), np.float32)
    for m in range(NCORES):
        o = np.asarray(results[m]["out"]).astype(np.float32)
        for lc, ch in enumerate(meta["chunks2"][m * CPC2:(m + 1) * CPC2]):
            if ch:
                rows = lc * 128 + np.arange(len(ch))
                out[np.asarray(ch) - NC_NODES] = o[rows]
    return out + b_out[None, :]


# revision 45
# speedup vs baseline: 1.2119x; 1.0171x over previous
"""GAT (2-layer, PyG-style) Trainium2 kernel, 8-core SPMD.

Strategy:
  - Nodes assigned to (core, 128-chunk) slots, load-balanced by in-degree;
    L2 chunks are co-located with the L1 column chunks (same membership), so
    layer-2 dst attention values stay core-local.
  - Aggregation in x-space (256-wide) with TRANSPOSED layout: the gathered
    source features are the matmul STATIONARY operand; the moving operand is
    a per-edge alpha-scaled one-hot mask block M8[e, (head, dstpos)] built in
    ONE fused DVE op (scalar_tensor_tensor: (iota==drel)*alpha) per edge
    tile. Output lands feature-major, which is exactly the layout the W1/W2
    projections need, so no transpose roundtrip.
  - Attention: a_src rides the feature gather (f32 cols in the same 768B
    table row); a_dst is expanded per-edge with tiny matmuls against
    host-uploaded static one-hot masks (m = [e,dst], mT = [dst,e]; bf16
    hi/lo splits keep the lookups near-exact); softmax denominators via
    m-matmuls; masks are pre-normalized by 1/den (alpha), so no
    post-scaling of the aggregate.
  - Software pipelining: chunk c's attention chain (DVE/ScalarE/small PE
    lookups) runs while chunk c-1's aggregation matmuls keep the PE busy;
    the layer-1->layer-2 projections (phases 5/6) run per 512-node group
    inside the same loop with small rotating buffers.
  - Cross-core: two AllGathers of the 768B-row node tables (G1, G2).
"""
import os, sys
import numpy as np
import ml_dtypes

sys.path.insert(0, "/opt/trn_rl_repo")
import concourse.bass as bass
import concourse.mybir as mybir
import concourse.tile as tile
import concourse.bacc as bacc
from concourse import bass_utils

F32 = mybir.dt.float32
BF16 = mybir.dt.bfloat16
I16 = mybir.dt.int16
BF = ml_dtypes.bfloat16

# ---------------- problem constants ----------------
NC_NODES = 4000
NCOL = 16000
N = NC_NODES + NCOL
NF, CF = 64, 128
HID = 256
H = 8
EMB = 128
NEG = 0.2

NCORES = 8
CON_CH = 4
COL_CH = 16
CPC1 = CON_CH + COL_CH          # 20
SLOT1 = CPC1 * 128              # 2560
CPC2 = 16
SLOT2 = CPC2 * 128              # 2048
GW = 384                        # bf16 table width (768B stride); f32 view 192
                                # (dma_gather elem size must be a multiple of
                                #  256B: 384*2 = 768B)

_prog_cache = {}
LAST_EXEC_NS = None
LAST_RESULTS = None


# ================= host-side preprocessing =================

def _balance(nodes, deg, n_chunks, cap=128):
    import heapq
    order = nodes[np.argsort(-deg[nodes], kind="stable")]
    loads = np.zeros(n_chunks, dtype=np.int64)
    counts = np.zeros(n_chunks, dtype=np.int64)
    heap = [(0, c) for c in range(n_chunks)]
    heapq.heapify(heap)
    members = [[] for _ in range(n_chunks)]
    for nd in order:
        while True:
            _, c = heapq.heappop(heap)
            if counts[c] < cap:
                break
        members[c].append(int(nd))
        counts[c] += 1
        loads[c] += int(deg[nd])
        if counts[c] < cap:
            heapq.heappush(heap, (loads[c], c))
    return members, loads


def _wrap_idx(idx):
    """dma_gather int16 index layout: [128, n/16]; row p holds idx[s*16+p%16]."""
    idx = np.asarray(idx, dtype=np.int16)
    n = len(idx)
    assert n % 16 == 0
    m = idx.reshape(n // 16, 16).T
    return np.tile(m, (8, 1)).copy()


def _onehots(drel, cpc, T):
    """drel: [cpc, T*128] float (dst position in chunk, or -1 pad).
    Returns m  [128(e), cpc*T, 128(p)]  and mT [128(p), cpc*T, 128(e)]  bf16."""
    d = drel.reshape(cpc, T, 128).astype(np.int32)       # [c, j, e]
    oh = (d[:, :, :, None] == np.arange(128)[None, None, None, :])  # [c,j,e,p]
    m = np.ascontiguousarray(
        oh.transpose(2, 0, 1, 3).reshape(128, cpc * T, 128)
    ).astype(BF)
    mT = np.ascontiguousarray(
        oh.transpose(3, 0, 1, 2).reshape(128, cpc * T, 128)
    ).astype(BF)
    return m, mT


def _prep(edges):
    src1 = np.concatenate([edges[0], np.arange(N)]).astype(np.int64)
    dst1 = np.concatenate([edges[1], np.arange(N)]).astype(np.int64)
    s2 = np.concatenate([edges[1], np.arange(N)]).astype(np.int64)
    d2 = np.concatenate([edges[0], np.arange(N)]).astype(np.int64)
    keep = d2 >= NC_NODES
    src2, dst2 = s2[keep], d2[keep]

    deg1 = np.bincount(dst1, minlength=N)
    deg2 = np.bincount(dst2, minlength=N)

    con_members, con_loads = _balance(np.arange(NC_NODES), deg1, NCORES * CON_CH)
    # column chunks serve BOTH layers (L2 chunks == L1 col chunks); balance on
    # deg1 (the larger layer) and accept the resulting T2
    col_members, _ = _balance(np.arange(NC_NODES, N), deg1, NCORES * COL_CH)
    gslot1 = np.full(N, -1, dtype=np.int64)
    chunks1 = [[] for _ in range(NCORES * CPC1)]
    for g, mem in enumerate(con_members):
        core, lc = g % NCORES, g // NCORES
        chunks1[core * CPC1 + lc] = mem
    for g, mem in enumerate(col_members):
        core, lc = g % NCORES, CON_CH + g // NCORES
        chunks1[core * CPC1 + lc] = mem
    for ci, mem in enumerate(chunks1):
        core, lc = divmod(ci, CPC1)
        for pos, nd in enumerate(mem):
            gslot1[nd] = core * SLOT1 + lc * 128 + pos
    assert (gslot1 >= 0).all()

    # chunks2 = the column chunks of layer 1 (identity co-location)
    chunks2 = [
        chunks1[core * CPC1 + CON_CH + lc]
        for core in range(NCORES) for lc in range(CPC2)
    ]
    # realized per-chunk loads determine the tile counts
    def chunk_load(members_list, deg):
        return max(
            (sum(deg[nd] for nd in mem) for mem in members_list if mem),
            default=0,
        )
    T1 = max(4, int(np.ceil(max(
        chunk_load([chunks1[i] for i in range(len(chunks1))], deg1), 1
    ) / 128)))
    T2 = max(4, int(np.ceil(max(chunk_load(chunks2, deg2), 1) / 128)))

    # table row layout is half-major (for split AllGathers):
    # row = half*(NCORES*HR) + core*HR + loc%HR,  HR = SLOT1//2
    HR = SLOT1 // 2
    def row_of(g):
        core, loc = g // SLOT1, g % SLOT1
        return (loc // HR) * (NCORES * HR) + core * HR + loc % HR

    # dst slot mapping for L2: position within the L1 col-chunk
    def edge_tables(src, dst, cpc, T, chunk_of_node, pos_of_node, remap):
        """Per core: src gather idx, drel, and static one-hot masks."""
        dcore = gslot1[dst] // SLOT1
        order = np.argsort(
            dcore * (cpc * 128) + chunk_of_node[dst] * 128 + pos_of_node[dst],
            kind="stable",
        )
        so, do = src[order], dst[order]
        core_of = dcore[order]
        cm_all, pm_all = chunk_of_node[do], pos_of_node[do]
        res = []
        for mcore in range(NCORES):
            esrc = np.zeros((cpc, T * 128), dtype=np.int64)
            drel = np.full((cpc, T * 128), -1.0, dtype=np.float32)
            sel = core_of == mcore
            sm, cm, pm = so[sel], cm_all[sel], pm_all[sel]
            for lc in range(cpc):
                s = cm == lc
                k = int(s.sum())
                assert k <= T * 128, f"chunk overflow {k} > {T*128}"
                esrc[lc, :k] = remap(gslot1[sm[s]])
                drel[lc, :k] = pm[s]
            idx = _wrap_idx(esrc.reshape(-1))
            m, mT = _onehots(drel, cpc, T)
            drel_dev = np.ascontiguousarray(
                drel.reshape(cpc, T, 128).transpose(2, 0, 1).reshape(128, cpc * T)
            )
            res.append((idx, drel_dev, m, mT))
        return res

    chunk1_of = (gslot1 % SLOT1) // 128          # L1 chunk index per node
    pos_of = gslot1 % 128
    chunk2_of = chunk1_of - CON_CH               # L2 chunk index (col nodes)
    et1 = edge_tables(src1, dst1, CPC1, T1, chunk1_of, pos_of, lambda g: g)
    et2 = edge_tables(src2, dst2, CPC2, T2, chunk2_of, pos_of, lambda g: g)
    return dict(gslot1=gslot1, chunks1=chunks1, chunks2=chunks2,
                T1=T1, T2=T2, et1=et1, et2=et2)


def _weights_prep(inp):
    W1 = inp["W1"].astype(np.float32)       # [2048, 256]
    W2 = inp["W2"].astype(np.float32)       # [256, 2048]
    out = {}
    out["wnodet"] = np.ascontiguousarray(inp["W_node"].T).astype(np.float32)  # [128,256]
    wct = inp["W_col"].T.astype(np.float32)  # [256, 256]
    out["wcolt"] = np.stack([wct[0:128], wct[128:256]], axis=1)  # [128, 2, 256]
    V1 = np.zeros((256, 16), np.float32)
    for h in range(H):
        Wh = W1[h * HID:(h + 1) * HID, :]
        V1[:, h] = Wh.T @ inp["att_src1"][h]
        V1[:, 8 + h] = Wh.T @ inp["att_dst1"][h]
    out["v1"] = np.stack([V1[0:128], V1[128:256]], axis=1)       # [128, 2, 16]
    W1T = W1.T                                                   # [256, 2048]
    w1tb = np.zeros((128, 32, 128), BF)
    for h in range(H):
        for os_ in range(2):
            for fs in range(2):
                w1tb[:, h * 4 + os_ * 2 + fs, :] = W1T[
                    fs * 128:(fs + 1) * 128,
                    h * 256 + os_ * 128: h * 256 + (os_ + 1) * 128,
                ].astype(BF)
    out["w1tb"] = w1tb
    W2T = W2.T                                                   # [2048, 256]
    w2tb = np.zeros((128, 32, 128), BF)
    for f16 in range(16):
        for os_ in range(2):
            w2tb[:, f16 * 2 + os_, :] = W2T[
                f16 * 128:(f16 + 1) * 128, os_ * 128:(os_ + 1) * 128
            ].astype(BF)
    out["w2tb"] = w2tb
    a2 = np.stack([inp["att_src2"][0], inp["att_dst2"][0]], axis=1)  # [256, 2]
    out["att2"] = np.stack([a2[0:128], a2[128:256]], axis=1).astype(BF)  # [128,2,2]
    wot = inp["W_out"].T.astype(np.float32)  # [256, 128]
    out["woutt"] = np.stack([wot[0:128], wot[128:256]], axis=1).astype(BF)  # [128,2,128]
    # iota8i[e, p*8+h] = p  (interleaved one-hot comparison pattern, L1)
    out["iota8i"] = np.broadcast_to(
        (np.arange(1024) // 8).astype(BF), (128, 1024)
    ).copy()
    # iota128[e, p] = p (L2)
    out["iota128"] = np.broadcast_to(
        np.arange(128).astype(BF), (128, 128)
    ).copy()
    return out


# ================= device program =================

def _build_program(T1, T2):
    nc = bacc.Bacc(None, target_bir_lowering=False)
    NT1, NT2 = CPC1 * T1, CPC2 * T2

    xct = nc.dram_tensor("xct", [128, CON_CH * 128], F32, kind="ExternalInput")
    xcolt = nc.dram_tensor("xcolt", [128, 2, COL_CH * 128], F32, kind="ExternalInput")
    wnodet = nc.dram_tensor("wnodet", [128, 256], F32, kind="ExternalInput")
    wcolt = nc.dram_tensor("wcolt", [128, 2, 256], F32, kind="ExternalInput")
    v1 = nc.dram_tensor("v1", [128, 2, 16], F32, kind="ExternalInput")
    w1tb = nc.dram_tensor("w1tb", [128, 32, 128], BF16, kind="ExternalInput")
    w2tb = nc.dram_tensor("w2tb", [128, 32, 128], BF16, kind="ExternalInput")
    att2 = nc.dram_tensor("att2", [128, 2, 2], BF16, kind="ExternalInput")
    woutt = nc.dram_tensor("woutt", [128, 2, 128], BF16, kind="ExternalInput")
    iota8i = nc.dram_tensor("iota8i", [128, 1024], BF16, kind="ExternalInput")
    iota128 = nc.dram_tensor("iota128", [128, 128], BF16, kind="ExternalInput")
    esrc1 = nc.dram_tensor("esrc1", [128, NT1 * 8], I16, kind="ExternalInput")
    drel1 = nc.dram_tensor("drel1", [128, NT1], F32, kind="ExternalInput")
    m1d = nc.dram_tensor("m1", [128, NT1, 128], BF16, kind="ExternalInput")
    mT1d = nc.dram_tensor("mT1", [128, NT1, 128], BF16, kind="ExternalInput")
    esrc2 = nc.dram_tensor("esrc2", [128, NT2 * 8], I16, kind="ExternalInput")
    drel2 = nc.dram_tensor("drel2", [128, NT2], F32, kind="ExternalInput")
    m2d = nc.dram_tensor("m2", [128, NT2, 128], BF16, kind="ExternalInput")
    mT2d = nc.dram_tensor("mT2", [128, NT2, 128], BF16, kind="ExternalInput")
    out_dram = nc.dram_tensor("out", [SLOT2, EMB], F32, kind="ExternalOutput")

    Copy = mybir.ActivationFunctionType.Copy
    Relu = mybir.ActivationFunctionType.Relu
    Exp = mybir.ActivationFunctionType.Exp
    ADD, EQ, MUL, MAX, SUB = (
        mybir.AluOpType.add, mybir.AluOpType.is_equal,
        mybir.AluOpType.mult, mybir.AluOpType.max,
        mybir.AluOpType.subtract,
    )

    with tile.TileContext(nc) as tc:
        with (
            tc.tile_pool(name="const", bufs=1) as cpool,
            tc.tile_pool(name="sb", bufs=3) as sb,
            tc.tile_pool(name="dram", bufs=1, space="DRAM") as dram,
        ):
            def cload(t, shape, dtype):
                nm = t.name + "_sb"
                s = cpool.tile(shape, dtype, name=nm, tag=nm)
                nc.sync.dma_start(s[:], t[:])
                return s

            # phase-1-critical loads first (everything else can trickle in
            # behind them on the DMA queue)
            wnodet_sb = cload(wnodet, [128, 256], F32)
            wcolt_sb = cload(wcolt, [128, 2, 256], F32)
            v1_sb = cload(v1, [128, 2, 16], F32)

            g1_loc = dram.tile([SLOT1, GW], BF16)
            g1_full = dram.tile([NCORES * SLOT1, GW], BF16, addr_space="Shared")
            g2_loc = dram.tile([SLOT1, GW], BF16)
            g2_full = dram.tile([NCORES * SLOT1, GW], BF16, addr_space="Shared")

            ad2f = cpool.tile([128, CPC2, 1], F32, name="ad2f", tag="ad2f")
            ad2hla = cpool.tile([128, CPC2, 2], BF16, name="ad2hla", tag="ad2hla")

            # long-lived L1 pool (adhl written in phase 1, read through L1)
            l1 = tc.alloc_tile_pool(name="l1", bufs=1)
            aggnT4 = l1.tile([128, 2, 4, 8, 128], BF16, tag="aggnT4")
            adhl = l1.tile([128, CPC1, 16], BF16, tag="adhl")

            # ======== phase 1: input MLPs ========
            p1 = tc.alloc_tile_pool(name="p1", bufs=1)
            psA = tc.alloc_tile_pool(name="psA", bufs=2, space="PSUM")
            xct_sb = p1.tile([128, CON_CH * 128], F32, tag="xct_sb")
            nc.sync.dma_start(xct_sb[:], xct[:])
            xcolt_sb = p1.tile([128, 2, COL_CH * 128], F32, tag="xcolt_sb")
            nc.sync.dma_start(xcolt_sb[:], xcolt[:])
            # remaining constants (not needed until later phases)
            iota8i_sb = cload(iota8i, [128, 1024], BF16)
            iota128_sb = cload(iota128, [128, 128], BF16)
            w1tb_sb = cload(w1tb, [128, 32, 128], BF16)
            w2tb_sb = cload(w2tb, [128, 32, 128], BF16)
            att2_sb = cload(att2, [128, 2, 2], BF16)
            woutt_sb = cload(woutt, [128, 2, 128], BF16)
            idx1_sb = cload(esrc1, [128, NT1 * 8], I16)
            drel1_sb = cload(drel1, [128, NT1], F32)
            idx2_sb = cload(esrc2, [128, NT2 * 8], I16)
            drel2_sb = cload(drel2, [128, NT2], F32)
            xT = p1.tile([128, 2, SLOT1], F32, tag="xT")
            for os_ in range(2):
                p = psA.tile([128, CON_CH * 128], F32, tag="pmlp")
                nc.tensor.matmul(
                    p[:], wnodet_sb[:, os_ * 128:(os_ + 1) * 128], xct_sb[:],
                    start=True, stop=True,
                )
                nc.scalar.activation(xT[:, os_, 0:CON_CH * 128], p[:], Relu)
                for nch in range(4):
                    p2 = psA.tile([128, 512], F32, tag="pmlp2")
                    for fs in range(2):
                        nc.tensor.matmul(
                            p2[:],
                            wcolt_sb[:, fs, os_ * 128:(os_ + 1) * 128],
                            xcolt_sb[:, fs, nch * 512:(nch + 1) * 512],
                            start=(fs == 0), stop=(fs == 1),
                        )
                    nc.scalar.activation(
                        xT[:, os_, CON_CH * 128 + nch * 512: CON_CH * 128 + (nch + 1) * 512],
                        p2[:], Relu,
                    )

            # node-major x + attention values -> G1 rows; keep a_d locally
            g1sb = p1.tile([128, CPC1, GW], BF16, tag="g1sb")
            g1sb_f32 = g1sb[:].bitcast(F32)       # [128, CPC1, 192]
            pa_all = p1.tile([128, CPC1, 16], F32, tag="pa_all")
            for c in range(CPC1):
                nsl = slice(c * 128, (c + 1) * 128)
                px = psA.tile([128, 256], F32, tag="px")
                if c < CON_CH:
                    nc.tensor.matmul(
                        px[:], xct_sb[:, nsl], wnodet_sb[:], start=True, stop=True
                    )
                else:
                    ksl = slice((c - CON_CH) * 128, (c - CON_CH) * 128 + 128)
                    for fs in range(2):
                        nc.tensor.matmul(
                            px[:], xcolt_sb[:, fs, ksl], wcolt_sb[:, fs, :],
                            start=(fs == 0), stop=(fs == 1),
                        )
                pa = psA.tile([128, 16], F32, tag="pa")
                for fs in range(2):
                    nc.tensor.matmul(
                        pa[:], xT[:, fs, nsl], v1_sb[:, fs, :],
                        start=(fs == 0), stop=(fs == 1),
                    )
                nc.scalar.activation(g1sb[:, c, 0:256], px[:], Relu)
                nc.vector.tensor_copy(g1sb_f32[:, c, 128:136], pa[:, 0:8])
                nc.vector.tensor_copy(pa_all[:, c, :], pa[:])

            # a_d hi/lo split for exact bf16-matmul lookups: [128, CPC1, 16]
            nc.vector.tensor_copy(adhl[:, :, 0:8], pa_all[:, :, 8:16])
            adhif = p1.tile([128, CPC1, 8], F32, tag="adhif")
            nc.vector.tensor_copy(adhif[:], adhl[:, :, 0:8])
            nc.vector.tensor_tensor(
                adhl[:, :, 8:16], pa_all[:, :, 8:16], adhif[:], SUB
            )
            nc.sync.dma_start(
                g1_loc[:].rearrange("(c p) w -> p c w", p=128), g1sb[:]
            )

            psA.release()

            # ======== phase 2: AllGather G1 ========
            nc.gpsimd.collective_compute(
                "AllGather", mybir.AluOpType.bypass,
                ins=[g1_loc.opt()], outs=[g1_full.opt()],
                replica_groups=[list(range(NCORES))],
            )
            p1.release()
            g1f = g1_full

            # ======== phase 3+4: layer-1 edge weights + aggregation ========
            l1m = tc.alloc_tile_pool(name="l1m", bufs=4)
            l1g = tc.alloc_tile_pool(name="l1g", bufs=2)
            l1k = tc.alloc_tile_pool(name="l1k", bufs=4)
            l1w = tc.alloc_tile_pool(name="l1w", bufs=3)
            psS = tc.alloc_tile_pool(name="psS", bufs=2, space="PSUM")
            psT = tc.alloc_tile_pool(name="psT", bufs=1, space="PSUM")
            psP = tc.alloc_tile_pool(name="psP", bufs=2, space="PSUM")

            # software-pipelined over chunks: while chunk c's attention chain
            # runs on DVE/ScalarE, chunk c-1's aggregation matmuls keep the PE
            # busy (emitted into the chain's dependency gaps).

            def emit_agg_half(st, which):
                c, xg, M8a = st["c"], st["xg"], st["M8a"]
                pT = psT.tile([128, 1024], F32, tag=f"pT{which}")
                st[f"pT{which}"] = pT
                fsl = slice(which * 128, (which + 1) * 128)
                for j in range(T1):
                    M8f = M8a[:, j, :, :].rearrange("p a b -> p (a b)")
                    for half in range(2):
                        nc.tensor.matmul(
                            pT[:, half * 512:(half + 1) * 512],
                            xg[:, j, fsl], M8f[:, half * 512:(half + 1) * 512],
                            start=(j == 0), stop=(j == T1 - 1),
                        )

            def emit_drain_and_group(st):
                c = st["c"]
                nc.scalar.activation(
                    aggnT4[:, 0, c % 4, :, :].rearrange("p a b -> p (a b)"),
                    st["pT0"][:], Copy,
                )
                nc.scalar.activation(
                    aggnT4[:, 1, c % 4, :, :].rearrange("p a b -> p (a b)"),
                    st["pT1"][:], Copy,
                )
                if c % 4 != 3:
                    return
                # phases 5+6 for the completed 4-chunk group (512 nodes)
                g = c // 4
                x2Tg = l1g.tile([128, 16, 512], BF16, tag="x2Tg")
                for hh in range(16):
                    h, os_ = hh // 2, hh % 2
                    px2 = psP.tile([128, 512], F32, tag="pproj")
                    for fs in range(2):
                        rhs = aggnT4[:, fs, :, h, :]   # [128, 4, 128]
                        nc.tensor.matmul(
                            px2[:],
                            w1tb_sb[:, h * 4 + os_ * 2 + fs, :],
                            rhs,
                            start=(fs == 0), stop=(fs == 1),
                        )
                    nc.scalar.activation(x2Tg[:, hh, :], px2[:], Relu)
                h2Tg = l1g.tile([128, 2, 512], BF16, tag="h2Tg")
                for os_ in range(2):
                    ph2 = psP.tile([128, 512], F32, tag="pproj")
                    for f16 in range(16):
                        nc.tensor.matmul(
                            ph2[:], w2tb_sb[:, f16 * 2 + os_, :],
                            x2Tg[:, f16, :],
                            start=(f16 == 0), stop=(f16 == 15),
                        )
                    nc.scalar.activation(h2Tg[:, os_, :], ph2[:], Copy)
                for ci in range(4):
                    cg = g * 4 + ci
                    nsl = slice(ci * 128, (ci + 1) * 128)
                    pa2 = psP.tile([128, 2], F32, tag="pproj")
                    for fs in range(2):
                        nc.tensor.matmul(
                            pa2[:], h2Tg[:, fs, nsl], att2_sb[:, fs, :],
                            start=(fs == 0), stop=(fs == 1),
                        )
                    g2c = l1g.tile([128, 1, GW], BF16, tag="g2c")
                    g2c_f32 = g2c[:].bitcast(F32)
                    for fs in range(2):
                        nc.sync.dma_start(
                            g2c[:, 0, fs * 128:(fs + 1) * 128],
                            h2Tg[:, fs, nsl], transpose=True,
                        )
                    nc.vector.tensor_copy(g2c_f32[:, 0, 128:130], pa2[:])
                    if cg >= CON_CH:
                        nc.vector.tensor_copy(
                            ad2f[:, cg - CON_CH, :], pa2[:, 1:2]
                        )
                    nc.sync.dma_start(
                        g2_loc[:].rearrange("(c p) w -> p c w", p=128)[
                            :, cg:cg + 1, :
                        ],
                        g2c[:],
                    )

            pipe = []
            for c in range(CPC1):
                tsl = slice(c * T1, (c + 1) * T1)
                xgt = l1m.tile([128, T1, GW], BF16, tag="xg1")
                nc.gpsimd.dma_gather(
                    xgt[:], g1f[:, 0:GW],
                    idx1_sb[:, c * T1 * 8:(c + 1) * T1 * 8],
                    T1 * 128, T1 * 128, GW, elem_step=GW,
                )
                xg = xgt[:]
                xg_f32 = xg.bitcast(F32)        # [128, T1, 192]
                mt_sb = l1m.tile([128, T1, 128], BF16, tag="mt1", name="mt1s")
                nc.sync.dma_start(mt_sb[:], mT1d[:, tsl, :])
                mm_sb = l1m.tile([128, T1, 128], BF16, tag="mm1", name="mm1s")
                nc.sync.dma_start(mm_sb[:], m1d[:, tsl, :])

                # one packed PSUM bank for the small per-chunk matmul outs
                psmall = psS.tile([128, 2 * T1 * 16 + 16], F32, tag="psmall")
                adps = psmall[:, 0:T1 * 16].rearrange("p (a b) -> p a b", b=16)
                den = psmall[:, T1 * 16:T1 * 16 + 8]
                rcps = psmall[:, T1 * 16 + 16:2 * T1 * 16 + 16].rearrange(
                    "p (a b) -> p a b", b=16
                )
                for j in range(T1):
                    nc.tensor.matmul(
                        adps[:, j, :], mt_sb[:, j, :], adhl[:, c, :],
                        start=True, stop=True,
                    )
                # PE filler while chunk c's chain runs on DVE/ScalarE
                if len(pipe) == 3:
                    emit_agg_half(pipe[0], 0)
                ads = l1w.tile([128, T1, 16], F32, tag="ads")
                nc.scalar.activation(ads[:], adps, Copy)
                w1e = l1w.tile([128, T1, 8], F32, tag="w1e")
                # e = a_s + ad_hi + ad_lo
                nc.vector.tensor_tensor(
                    w1e[:], ads[:, :, 0:8], ads[:, :, 8:16], ADD
                )
                nc.vector.tensor_tensor(
                    w1e[:], w1e[:],
                    xg_f32[:, :, 128:136], ADD
                )
                nc.vector.scalar_tensor_tensor(
                    w1e[:], w1e[:], NEG, w1e[:], MUL, MAX
                )
                nc.scalar.activation(w1e[:], w1e[:], Exp)
                w1ebf = l1w.tile([128, T1, 8], BF16, tag="w1ebf")
                nc.scalar.activation(w1ebf[:], w1e[:], Copy)

                # denominators + reciprocal hi/lo
                for j in range(T1):
                    nc.tensor.matmul(
                        den[:], mm_sb[:, j, :], w1ebf[:, j, :],
                        start=(j == 0), stop=(j == T1 - 1),
                    )
                # PE filler while the reciprocal chain runs
                if len(pipe) == 3:
                    st = pipe.pop(0)
                    emit_agg_half(st, 1)
                    emit_drain_and_group(st)

                rec = l1w.tile([128, 8], F32, tag="rec")
                nc.vector.tensor_scalar_add(rec[:], den[:], 1e-16)
                nc.vector.reciprocal(rec[:], rec[:])
                rechl = l1w.tile([128, 16], BF16, tag="rechl")
                nc.vector.tensor_copy(rechl[:, 0:8], rec[:])
                rechf = l1w.tile([128, 8], F32, tag="rechf")
                nc.vector.tensor_copy(rechf[:], rechl[:, 0:8])
                nc.vector.tensor_tensor(rechl[:, 8:16], rec[:], rechf[:], SUB)

                # rec per edge + alpha
                for j in range(T1):
                    nc.tensor.matmul(
                        rcps[:, j, :], mt_sb[:, j, :], rechl[:],
                        start=True, stop=True,
                    )
                rcs = l1w.tile([128, T1, 16], F32, tag="rcs")
                nc.scalar.activation(rcs[:], rcps, Copy)
                alpha = l1w.tile([128, T1, 8], BF16, tag="alpha")
                rsum = l1w.tile([128, T1, 8], F32, tag="rsum")
                nc.vector.tensor_tensor(
                    rsum[:], rcs[:, :, 0:8], rcs[:, :, 8:16], ADD
                )
                nc.vector.tensor_tensor(alpha[:], w1e[:], rsum[:], MUL)

                # alpha-scaled interleaved masks for all tiles of this chunk
                M8a = l1k.tile([128, T1, 8, 128], BF16, tag="M8a")
                for j in range(T1):
                    t = c * T1 + j
                    nc.vector.scalar_tensor_tensor(
                        M8a[:, j, :, :],
                        iota128_sb[:].unsqueeze(1).broadcast_to([128, 8, 128]),
                        drel1_sb[:, t:t + 1],
                        alpha[:, j, :].unsqueeze(2).broadcast_to([128, 8, 128]),
                        EQ, MUL,
                    )
                pipe.append({"c": c, "xg": xg, "M8a": M8a})

            for st in pipe:
                emit_agg_half(st, 0)
                emit_agg_half(st, 1)
                emit_drain_and_group(st)

            psP.release()
            psT.release()
            psS.release()
            l1w.release()
            l1k.release()
            l1g.release()
            l1m.release()

            # ======== phase 7: AllGather G2 ========
            nc.gpsimd.collective_compute(
                "AllGather", mybir.AluOpType.bypass,
                ins=[g2_loc.opt()], outs=[g2_full.opt()],
                replica_groups=[list(range(NCORES))],
            )
            l1.release()
            g2f = g2_full

            # ======== phase 8+9: layer-2 edge weights + aggregation + W_out ========
            w2m = tc.alloc_tile_pool(name="w2m", bufs=4)
            w2w = tc.alloc_tile_pool(name="w2w", bufs=2)
            psE = tc.alloc_tile_pool(name="psE", bufs=2, space="PSUM")
            psF = tc.alloc_tile_pool(name="psF", bufs=2, space="PSUM")
            # a_d2 hi/lo from the locally saved phase-6 attention values
            nc.vector.tensor_copy(ad2hla[:, :, 0:1], ad2f[:])
            ad2hf = cpool.tile([128, CPC2, 1], F32, name="ad2hf", tag="ad2hf")
            nc.vector.tensor_copy(ad2hf[:], ad2hla[:, :, 0:1])
            nc.vector.tensor_tensor(ad2hla[:, :, 1:2], ad2f[:], ad2hf[:], SUB)
            # software-pipelined like L1: chunk c's chain runs on DVE/ScalarE
            # while chunk c-1's aggregation + output matmuls keep the PE busy
            def emit_l2_agg(st):
                xg2p, M1a = st["xg2"], st["M1a"]
                p30 = psF.tile([128, 128], F32, tag="p30")
                p31 = psF.tile([128, 128], F32, tag="p31")
                st["p30"], st["p31"] = p30, p31
                for j in range(T2):
                    nc.tensor.matmul(
                        p30[:], xg2p[:, j, 0:128], M1a[:, j, :],
                        start=(j == 0), stop=(j == T2 - 1),
                    )
                    nc.tensor.matmul(
                        p31[:], xg2p[:, j, 128:256], M1a[:, j, :],
                        start=(j == 0), stop=(j == T2 - 1),
                    )

            def emit_l2_out(st):
                c2 = st["c"]
                # x3T = relu(agg)  (feature-major: [feat, dst])
                x3T = sb.tile([128, 2, 128], BF16, tag="x3T")
                nc.scalar.activation(x3T[:, 0, :], st["p30"][:], Relu)
                nc.scalar.activation(x3T[:, 1, :], st["p31"][:], Relu)
                pout = psF.tile([128, EMB], F32, tag="pout")
                for fs in range(2):
                    nc.tensor.matmul(
                        pout[:], x3T[:, fs, :], woutt_sb[:, fs, :],
                        start=(fs == 0), stop=(fs == 1),
                    )
                osb = sb.tile([128, EMB], F32, tag="osb")
                nc.scalar.activation(osb[:], pout[:], Copy)
                nc.sync.dma_start(
                    out_dram[c2 * 128:(c2 + 1) * 128, :], osb[:]
                )

            prev2 = None
            for c in range(CPC2):
                tsl = slice(c * T2, (c + 1) * T2)
                xg2t = w2m.tile([128, T2, GW], BF16, tag="xg2")
                nc.gpsimd.dma_gather(
                    xg2t[:], g2f[:, 0:GW],
                    idx2_sb[:, c * T2 * 8:(c + 1) * T2 * 8],
                    T2 * 128, T2 * 128, GW, elem_step=GW,
                )
                xg2 = xg2t[:]
                xg2_f32 = xg2.bitcast(F32)      # [128, T2, 192]
                mt2_sb = w2m.tile([128, T2, 128], BF16, tag="mt2", name="mt2s")
                nc.sync.dma_start(mt2_sb[:], mT2d[:, tsl, :])
                mm2_sb = w2m.tile([128, T2, 128], BF16, tag="mm2", name="mm2s")
                nc.sync.dma_start(mm2_sb[:], m2d[:, tsl, :])

                # packed PSUM bank: [0:12]=a_d lookups, [12:13]=den, [16:28]=rec
                ps2 = psE.tile([128, 4 * T2 + 4], F32, tag="ps2")
                ad2ps = ps2[:, 0:T2 * 2].rearrange("p (a b) -> p a b", b=2)
                den2 = ps2[:, T2 * 2:T2 * 2 + 1]
                rc2ps = ps2[:, T2 * 2 + 2:4 * T2 + 2].rearrange(
                    "p (a b) -> p a b", b=2
                )
                for j in range(T2):
                    nc.tensor.matmul(
                        ad2ps[:, j, :], mt2_sb[:, j, :], ad2hla[:, c, :],
                        start=True, stop=True,
                    )
                # PE filler while chunk c's chain runs
                if prev2 is not None:
                    emit_l2_agg(prev2)
                ad2s = w2w.tile([128, T2, 2], F32, tag="ad2s")
                nc.scalar.activation(ad2s[:], ad2ps, Copy)
                w2e = w2w.tile([128, T2, 1], F32, tag="w2e")
                nc.vector.tensor_tensor(
                    w2e[:], ad2s[:, :, 0:1], ad2s[:, :, 1:2], ADD
                )
                nc.vector.tensor_tensor(
                    w2e[:], w2e[:], xg2_f32[:, 0:T2, 128:129], ADD
                )
                nc.vector.scalar_tensor_tensor(
                    w2e[:], w2e[:], NEG, w2e[:], MUL, MAX
                )
                nc.scalar.activation(w2e[:], w2e[:], Exp)
                w2ebf = w2w.tile([128, T2, 1], BF16, tag="w2ebf")
                nc.scalar.activation(w2ebf[:], w2e[:], Copy)

                for j in range(T2):
                    nc.tensor.matmul(
                        den2[:], mm2_sb[:, j, :], w2ebf[:, j, :],
                        start=(j == 0), stop=(j == T2 - 1),
                    )
                # PE filler while the reciprocal chain runs
                if prev2 is not None:
                    emit_l2_out(prev2)
                rec2 = w2w.tile([128, 1], F32, tag="rec2")
                nc.vector.tensor_scalar(rec2[:], den2[:], 1e-16, None, ADD)
                nc.vector.reciprocal(rec2[:], rec2[:])
                rec2hl = w2w.tile([128, 2], BF16, tag="rec2hl")
                nc.vector.tensor_copy(rec2hl[:, 0:1], rec2[:])
                rec2hf = w2w.tile([128, 1], F32, tag="rec2hf")
                nc.vector.tensor_copy(rec2hf[:], rec2hl[:, 0:1])
                nc.vector.tensor_tensor(rec2hl[:, 1:2], rec2[:], rec2hf[:], SUB)
                for j in range(T2):
                    nc.tensor.matmul(
                        rc2ps[:, j, :], mt2_sb[:, j, :], rec2hl[:],
                        start=True, stop=True,
                    )
                rc2s = w2w.tile([128, T2, 2], F32, tag="rc2s")
                nc.scalar.activation(rc2s[:], rc2ps, Copy)
                alpha2 = w2w.tile([128, T2, 1], F32, tag="alpha2")
                nc.vector.tensor_tensor(
                    alpha2[:], rc2s[:, :, 0:1], rc2s[:, :, 1:2], ADD
                )
                nc.vector.tensor_tensor(alpha2[:], alpha2[:], w2e[:], MUL)

                M1a = w2m.tile([128, T2, 128], BF16, tag="M1a")
                for j in range(T2):
                    t = c * T2 + j
                    nc.vector.scalar_tensor_tensor(
                        M1a[:, j, :], iota128_sb[:], drel2_sb[:, t:t + 1],
                        alpha2[:, j, :].broadcast_to([128, 128]),
                        EQ, MUL,
                    )
                prev2 = {"c": c, "xg2": xg2, "M1a": M1a}

            emit_l2_agg(prev2)
            emit_l2_out(prev2)
            psF.release()
            psE.release()
            w2w.release()
            w2m.release()

    nc.compile()
    return nc


# ================= pjrt execution (axon) with timing =================

_exec_cache = {}


def _run_pjrt(nc, in_maps, key):
    """Mirror of bass2jax.run_bass_via_pjrt with executable caching and
    device-side timing (warmup + timed run when BASS_GAT_TIME=1)."""
    import jax
    from jax.experimental.shard_map import shard_map
    from jax.sharding import Mesh, PartitionSpec
    from concourse import bass2jax, mybir as mb

    global LAST_EXEC_NS
    bass2jax.install_neuronx_cc_hook()

    if key not in _exec_cache:
        partition_name = (
            nc.partition_id_tensor.name if nc.partition_id_tensor else None
        )
        in_names, out_names, out_avals, zero_outs = [], [], [], []
        for alloc in nc.m.functions[0].allocations:
            if not isinstance(alloc, mb.MemoryLocationSet):
                continue
            name = alloc.memorylocations[0].name
            if alloc.kind == "ExternalInput":
                if name != partition_name:
                    in_names.append(name)
            elif alloc.kind == "ExternalOutput":
                shape = tuple(alloc.tensor_shape)
                dtype = mb.dt.np(alloc.dtype)
                out_names.append(name)
                out_avals.append(jax.core.ShapedArray(shape, dtype))
                zero_outs.append(np.zeros(shape, dtype))
        n_params = len(in_names)
        all_in_names = list(in_names) + list(out_names)
        if partition_name is not None:
            all_in_names.append(partition_name)

        def _body(*args):
            operands = list(args)
            if partition_name is not None:
                operands.append(bass2jax.partition_id_tensor())
            outs = bass2jax._bass_exec_p.bind(
                *operands,
                out_avals=tuple(out_avals),
                in_names=tuple(all_in_names),
                out_names=tuple(out_names),
                lowering_input_output_aliases=(),
                sim_require_finite=True,
                sim_require_nnan=True,
                nc=nc,
            )
            return tuple(outs)

        devices = jax.devices()[:NCORES]
        mesh = Mesh(np.asarray(devices), ("core",))
        n_outs = len(out_avals)
        sharded = jax.jit(
            shard_map(
                _body, mesh=mesh,
                in_specs=(PartitionSpec("core"),) * (n_params + n_outs),
                out_specs=(PartitionSpec("core"),) * n_outs,
                check_rep=False,
            ),
            keep_unused=True,
        )
        _exec_cache[key] = (sharded, in_names, out_names, out_avals, zero_outs)
    sharded, in_names, out_names, out_avals, zero_outs = _exec_cache[key]

    import jax
    concat_in = [
        np.concatenate([np.asarray(in_maps[c][n]) for c in range(NCORES)], axis=0)
        for n in in_names
    ]
    concat_zeros = [
        np.zeros((NCORES * z.shape[0], *z.shape[1:]), z.dtype) for z in zero_outs
    ]
    out_arrs = sharded(*concat_in, *concat_zeros)
    jax.block_until_ready(out_arrs)

    if os.environ.get("BASS_GAT_TIME", "0") == "1":
        import time as _time
        args = [jax.device_put(a) for a in concat_in + concat_zeros]
        jax.block_until_ready(args)
        reps = int(os.environ.get("BASS_GAT_REPS", "5"))
        ts = []
        for _ in range(reps):
            t0 = _time.perf_counter()
            o = sharded(*args)
            jax.block_until_ready(o)
            ts.append(_time.perf_counter() - t0)
        LAST_EXEC_NS = int(min(ts) * 1e9)

    return [
        {
            n: np.asarray(out_arrs[i]).reshape(NCORES, *out_avals[i].shape)[c]
            for i, n in enumerate(out_names)
        }
        for c in range(NCORES)
    ]

# ================= entry point =================

def kernel(**inputs) -> np.ndarray:
    inp = {k: np.asarray(v) for k, v in inputs.items()}
    for b in ("b_node", "b_col", "b1", "b2"):
        assert np.abs(inp[b]).max() == 0.0, f"nonzero {b} unsupported"
    b_out = inp["b_out"].astype(np.float32)

    meta = _prep(inp["edges"].astype(np.int64))
    T1, T2 = meta["T1"], meta["T2"]
    wts = _weights_prep(inp)

    key = (T1, T2)
    if key not in _prog_cache:
        _prog_cache[key] = _build_program(T1, T2)
    nc = _prog_cache[key]

    xn = np.tile(inp["constraints_state"].astype(np.float32), (1, 2))  # [4000,128]
    xc = np.tile(inp["columns_state"].astype(np.float32), (1, 2))      # [16000,256]

    in_maps = []
    for m in range(NCORES):
        xct = np.zeros((128, CON_CH * 128), np.float32)
        xcolt = np.zeros((128, 2, COL_CH * 128), np.float32)
        for lc, ch in enumerate(meta["chunks1"][m * CPC1:(m + 1) * CPC1]):
            if lc < CON_CH:
                cols = lc * 128 + np.arange(len(ch))
                xct[:, cols] = xn[ch].T
            else:
                cols = (lc - CON_CH) * 128 + np.arange(len(ch))
                xcv = xc[np.asarray(ch) - NC_NODES]  # [k, 256]
                xcolt[:, 0, cols] = xcv[:, 0:128].T
                xcolt[:, 1, cols] = xcv[:, 128:256].T
        idx1, dr1, m1, mT1 = meta["et1"][m]
        idx2, dr2, m2, mT2 = meta["et2"][m]
        in_maps.append(dict(
            xct=xct, xcolt=xcolt,
            wnodet=wts["wnodet"], wcolt=wts["wcolt"], v1=wts["v1"],
            w1tb=wts["w1tb"], w2tb=wts["w2tb"], att2=wts["att2"],
            woutt=wts["woutt"], iota8i=wts["iota8i"], iota128=wts["iota128"],
            esrc1=idx1, drel1=dr1, m1=m1, mT1=mT1,
            esrc2=idx2, drel2=dr2, m2=m2, mT2=mT2,
        ))

    if os.environ.get("BASS_GAT_NTFF", "0") == "1":
        import ntff_hook
        ntff_hook.install()
        import tempfile
        global LAST_EXEC_NS, LAST_RESULTS
        td = tempfile.mkdtemp(prefix="gat_trace_")
        res = bass_utils.run_bass_kernel_spmd(
            nc, in_maps, core_ids=list(range(NCORES)), trace=True, tmpdir=td,
        )
        LAST_EXEC_NS = res.exec_time_ns
        LAST_RESULTS = res
        print("trace dir:", td)
        results = res.results
    else:
        results = _run_pjrt(nc, in_maps, key)

    out = np.zeros((NCOL, EMB), np.float32)
    for m in range(NCORES):
        o = np.asarray(results[m]["out"]).astype(np.float32)
        for lc, ch in enumerate(meta["chunks2"][m * CPC2:(m + 1) * CPC2]):
            if ch:
                rows = lc * 128 + np.arange(len(ch))
                out[np.asarray(ch) - NC_NODES] = o[rows]
    return out + b_out[None, :]


# revision 46
# speedup vs baseline: 1.2249x; 1.0108x over previous
"""GAT (2-layer, PyG-style) Trainium2 kernel, 8-core SPMD.

Strategy:
  - Nodes assigned to (core, 128-chunk) slots, load-balanced by in-degree;
    L2 chunks are co-located with the L1 column chunks (same membership), so
    layer-2 dst attention values stay core-local.
  - Aggregation in x-space (256-wide) with TRANSPOSED layout: the gathered
    source features are the matmul STATIONARY operand; the moving operand is
    a per-edge alpha-scaled one-hot mask block M8[e, (head, dstpos)] built in
    ONE fused DVE op (scalar_tensor_tensor: (iota==drel)*alpha) per edge
    tile. Output lands feature-major, which is exactly the layout the W1/W2
    projections need, so no transpose roundtrip.
  - Attention: a_src rides the feature gather (f32 cols in the same 768B
    table row); a_dst is expanded per-edge with tiny matmuls against
    host-uploaded static one-hot masks (m = [e,dst], mT = [dst,e]; bf16
    hi/lo splits keep the lookups near-exact); softmax denominators via
    m-matmuls; masks are pre-normalized by 1/den (alpha), so no
    post-scaling of the aggregate.
  - Software pipelining: chunk c's attention chain (DVE/ScalarE/small PE
    lookups) runs while chunk c-1's aggregation matmuls keep the PE busy;
    the layer-1->layer-2 projections (phases 5/6) run per 512-node group
    inside the same loop with small rotating buffers.
  - Cross-core: two AllGathers of the 768B-row node tables (G1, G2).
"""
import os, sys
import numpy as np
import ml_dtypes

sys.path.insert(0, "/opt/trn_rl_repo")
import concourse.bass as bass
import concourse.mybir as mybir
import concourse.tile as tile
import concourse.bacc as bacc
from concourse import bass_utils

F32 = mybir.dt.float32
BF16 = mybir.dt.bfloat16
I16 = mybir.dt.int16
BF = ml_dtypes.bfloat16

# ---------------- problem constants ----------------
NC_NODES = 4000
NCOL = 16000
N = NC_NODES + NCOL
NF, CF = 64, 128
HID = 256
H = 8
EMB = 128
NEG = 0.2

NCORES = 8
CON_CH = 4
COL_CH = 16
CPC1 = CON_CH + COL_CH          # 20
SLOT1 = CPC1 * 128              # 2560
CPC2 = 16
SLOT2 = CPC2 * 128              # 2048
GW = 384                        # bf16 table width (768B stride); f32 view 192
                                # (dma_gather elem size must be a multiple of
                                #  256B: 384*2 = 768B)

_prog_cache = {}
LAST_EXEC_NS = None
LAST_RESULTS = None


# ================= host-side preprocessing =================

def _balance(nodes, deg, n_chunks, cap=128):
    import heapq
    order = nodes[np.argsort(-deg[nodes], kind="stable")]
    loads = np.zeros(n_chunks, dtype=np.int64)
    counts = np.zeros(n_chunks, dtype=np.int64)
    heap = [(0, c) for c in range(n_chunks)]
    heapq.heapify(heap)
    members = [[] for _ in range(n_chunks)]
    for nd in order:
        while True:
            _, c = heapq.heappop(heap)
            if counts[c] < cap:
                break
        members[c].append(int(nd))
        counts[c] += 1
        loads[c] += int(deg[nd])
        if counts[c] < cap:
            heapq.heappush(heap, (loads[c], c))
    return members, loads


def _wrap_idx(idx):
    """dma_gather int16 index layout: [128, n/16]; row p holds idx[s*16+p%16]."""
    idx = np.asarray(idx, dtype=np.int16)
    n = len(idx)
    assert n % 16 == 0
    m = idx.reshape(n // 16, 16).T
    return np.tile(m, (8, 1)).copy()


def _onehots(drel, cpc, T):
    """drel: [cpc, T*128] float (dst position in chunk, or -1 pad).
    Returns m  [128(e), cpc*T, 128(p)]  and mT [128(p), cpc*T, 128(e)]  bf16."""
    d = drel.reshape(cpc, T, 128).astype(np.int32)       # [c, j, e]
    oh = (d[:, :, :, None] == np.arange(128)[None, None, None, :])  # [c,j,e,p]
    m = np.ascontiguousarray(
        oh.transpose(2, 0, 1, 3).reshape(128, cpc * T, 128)
    ).astype(BF)
    mT = np.ascontiguousarray(
        oh.transpose(3, 0, 1, 2).reshape(128, cpc * T, 128)
    ).astype(BF)
    return m, mT


def _prep(edges):
    src1 = np.concatenate([edges[0], np.arange(N)]).astype(np.int64)
    dst1 = np.concatenate([edges[1], np.arange(N)]).astype(np.int64)
    s2 = np.concatenate([edges[1], np.arange(N)]).astype(np.int64)
    d2 = np.concatenate([edges[0], np.arange(N)]).astype(np.int64)
    keep = d2 >= NC_NODES
    src2, dst2 = s2[keep], d2[keep]

    deg1 = np.bincount(dst1, minlength=N)
    deg2 = np.bincount(dst2, minlength=N)

    con_members, con_loads = _balance(np.arange(NC_NODES), deg1, NCORES * CON_CH)
    # column chunks serve BOTH layers (L2 chunks == L1 col chunks); balance on
    # deg1 (the larger layer) and accept the resulting T2
    col_members, _ = _balance(np.arange(NC_NODES, N), deg1, NCORES * COL_CH)
    gslot1 = np.full(N, -1, dtype=np.int64)
    chunks1 = [[] for _ in range(NCORES * CPC1)]
    for g, mem in enumerate(con_members):
        core, lc = g % NCORES, g // NCORES
        chunks1[core * CPC1 + lc] = mem
    for g, mem in enumerate(col_members):
        core, lc = g % NCORES, CON_CH + g // NCORES
        chunks1[core * CPC1 + lc] = mem
    for ci, mem in enumerate(chunks1):
        core, lc = divmod(ci, CPC1)
        for pos, nd in enumerate(mem):
            gslot1[nd] = core * SLOT1 + lc * 128 + pos
    assert (gslot1 >= 0).all()

    # chunks2 = the column chunks of layer 1 (identity co-location)
    chunks2 = [
        chunks1[core * CPC1 + CON_CH + lc]
        for core in range(NCORES) for lc in range(CPC2)
    ]
    # realized per-chunk loads determine the tile counts
    def chunk_load(members_list, deg):
        return max(
            (sum(deg[nd] for nd in mem) for mem in members_list if mem),
            default=0,
        )
    T1 = max(4, int(np.ceil(max(
        chunk_load([chunks1[i] for i in range(len(chunks1))], deg1), 1
    ) / 128)))
    T2 = max(4, int(np.ceil(max(chunk_load(chunks2, deg2), 1) / 128)))

    # table row layout is half-major (for split AllGathers):
    # row = half*(NCORES*HR) + core*HR + loc%HR,  HR = SLOT1//2
    HR = SLOT1 // 2
    def row_of(g):
        core, loc = g // SLOT1, g % SLOT1
        return (loc // HR) * (NCORES * HR) + core * HR + loc % HR

    # dst slot mapping for L2: position within the L1 col-chunk
    def edge_tables(src, dst, cpc, T, chunk_of_node, pos_of_node, remap):
        """Per core: src gather idx, drel, and static one-hot masks."""
        dcore = gslot1[dst] // SLOT1
        order = np.argsort(
            dcore * (cpc * 128) + chunk_of_node[dst] * 128 + pos_of_node[dst],
            kind="stable",
        )
        so, do = src[order], dst[order]
        core_of = dcore[order]
        cm_all, pm_all = chunk_of_node[do], pos_of_node[do]
        res = []
        for mcore in range(NCORES):
            esrc = np.zeros((cpc, T * 128), dtype=np.int64)
            drel = np.full((cpc, T * 128), -1.0, dtype=np.float32)
            sel = core_of == mcore
            sm, cm, pm = so[sel], cm_all[sel], pm_all[sel]
            for lc in range(cpc):
                s = cm == lc
                k = int(s.sum())
                assert k <= T * 128, f"chunk overflow {k} > {T*128}"
                esrc[lc, :k] = remap(gslot1[sm[s]])
                drel[lc, :k] = pm[s]
            idx = _wrap_idx(esrc.reshape(-1))
            m, mT = _onehots(drel, cpc, T)
            drel_dev = np.ascontiguousarray(
                drel.reshape(cpc, T, 128).transpose(2, 0, 1).reshape(128, cpc * T)
            )
            res.append((idx, drel_dev, m, mT))
        return res

    chunk1_of = (gslot1 % SLOT1) // 128          # L1 chunk index per node
    pos_of = gslot1 % 128
    chunk2_of = chunk1_of - CON_CH               # L2 chunk index (col nodes)
    et1 = edge_tables(src1, dst1, CPC1, T1, chunk1_of, pos_of, lambda g: g)
    et2 = edge_tables(src2, dst2, CPC2, T2, chunk2_of, pos_of, lambda g: g)
    return dict(gslot1=gslot1, chunks1=chunks1, chunks2=chunks2,
                T1=T1, T2=T2, et1=et1, et2=et2)


def _weights_prep(inp):
    W1 = inp["W1"].astype(np.float32)       # [2048, 256]
    W2 = inp["W2"].astype(np.float32)       # [256, 2048]
    out = {}
    out["wnodet"] = np.ascontiguousarray(inp["W_node"].T).astype(np.float32)  # [128,256]
    wct = inp["W_col"].T.astype(np.float32)  # [256, 256]
    out["wcolt"] = np.stack([wct[0:128], wct[128:256]], axis=1)  # [128, 2, 256]
    V1 = np.zeros((256, 16), np.float32)
    for h in range(H):
        Wh = W1[h * HID:(h + 1) * HID, :]
        V1[:, h] = Wh.T @ inp["att_src1"][h]
        V1[:, 8 + h] = Wh.T @ inp["att_dst1"][h]
    out["v1"] = np.stack([V1[0:128], V1[128:256]], axis=1)       # [128, 2, 16]
    W1T = W1.T                                                   # [256, 2048]
    w1tb = np.zeros((128, 32, 128), BF)
    for h in range(H):
        for os_ in range(2):
            for fs in range(2):
                w1tb[:, h * 4 + os_ * 2 + fs, :] = W1T[
                    fs * 128:(fs + 1) * 128,
                    h * 256 + os_ * 128: h * 256 + (os_ + 1) * 128,
                ].astype(BF)
    out["w1tb"] = w1tb
    W2T = W2.T                                                   # [2048, 256]
    w2tb = np.zeros((128, 32, 128), BF)
    for f16 in range(16):
        for os_ in range(2):
            w2tb[:, f16 * 2 + os_, :] = W2T[
                f16 * 128:(f16 + 1) * 128, os_ * 128:(os_ + 1) * 128
            ].astype(BF)
    out["w2tb"] = w2tb
    a2 = np.stack([inp["att_src2"][0], inp["att_dst2"][0]], axis=1)  # [256, 2]
    out["att2"] = np.stack([a2[0:128], a2[128:256]], axis=1).astype(BF)  # [128,2,2]
    wot = inp["W_out"].T.astype(np.float32)  # [256, 128]
    out["woutt"] = np.stack([wot[0:128], wot[128:256]], axis=1).astype(BF)  # [128,2,128]
    # iota8i[e, p*8+h] = p  (interleaved one-hot comparison pattern, L1)
    out["iota8i"] = np.broadcast_to(
        (np.arange(1024) // 8).astype(BF), (128, 1024)
    ).copy()
    # iota128[e, p] = p (L2)
    out["iota128"] = np.broadcast_to(
        np.arange(128).astype(BF), (128, 128)
    ).copy()
    return out


# ================= device program =================

def _build_program(T1, T2):
    nc = bacc.Bacc(None, target_bir_lowering=False)
    NT1, NT2 = CPC1 * T1, CPC2 * T2

    xct = nc.dram_tensor("xct", [128, CON_CH * 128], F32, kind="ExternalInput")
    xcolt = nc.dram_tensor("xcolt", [128, 2, COL_CH * 128], F32, kind="ExternalInput")
    wnodet = nc.dram_tensor("wnodet", [128, 256], F32, kind="ExternalInput")
    wcolt = nc.dram_tensor("wcolt", [128, 2, 256], F32, kind="ExternalInput")
    v1 = nc.dram_tensor("v1", [128, 2, 16], F32, kind="ExternalInput")
    w1tb = nc.dram_tensor("w1tb", [128, 32, 128], BF16, kind="ExternalInput")
    w2tb = nc.dram_tensor("w2tb", [128, 32, 128], BF16, kind="ExternalInput")
    att2 = nc.dram_tensor("att2", [128, 2, 2], BF16, kind="ExternalInput")
    woutt = nc.dram_tensor("woutt", [128, 2, 128], BF16, kind="ExternalInput")
    iota8i = nc.dram_tensor("iota8i", [128, 1024], BF16, kind="ExternalInput")
    iota128 = nc.dram_tensor("iota128", [128, 128], BF16, kind="ExternalInput")
    esrc1 = nc.dram_tensor("esrc1", [128, NT1 * 8], I16, kind="ExternalInput")
    drel1 = nc.dram_tensor("drel1", [128, NT1], F32, kind="ExternalInput")
    m1d = nc.dram_tensor("m1", [128, NT1, 128], BF16, kind="ExternalInput")
    mT1d = nc.dram_tensor("mT1", [128, NT1, 128], BF16, kind="ExternalInput")
    esrc2 = nc.dram_tensor("esrc2", [128, NT2 * 8], I16, kind="ExternalInput")
    drel2 = nc.dram_tensor("drel2", [128, NT2], F32, kind="ExternalInput")
    m2d = nc.dram_tensor("m2", [128, NT2, 128], BF16, kind="ExternalInput")
    mT2d = nc.dram_tensor("mT2", [128, NT2, 128], BF16, kind="ExternalInput")
    out_dram = nc.dram_tensor("out", [SLOT2, EMB], F32, kind="ExternalOutput")

    Copy = mybir.ActivationFunctionType.Copy
    Relu = mybir.ActivationFunctionType.Relu
    Exp = mybir.ActivationFunctionType.Exp
    ADD, EQ, MUL, MAX, SUB = (
        mybir.AluOpType.add, mybir.AluOpType.is_equal,
        mybir.AluOpType.mult, mybir.AluOpType.max,
        mybir.AluOpType.subtract,
    )

    with tile.TileContext(nc) as tc:
        with (
            tc.tile_pool(name="const", bufs=1) as cpool,
            tc.tile_pool(name="sb", bufs=3) as sb,
            tc.tile_pool(name="dram", bufs=1, space="DRAM") as dram,
        ):
            def cload(t, shape, dtype):
                nm = t.name + "_sb"
                s = cpool.tile(shape, dtype, name=nm, tag=nm)
                nc.sync.dma_start(s[:], t[:])
                return s

            # phase-1-critical loads first (everything else can trickle in
            # behind them on the DMA queue)
            wnodet_sb = cload(wnodet, [128, 256], F32)
            wcolt_sb = cload(wcolt, [128, 2, 256], F32)
            v1_sb = cload(v1, [128, 2, 16], F32)

            g1_loc = dram.tile([SLOT1, GW], BF16)
            g1_full = dram.tile([NCORES * SLOT1, GW], BF16, addr_space="Shared")
            g2_loc = dram.tile([SLOT1, GW], BF16)
            g2_full = dram.tile([NCORES * SLOT1, GW], BF16, addr_space="Shared")

            ad2f = cpool.tile([128, CPC2, 1], F32, name="ad2f", tag="ad2f")
            ad2hla = cpool.tile([128, CPC2, 2], BF16, name="ad2hla", tag="ad2hla")

            # long-lived L1 pool (adhl written in phase 1, read through L1)
            l1 = tc.alloc_tile_pool(name="l1", bufs=1)
            aggnT4 = l1.tile([128, 2, 4, 8, 128], BF16, tag="aggnT4")
            adhl = l1.tile([128, CPC1, 16], BF16, tag="adhl")

            # ======== phase 1: input MLPs ========
            p1 = tc.alloc_tile_pool(name="p1", bufs=1)
            psA = tc.alloc_tile_pool(name="psA", bufs=2, space="PSUM")
            xct_sb = p1.tile([128, CON_CH * 128], F32, tag="xct_sb")
            nc.sync.dma_start(xct_sb[:], xct[:])
            xcolt_sb = p1.tile([128, 2, COL_CH * 128], F32, tag="xcolt_sb")
            nc.sync.dma_start(xcolt_sb[:], xcolt[:])
            # remaining constants (not needed until later phases)
            iota8i_sb = cload(iota8i, [128, 1024], BF16)
            iota128_sb = cload(iota128, [128, 128], BF16)
            w1tb_sb = cload(w1tb, [128, 32, 128], BF16)
            w2tb_sb = cload(w2tb, [128, 32, 128], BF16)
            att2_sb = cload(att2, [128, 2, 2], BF16)
            woutt_sb = cload(woutt, [128, 2, 128], BF16)
            idx1_sb = cload(esrc1, [128, NT1 * 8], I16)
            drel1_sb = cload(drel1, [128, NT1], F32)
            idx2_sb = cload(esrc2, [128, NT2 * 8], I16)
            drel2_sb = cload(drel2, [128, NT2], F32)
            xT = p1.tile([128, 2, SLOT1], F32, tag="xT")
            for os_ in range(2):
                p = psA.tile([128, CON_CH * 128], F32, tag="pmlp")
                nc.tensor.matmul(
                    p[:], wnodet_sb[:, os_ * 128:(os_ + 1) * 128], xct_sb[:],
                    start=True, stop=True,
                )
                nc.scalar.activation(xT[:, os_, 0:CON_CH * 128], p[:], Relu)
                for nch in range(4):
                    p2 = psA.tile([128, 512], F32, tag="pmlp2")
                    for fs in range(2):
                        nc.tensor.matmul(
                            p2[:],
                            wcolt_sb[:, fs, os_ * 128:(os_ + 1) * 128],
                            xcolt_sb[:, fs, nch * 512:(nch + 1) * 512],
                            start=(fs == 0), stop=(fs == 1),
                        )
                    nc.scalar.activation(
                        xT[:, os_, CON_CH * 128 + nch * 512: CON_CH * 128 + (nch + 1) * 512],
                        p2[:], Relu,
                    )

            # node-major x + attention values -> G1 rows; keep a_d locally
            g1sb = p1.tile([128, CPC1, GW], BF16, tag="g1sb")
            g1sb_f32 = g1sb[:].bitcast(F32)       # [128, CPC1, 192]
            pa_all = p1.tile([128, CPC1, 16], F32, tag="pa_all")
            for c in range(CPC1):
                nsl = slice(c * 128, (c + 1) * 128)
                px = psA.tile([128, 256], F32, tag="px")
                if c < CON_CH:
                    nc.tensor.matmul(
                        px[:], xct_sb[:, nsl], wnodet_sb[:], start=True, stop=True
                    )
                else:
                    ksl = slice((c - CON_CH) * 128, (c - CON_CH) * 128 + 128)
                    for fs in range(2):
                        nc.tensor.matmul(
                            px[:], xcolt_sb[:, fs, ksl], wcolt_sb[:, fs, :],
                            start=(fs == 0), stop=(fs == 1),
                        )
                pa = psA.tile([128, 16], F32, tag="pa")
                for fs in range(2):
                    nc.tensor.matmul(
                        pa[:], xT[:, fs, nsl], v1_sb[:, fs, :],
                        start=(fs == 0), stop=(fs == 1),
                    )
                nc.scalar.activation(g1sb[:, c, 0:256], px[:], Relu)
                nc.vector.tensor_copy(g1sb_f32[:, c, 128:136], pa[:, 0:8])
                nc.vector.tensor_copy(pa_all[:, c, :], pa[:])

            # a_d hi/lo split for exact bf16-matmul lookups: [128, CPC1, 16]
            nc.vector.tensor_copy(adhl[:, :, 0:8], pa_all[:, :, 8:16])
            adhif = p1.tile([128, CPC1, 8], F32, tag="adhif")
            nc.vector.tensor_copy(adhif[:], adhl[:, :, 0:8])
            nc.vector.tensor_tensor(
                adhl[:, :, 8:16], pa_all[:, :, 8:16], adhif[:], SUB
            )
            nc.sync.dma_start(
                g1_loc[:].rearrange("(c p) w -> p c w", p=128), g1sb[:]
            )

            psA.release()

            # ======== phase 2: AllGather G1 ========
            nc.gpsimd.collective_compute(
                "AllGather", mybir.AluOpType.bypass,
                ins=[g1_loc.opt()], outs=[g1_full.opt()],
                replica_groups=[list(range(NCORES))],
            )
            p1.release()
            g1f = g1_full

            # ======== phase 3+4: layer-1 edge weights + aggregation ========
            l1m = tc.alloc_tile_pool(name="l1m", bufs=5)
            l1g = tc.alloc_tile_pool(name="l1g", bufs=2)
            l1k = tc.alloc_tile_pool(name="l1k", bufs=5)
            l1w = tc.alloc_tile_pool(name="l1w", bufs=3)
            psS = tc.alloc_tile_pool(name="psS", bufs=2, space="PSUM")
            psT = tc.alloc_tile_pool(name="psT", bufs=1, space="PSUM")
            psP = tc.alloc_tile_pool(name="psP", bufs=2, space="PSUM")

            # software-pipelined over chunks: while chunk c's attention chain
            # runs on DVE/ScalarE, chunk c-1's aggregation matmuls keep the PE
            # busy (emitted into the chain's dependency gaps).

            def emit_agg_half(st, which):
                c, xg, M8a = st["c"], st["xg"], st["M8a"]
                pT = psT.tile([128, 1024], F32, tag=f"pT{which}")
                st[f"pT{which}"] = pT
                fsl = slice(which * 128, (which + 1) * 128)
                for j in range(T1):
                    M8f = M8a[:, j, :, :].rearrange("p a b -> p (a b)")
                    for half in range(2):
                        nc.tensor.matmul(
                            pT[:, half * 512:(half + 1) * 512],
                            xg[:, j, fsl], M8f[:, half * 512:(half + 1) * 512],
                            start=(j == 0), stop=(j == T1 - 1),
                        )

            def emit_drain_and_group(st):
                c = st["c"]
                nc.scalar.activation(
                    aggnT4[:, 0, c % 4, :, :].rearrange("p a b -> p (a b)"),
                    st["pT0"][:], Copy,
                )
                nc.scalar.activation(
                    aggnT4[:, 1, c % 4, :, :].rearrange("p a b -> p (a b)"),
                    st["pT1"][:], Copy,
                )
                if c % 4 != 3:
                    return
                # phases 5+6 for the completed 4-chunk group (512 nodes)
                g = c // 4
                x2Tg = l1g.tile([128, 16, 512], BF16, tag="x2Tg")
                for hh in range(16):
                    h, os_ = hh // 2, hh % 2
                    px2 = psP.tile([128, 512], F32, tag="pproj")
                    for fs in range(2):
                        rhs = aggnT4[:, fs, :, h, :]   # [128, 4, 128]
                        nc.tensor.matmul(
                            px2[:],
                            w1tb_sb[:, h * 4 + os_ * 2 + fs, :],
                            rhs,
                            start=(fs == 0), stop=(fs == 1),
                        )
                    nc.scalar.activation(x2Tg[:, hh, :], px2[:], Relu)
                h2Tg = l1g.tile([128, 2, 512], BF16, tag="h2Tg")
                for os_ in range(2):
                    ph2 = psP.tile([128, 512], F32, tag="pproj")
                    for f16 in range(16):
                        nc.tensor.matmul(
                            ph2[:], w2tb_sb[:, f16 * 2 + os_, :],
                            x2Tg[:, f16, :],
                            start=(f16 == 0), stop=(f16 == 15),
                        )
                    nc.scalar.activation(h2Tg[:, os_, :], ph2[:], Copy)
                for ci in range(4):
                    cg = g * 4 + ci
                    nsl = slice(ci * 128, (ci + 1) * 128)
                    pa2 = psP.tile([128, 2], F32, tag="pproj")
                    for fs in range(2):
                        nc.tensor.matmul(
                            pa2[:], h2Tg[:, fs, nsl], att2_sb[:, fs, :],
                            start=(fs == 0), stop=(fs == 1),
                        )
                    g2c = l1g.tile([128, 1, GW], BF16, tag="g2c")
                    g2c_f32 = g2c[:].bitcast(F32)
                    for fs in range(2):
                        nc.sync.dma_start(
                            g2c[:, 0, fs * 128:(fs + 1) * 128],
                            h2Tg[:, fs, nsl], transpose=True,
                        )
                    nc.vector.tensor_copy(g2c_f32[:, 0, 128:130], pa2[:])
                    if cg >= CON_CH:
                        nc.vector.tensor_copy(
                            ad2f[:, cg - CON_CH, :], pa2[:, 1:2]
                        )
                    nc.sync.dma_start(
                        g2_loc[:].rearrange("(c p) w -> p c w", p=128)[
                            :, cg:cg + 1, :
                        ],
                        g2c[:],
                    )

            pipe = []
            for c in range(CPC1):
                tsl = slice(c * T1, (c + 1) * T1)
                xgt = l1m.tile([128, T1, GW], BF16, tag="xg1")
                nc.gpsimd.dma_gather(
                    xgt[:], g1f[:, 0:GW],
                    idx1_sb[:, c * T1 * 8:(c + 1) * T1 * 8],
                    T1 * 128, T1 * 128, GW, elem_step=GW,
                )
                xg = xgt[:]
                xg_f32 = xg.bitcast(F32)        # [128, T1, 192]
                mt_sb = l1m.tile([128, T1, 128], BF16, tag="mt1", name="mt1s")
                nc.sync.dma_start(mt_sb[:], mT1d[:, tsl, :])
                mm_sb = l1m.tile([128, T1, 128], BF16, tag="mm1", name="mm1s")
                nc.sync.dma_start(mm_sb[:], m1d[:, tsl, :])

                # one packed PSUM bank for the small per-chunk matmul outs
                psmall = psS.tile([128, 2 * T1 * 16 + 16], F32, tag="psmall")
                adps = psmall[:, 0:T1 * 16].rearrange("p (a b) -> p a b", b=16)
                den = psmall[:, T1 * 16:T1 * 16 + 8]
                rcps = psmall[:, T1 * 16 + 16:2 * T1 * 16 + 16].rearrange(
                    "p (a b) -> p a b", b=16
                )
                for j in range(T1):
                    nc.tensor.matmul(
                        adps[:, j, :], mt_sb[:, j, :], adhl[:, c, :],
                        start=True, stop=True,
                    )
                # PE filler while chunk c's chain runs on DVE/ScalarE
                if len(pipe) == 4:
                    emit_agg_half(pipe[0], 0)
                ads = l1w.tile([128, T1, 16], F32, tag="ads")
                nc.scalar.activation(ads[:], adps, Copy)
                w1e = l1w.tile([128, T1, 8], F32, tag="w1e")
                # e = a_s + ad_hi + ad_lo
                nc.vector.tensor_tensor(
                    w1e[:], ads[:, :, 0:8], ads[:, :, 8:16], ADD
                )
                nc.vector.tensor_tensor(
                    w1e[:], w1e[:],
                    xg_f32[:, :, 128:136], ADD
                )
                nc.vector.scalar_tensor_tensor(
                    w1e[:], w1e[:], NEG, w1e[:], MUL, MAX
                )
                nc.scalar.activation(w1e[:], w1e[:], Exp)
                w1ebf = l1w.tile([128, T1, 8], BF16, tag="w1ebf")
                nc.scalar.activation(w1ebf[:], w1e[:], Copy)

                # denominators + reciprocal hi/lo
                for j in range(T1):
                    nc.tensor.matmul(
                        den[:], mm_sb[:, j, :], w1ebf[:, j, :],
                        start=(j == 0), stop=(j == T1 - 1),
                    )
                # PE filler while the reciprocal chain runs
                if len(pipe) == 4:
                    st = pipe.pop(0)
                    emit_agg_half(st, 1)
                    emit_drain_and_group(st)

                rec = l1w.tile([128, 8], F32, tag="rec")
                nc.vector.tensor_scalar_add(rec[:], den[:], 1e-16)
                nc.vector.reciprocal(rec[:], rec[:])
                rechl = l1w.tile([128, 16], BF16, tag="rechl")
                nc.vector.tensor_copy(rechl[:, 0:8], rec[:])
                rechf = l1w.tile([128, 8], F32, tag="rechf")
                nc.vector.tensor_copy(rechf[:], rechl[:, 0:8])
                nc.vector.tensor_tensor(rechl[:, 8:16], rec[:], rechf[:], SUB)

                # rec per edge + alpha
                for j in range(T1):
                    nc.tensor.matmul(
                        rcps[:, j, :], mt_sb[:, j, :], rechl[:],
                        start=True, stop=True,
                    )
                rcs = l1w.tile([128, T1, 16], F32, tag="rcs")
                nc.scalar.activation(rcs[:], rcps, Copy)
                alpha = l1w.tile([128, T1, 8], BF16, tag="alpha")
                rsum = l1w.tile([128, T1, 8], F32, tag="rsum")
                nc.vector.tensor_tensor(
                    rsum[:], rcs[:, :, 0:8], rcs[:, :, 8:16], ADD
                )
                nc.vector.tensor_tensor(alpha[:], w1e[:], rsum[:], MUL)

                # alpha-scaled interleaved masks for all tiles of this chunk
                M8a = l1k.tile([128, T1, 8, 128], BF16, tag="M8a")
                for j in range(T1):
                    t = c * T1 + j
                    nc.vector.scalar_tensor_tensor(
                        M8a[:, j, :, :],
                        iota128_sb[:].unsqueeze(1).broadcast_to([128, 8, 128]),
                        drel1_sb[:, t:t + 1],
                        alpha[:, j, :].unsqueeze(2).broadcast_to([128, 8, 128]),
                        EQ, MUL,
                    )
                pipe.append({"c": c, "xg": xg, "M8a": M8a})

            for st in pipe:
                emit_agg_half(st, 0)
                emit_agg_half(st, 1)
                emit_drain_and_group(st)

            psP.release()
            psT.release()
            psS.release()
            l1w.release()
            l1k.release()
            l1g.release()
            l1m.release()

            # ======== phase 7: AllGather G2 ========
            nc.gpsimd.collective_compute(
                "AllGather", mybir.AluOpType.bypass,
                ins=[g2_loc.opt()], outs=[g2_full.opt()],
                replica_groups=[list(range(NCORES))],
            )
            l1.release()
            g2f = g2_full

            # ======== phase 8+9: layer-2 edge weights + aggregation + W_out ========
            w2m = tc.alloc_tile_pool(name="w2m", bufs=4)
            w2w = tc.alloc_tile_pool(name="w2w", bufs=2)
            psE = tc.alloc_tile_pool(name="psE", bufs=2, space="PSUM")
            psF = tc.alloc_tile_pool(name="psF", bufs=2, space="PSUM")
            # a_d2 hi/lo from the locally saved phase-6 attention values
            nc.vector.tensor_copy(ad2hla[:, :, 0:1], ad2f[:])
            ad2hf = cpool.tile([128, CPC2, 1], F32, name="ad2hf", tag="ad2hf")
            nc.vector.tensor_copy(ad2hf[:], ad2hla[:, :, 0:1])
            nc.vector.tensor_tensor(ad2hla[:, :, 1:2], ad2f[:], ad2hf[:], SUB)
            # software-pipelined like L1: chunk c's chain runs on DVE/ScalarE
            # while chunk c-1's aggregation + output matmuls keep the PE busy
            def emit_l2_agg(st):
                xg2p, M1a = st["xg2"], st["M1a"]
                p30 = psF.tile([128, 128], F32, tag="p30")
                p31 = psF.tile([128, 128], F32, tag="p31")
                st["p30"], st["p31"] = p30, p31
                for j in range(T2):
                    nc.tensor.matmul(
                        p30[:], xg2p[:, j, 0:128], M1a[:, j, :],
                        start=(j == 0), stop=(j == T2 - 1),
                    )
                    nc.tensor.matmul(
                        p31[:], xg2p[:, j, 128:256], M1a[:, j, :],
                        start=(j == 0), stop=(j == T2 - 1),
                    )

            def emit_l2_out(st):
                c2 = st["c"]
                # x3T = relu(agg)  (feature-major: [feat, dst])
                x3T = sb.tile([128, 2, 128], BF16, tag="x3T")
                nc.scalar.activation(x3T[:, 0, :], st["p30"][:], Relu)
                nc.scalar.activation(x3T[:, 1, :], st["p31"][:], Relu)
                pout = psF.tile([128, EMB], F32, tag="pout")
                for fs in range(2):
                    nc.tensor.matmul(
                        pout[:], x3T[:, fs, :], woutt_sb[:, fs, :],
                        start=(fs == 0), stop=(fs == 1),
                    )
                osb = sb.tile([128, EMB], F32, tag="osb")
                nc.scalar.activation(osb[:], pout[:], Copy)
                nc.sync.dma_start(
                    out_dram[c2 * 128:(c2 + 1) * 128, :], osb[:]
                )

            pipe2 = []
            for c in range(CPC2):
                tsl = slice(c * T2, (c + 1) * T2)
                xg2t = w2m.tile([128, T2, GW], BF16, tag="xg2")
                nc.gpsimd.dma_gather(
                    xg2t[:], g2f[:, 0:GW],
                    idx2_sb[:, c * T2 * 8:(c + 1) * T2 * 8],
                    T2 * 128, T2 * 128, GW, elem_step=GW,
                )
                xg2 = xg2t[:]
                xg2_f32 = xg2.bitcast(F32)      # [128, T2, 192]
                mt2_sb = w2m.tile([128, T2, 128], BF16, tag="mt2", name="mt2s")
                nc.sync.dma_start(mt2_sb[:], mT2d[:, tsl, :])
                mm2_sb = w2m.tile([128, T2, 128], BF16, tag="mm2", name="mm2s")
                nc.sync.dma_start(mm2_sb[:], m2d[:, tsl, :])

                # packed PSUM bank: [0:12]=a_d lookups, [12:13]=den, [16:28]=rec
                ps2 = psE.tile([128, 4 * T2 + 4], F32, tag="ps2")
                ad2ps = ps2[:, 0:T2 * 2].rearrange("p (a b) -> p a b", b=2)
                den2 = ps2[:, T2 * 2:T2 * 2 + 1]
                rc2ps = ps2[:, T2 * 2 + 2:4 * T2 + 2].rearrange(
                    "p (a b) -> p a b", b=2
                )
                for j in range(T2):
                    nc.tensor.matmul(
                        ad2ps[:, j, :], mt2_sb[:, j, :], ad2hla[:, c, :],
                        start=True, stop=True,
                    )
                # PE filler while chunk c's chain runs
                if len(pipe2) == 2:
                    emit_l2_agg(pipe2[0])
                ad2s = w2w.tile([128, T2, 2], F32, tag="ad2s")
                nc.scalar.activation(ad2s[:], ad2ps, Copy)
                w2e = w2w.tile([128, T2, 1], F32, tag="w2e")
                nc.vector.tensor_tensor(
                    w2e[:], ad2s[:, :, 0:1], ad2s[:, :, 1:2], ADD
                )
                nc.vector.tensor_tensor(
                    w2e[:], w2e[:], xg2_f32[:, 0:T2, 128:129], ADD
                )
                nc.vector.scalar_tensor_tensor(
                    w2e[:], w2e[:], NEG, w2e[:], MUL, MAX
                )
                nc.scalar.activation(w2e[:], w2e[:], Exp)
                w2ebf = w2w.tile([128, T2, 1], BF16, tag="w2ebf")
                nc.scalar.activation(w2ebf[:], w2e[:], Copy)

                for j in range(T2):
                    nc.tensor.matmul(
                        den2[:], mm2_sb[:, j, :], w2ebf[:, j, :],
                        start=(j == 0), stop=(j == T2 - 1),
                    )
                # PE filler while the reciprocal chain runs
                if len(pipe2) == 2:
                    emit_l2_out(pipe2.pop(0))
                rec2 = w2w.tile([128, 1], F32, tag="rec2")
                nc.vector.tensor_scalar(rec2[:], den2[:], 1e-16, None, ADD)
                nc.vector.reciprocal(rec2[:], rec2[:])
                rec2hl = w2w.tile([128, 2], BF16, tag="rec2hl")
                nc.vector.tensor_copy(rec2hl[:, 0:1], rec2[:])
                rec2hf = w2w.tile([128, 1], F32, tag="rec2hf")
                nc.vector.tensor_copy(rec2hf[:], rec2hl[:, 0:1])
                nc.vector.tensor_tensor(rec2hl[:, 1:2], rec2[:], rec2hf[:], SUB)
                for j in range(T2):
                    nc.tensor.matmul(
                        rc2ps[:, j, :], mt2_sb[:, j, :], rec2hl[:],
                        start=True, stop=True,
                    )
                rc2s = w2w.tile([128, T2, 2], F32, tag="rc2s")
                nc.scalar.activation(rc2s[:], rc2ps, Copy)
                alpha2 = w2w.tile([128, T2, 1], F32, tag="alpha2")
                nc.vector.tensor_tensor(
                    alpha2[:], rc2s[:, :, 0:1], rc2s[:, :, 1:2], ADD
                )
                nc.vector.tensor_tensor(alpha2[:], alpha2[:], w2e[:], MUL)

                M1a = w2m.tile([128, T2, 128], BF16, tag="M1a")
                for j in range(T2):
                    t = c * T2 + j
                    nc.vector.scalar_tensor_tensor(
                        M1a[:, j, :], iota128_sb[:], drel2_sb[:, t:t + 1],
                        alpha2[:, j, :].broadcast_to([128, 128]),
                        EQ, MUL,
                    )
                pipe2.append({"c": c, "xg2": xg2, "M1a": M1a})

            for st2 in pipe2:
                emit_l2_agg(st2)
                emit_l2_out(st2)
            psF.release()
            psE.release()
            w2w.release()
            w2m.release()

    nc.compile()
    return nc


# ================= pjrt execution (axon) with timing =================

_exec_cache = {}


def _run_pjrt(nc, in_maps, key):
    """Mirror of bass2jax.run_bass_via_pjrt with executable caching and
    device-side timing (warmup + timed run when BASS_GAT_TIME=1)."""
    import jax
    from jax.experimental.shard_map import shard_map
    from jax.sharding import Mesh, PartitionSpec
    from concourse import bass2jax, mybir as mb

    global LAST_EXEC_NS
    bass2jax.install_neuronx_cc_hook()

    if key not in _exec_cache:
        partition_name = (
            nc.partition_id_tensor.name if nc.partition_id_tensor else None
        )
        in_names, out_names, out_avals, zero_outs = [], [], [], []
        for alloc in nc.m.functions[0].allocations:
            if not isinstance(alloc, mb.MemoryLocationSet):
                continue
            name = alloc.memorylocations[0].name
            if alloc.kind == "ExternalInput":
                if name != partition_name:
                    in_names.append(name)
            elif alloc.kind == "ExternalOutput":
                shape = tuple(alloc.tensor_shape)
                dtype = mb.dt.np(alloc.dtype)
                out_names.append(name)
                out_avals.append(jax.core.ShapedArray(shape, dtype))
                zero_outs.append(np.zeros(shape, dtype))
        n_params = len(in_names)
        all_in_names = list(in_names) + list(out_names)
        if partition_name is not None:
            all_in_names.append(partition_name)

        def _body(*args):
            operands = list(args)
            if partition_name is not None:
                operands.append(bass2jax.partition_id_tensor())
            outs = bass2jax._bass_exec_p.bind(
                *operands,
                out_avals=tuple(out_avals),
                in_names=tuple(all_in_names),
                out_names=tuple(out_names),
                lowering_input_output_aliases=(),
                sim_require_finite=True,
                sim_require_nnan=True,
                nc=nc,
            )
            return tuple(outs)

        devices = jax.devices()[:NCORES]
        mesh = Mesh(np.asarray(devices), ("core",))
        n_outs = len(out_avals)
        sharded = jax.jit(
            shard_map(
                _body, mesh=mesh,
                in_specs=(PartitionSpec("core"),) * (n_params + n_outs),
                out_specs=(PartitionSpec("core"),) * n_outs,
                check_rep=False,
            ),
            keep_unused=True,
        )
        _exec_cache[key] = (sharded, in_names, out_names, out_avals, zero_outs)
    sharded, in_names, out_names, out_avals, zero_outs = _exec_cache[key]

    import jax
    concat_in = [
        np.concatenate([np.asarray(in_maps[c][n]) for c in range(NCORES)], axis=0)
        for n in in_names
    ]
    concat_zeros = [
        np.zeros((NCORES * z.shape[0], *z.shape[1:]), z.dtype) for z in zero_outs
    ]
    out_arrs = sharded(*concat_in, *concat_zeros)
    jax.block_until_ready(out_arrs)

    if os.environ.get("BASS_GAT_TIME", "0") == "1":
        import time as _time
        args = [jax.device_put(a) for a in concat_in + concat_zeros]
        jax.block_until_ready(args)
        reps = int(os.environ.get("BASS_GAT_REPS", "5"))
        ts = []
        for _ in range(reps):
            t0 = _time.perf_counter()
            o = sharded(*args)
            jax.block_until_ready(o)
            ts.append(_time.perf_counter() - t0)
        LAST_EXEC_NS = int(min(ts) * 1e9)

    return [
        {
            n: np.asarray(out_arrs[i]).reshape(NCORES, *out_avals[i].shape)[c]
            for i, n in enumerate(out_names)
        }
        for c in range(NCORES)
    ]

# ================= entry point =================

def kernel(**inputs) -> np.ndarray:
    inp = {k: np.asarray(v) for k, v in inputs.items()}
    for b in ("b_node", "b_col", "b1", "b2"):
        assert np.abs(inp[b]).max() == 0.0, f"nonzero {b} unsupported"
    b_out = inp["b_out"].astype(np.float32)

    meta = _prep(inp["edges"].astype(np.int64))
    T1, T2 = meta["T1"], meta["T2"]
    wts = _weights_prep(inp)

    key = (T1, T2)
    if key not in _prog_cache:
        _prog_cache[key] = _build_program(T1, T2)
    nc = _prog_cache[key]

    xn = np.tile(inp["constraints_state"].astype(np.float32), (1, 2))  # [4000,128]
    xc = np.tile(inp["columns_state"].astype(np.float32), (1, 2))      # [16000,256]

    in_maps = []
    for m in range(NCORES):
        xct = np.zeros((128, CON_CH * 128), np.float32)
        xcolt = np.zeros((128, 2, COL_CH * 128), np.float32)
        for lc, ch in enumerate(meta["chunks1"][m * CPC1:(m + 1) * CPC1]):
            if lc < CON_CH:
                cols = lc * 128 + np.arange(len(ch))
                xct[:, cols] = xn[ch].T
            else:
                cols = (lc - CON_CH) * 128 + np.arange(len(ch))
                xcv = xc[np.asarray(ch) - NC_NODES]  # [k, 256]
                xcolt[:, 0, cols] = xcv[:, 0:128].T
                xcolt[:, 1, cols] = xcv[:, 128:256].T
        idx1, dr1, m1, mT1 = meta["et1"][m]
        idx2, dr2, m2, mT2 = meta["et2"][m]
        in_maps.append(dict(
            xct=xct, xcolt=xcolt,
            wnodet=wts["wnodet"], wcolt=wts["wcolt"], v1=wts["v1"],
            w1tb=wts["w1tb"], w2tb=wts["w2tb"], att2=wts["att2"],
            woutt=wts["woutt"], iota8i=wts["iota8i"], iota128=wts["iota128"],
            esrc1=idx1, drel1=dr1, m1=m1, mT1=mT1,
            esrc2=idx2, drel2=dr2, m2=m2, mT2=mT2,
        ))

    if os.environ.get("BASS_GAT_NTFF", "0") == "1":
        import ntff_hook
        ntff_hook.install()
        import tempfile
        global LAST_EXEC_NS, LAST_RESULTS
        td = tempfile.mkdtemp(prefix="gat_trace_")
        res = bass_utils.run_bass_kernel_spmd(
            nc, in_maps, core_ids=list(range(NCORES)), trace=True, tmpdir=td,
        )
        LAST_EXEC_NS = res.exec_time_ns
        LAST_RESULTS = res
        print("trace dir:", td)
        results = res.results
    else:
        results = _run_pjrt(nc, in_maps, key)

    out = np.zeros((NCOL, EMB), np.float32)
    for m in range(NCORES):
        o = np.asarray(results[m]["out"]).astype(np.float32)
        for lc, ch in enumerate(meta["chunks2"][m * CPC2:(m + 1) * CPC2]):
            if ch:
                rows = lc * 128 + np.arange(len(ch))
                out[np.asarray(ch) - NC_NODES] = o[rows]
    return out + b_out[None, :]


# revision 48
# speedup vs baseline: 1.2396x; 1.0120x over previous
"""GAT (2-layer, PyG-style) Trainium2 kernel, 8-core SPMD.

Strategy:
  - Nodes assigned to (core, 128-chunk) slots, load-balanced by in-degree;
    L2 chunks are co-located with the L1 column chunks (same membership), so
    layer-2 dst attention values stay core-local.
  - Aggregation in x-space (256-wide) with TRANSPOSED layout: the gathered
    source features are the matmul STATIONARY operand; the moving operand is
    a per-edge alpha-scaled one-hot mask block M8[e, (head, dstpos)] built in
    ONE fused DVE op (scalar_tensor_tensor: (iota==drel)*alpha) per edge
    tile. Output lands feature-major, which is exactly the layout the W1/W2
    projections need, so no transpose roundtrip.
  - Attention: a_src rides the feature gather (f32 cols in the same 768B
    table row); a_dst is expanded per-edge with tiny matmuls against
    host-uploaded static one-hot masks (m = [e,dst], mT = [dst,e]; bf16
    hi/lo splits keep the lookups near-exact); softmax denominators via
    m-matmuls; masks are pre-normalized by 1/den (alpha), so no
    post-scaling of the aggregate.
  - Software pipelining: chunk c's attention chain (DVE/ScalarE/small PE
    lookups) runs while chunk c-1's aggregation matmuls keep the PE busy;
    the layer-1->layer-2 projections (phases 5/6) run per 512-node group
    inside the same loop with small rotating buffers.
  - Cross-core: two AllGathers of the 768B-row node tables (G1, G2).
"""
import os, sys
import numpy as np
import ml_dtypes

sys.path.insert(0, "/opt/trn_rl_repo")
import concourse.bass as bass
import concourse.mybir as mybir
import concourse.tile as tile
import concourse.bacc as bacc
from concourse import bass_utils

F32 = mybir.dt.float32
BF16 = mybir.dt.bfloat16
I16 = mybir.dt.int16
BF = ml_dtypes.bfloat16

# ---------------- problem constants ----------------
NC_NODES = 4000
NCOL = 16000
N = NC_NODES + NCOL
NF, CF = 64, 128
HID = 256
H = 8
EMB = 128
NEG = 0.2

NCORES = 8
CON_CH = 4
COL_CH = 16
CPC1 = CON_CH + COL_CH          # 20
SLOT1 = CPC1 * 128              # 2560
CPC2 = 16
SLOT2 = CPC2 * 128              # 2048
GW = 384                        # bf16 table width (768B stride); f32 view 192
                                # (dma_gather elem size must be a multiple of
                                #  256B: 384*2 = 768B)

_prog_cache = {}
LAST_EXEC_NS = None
LAST_RESULTS = None


# ================= host-side preprocessing =================

def _balance(nodes, deg, n_chunks, cap=128):
    import heapq
    order = nodes[np.argsort(-deg[nodes], kind="stable")]
    loads = np.zeros(n_chunks, dtype=np.int64)
    counts = np.zeros(n_chunks, dtype=np.int64)
    heap = [(0, c) for c in range(n_chunks)]
    heapq.heapify(heap)
    members = [[] for _ in range(n_chunks)]
    for nd in order:
        while True:
            _, c = heapq.heappop(heap)
            if counts[c] < cap:
                break
        members[c].append(int(nd))
        counts[c] += 1
        loads[c] += int(deg[nd])
        if counts[c] < cap:
            heapq.heappush(heap, (loads[c], c))
    return members, loads


def _wrap_idx(idx):
    """dma_gather int16 index layout: [128, n/16]; row p holds idx[s*16+p%16]."""
    idx = np.asarray(idx, dtype=np.int16)
    n = len(idx)
    assert n % 16 == 0
    m = idx.reshape(n // 16, 16).T
    return np.tile(m, (8, 1)).copy()


def _onehots(drel, cpc, T):
    """drel: [cpc, T*128] float (dst position in chunk, or -1 pad).
    Returns m  [128(e), cpc*T, 128(p)]  and mT [128(p), cpc*T, 128(e)]  bf16."""
    d = drel.reshape(cpc, T, 128).astype(np.int32)       # [c, j, e]
    oh = (d[:, :, :, None] == np.arange(128)[None, None, None, :])  # [c,j,e,p]
    m = np.ascontiguousarray(
        oh.transpose(2, 0, 1, 3).reshape(128, cpc * T, 128)
    ).astype(BF)
    mT = np.ascontiguousarray(
        oh.transpose(3, 0, 1, 2).reshape(128, cpc * T, 128)
    ).astype(BF)
    return m, mT


def _prep(edges):
    src1 = np.concatenate([edges[0], np.arange(N)]).astype(np.int64)
    dst1 = np.concatenate([edges[1], np.arange(N)]).astype(np.int64)
    s2 = np.concatenate([edges[1], np.arange(N)]).astype(np.int64)
    d2 = np.concatenate([edges[0], np.arange(N)]).astype(np.int64)
    keep = d2 >= NC_NODES
    src2, dst2 = s2[keep], d2[keep]

    deg1 = np.bincount(dst1, minlength=N)
    deg2 = np.bincount(dst2, minlength=N)

    con_members, con_loads = _balance(np.arange(NC_NODES), deg1, NCORES * CON_CH)
    # column chunks serve BOTH layers (L2 chunks == L1 col chunks); balance on
    # deg1 (the larger layer) and accept the resulting T2
    col_members, _ = _balance(np.arange(NC_NODES, N), deg1, NCORES * COL_CH)
    gslot1 = np.full(N, -1, dtype=np.int64)
    chunks1 = [[] for _ in range(NCORES * CPC1)]
    for g, mem in enumerate(con_members):
        core, lc = g % NCORES, g // NCORES
        chunks1[core * CPC1 + lc] = mem
    for g, mem in enumerate(col_members):
        core, lc = g % NCORES, CON_CH + g // NCORES
        chunks1[core * CPC1 + lc] = mem
    for ci, mem in enumerate(chunks1):
        core, lc = divmod(ci, CPC1)
        for pos, nd in enumerate(mem):
            gslot1[nd] = core * SLOT1 + lc * 128 + pos
    assert (gslot1 >= 0).all()

    # chunks2 = the column chunks of layer 1 (identity co-location)
    chunks2 = [
        chunks1[core * CPC1 + CON_CH + lc]
        for core in range(NCORES) for lc in range(CPC2)
    ]
    # realized per-chunk loads determine the tile counts
    def chunk_load(members_list, deg):
        return max(
            (sum(deg[nd] for nd in mem) for mem in members_list if mem),
            default=0,
        )
    T1 = max(4, int(np.ceil(max(
        chunk_load([chunks1[i] for i in range(len(chunks1))], deg1), 1
    ) / 128)))
    T2 = max(4, int(np.ceil(max(chunk_load(chunks2, deg2), 1) / 128)))

    # table row layout is half-major (for split AllGathers):
    # row = half*(NCORES*HR) + core*HR + loc%HR,  HR = SLOT1//2
    HR = SLOT1 // 2
    def row_of(g):
        core, loc = g // SLOT1, g % SLOT1
        return (loc // HR) * (NCORES * HR) + core * HR + loc % HR

    # dst slot mapping for L2: position within the L1 col-chunk
    def edge_tables(src, dst, cpc, T, chunk_of_node, pos_of_node, remap):
        """Per core: src gather idx, drel, and static one-hot masks."""
        dcore = gslot1[dst] // SLOT1
        order = np.argsort(
            dcore * (cpc * 128) + chunk_of_node[dst] * 128 + pos_of_node[dst],
            kind="stable",
        )
        so, do = src[order], dst[order]
        core_of = dcore[order]
        cm_all, pm_all = chunk_of_node[do], pos_of_node[do]
        res = []
        for mcore in range(NCORES):
            esrc = np.zeros((cpc, T * 128), dtype=np.int64)
            drel = np.full((cpc, T * 128), -1.0, dtype=np.float32)
            sel = core_of == mcore
            sm, cm, pm = so[sel], cm_all[sel], pm_all[sel]
            for lc in range(cpc):
                s = cm == lc
                k = int(s.sum())
                assert k <= T * 128, f"chunk overflow {k} > {T*128}"
                esrc[lc, :k] = remap(gslot1[sm[s]])
                drel[lc, :k] = pm[s]
            idx = _wrap_idx(esrc.reshape(-1))
            m, mT = _onehots(drel, cpc, T)
            drel_dev = np.ascontiguousarray(
                drel.reshape(cpc, T, 128).transpose(2, 0, 1).reshape(128, cpc * T)
            )
            res.append((idx, drel_dev, m, mT))
        return res

    chunk1_of = (gslot1 % SLOT1) // 128          # L1 chunk index per node
    pos_of = gslot1 % 128
    chunk2_of = chunk1_of - CON_CH               # L2 chunk index (col nodes)
    et1 = edge_tables(src1, dst1, CPC1, T1, chunk1_of, pos_of, lambda g: g)
    et2 = edge_tables(src2, dst2, CPC2, T2, chunk2_of, pos_of, lambda g: g)
    return dict(gslot1=gslot1, chunks1=chunks1, chunks2=chunks2,
                T1=T1, T2=T2, et1=et1, et2=et2)


def _weights_prep(inp):
    W1 = inp["W1"].astype(np.float32)       # [2048, 256]
    W2 = inp["W2"].astype(np.float32)       # [256, 2048]
    out = {}
    out["wnodet"] = np.ascontiguousarray(inp["W_node"].T).astype(np.float32)  # [128,256]
    wct = inp["W_col"].T.astype(np.float32)  # [256, 256]
    out["wcolt"] = np.stack([wct[0:128], wct[128:256]], axis=1)  # [128, 2, 256]
    V1 = np.zeros((256, 16), np.float32)
    for h in range(H):
        Wh = W1[h * HID:(h + 1) * HID, :]
        V1[:, h] = Wh.T @ inp["att_src1"][h]
        V1[:, 8 + h] = Wh.T @ inp["att_dst1"][h]
    out["v1"] = np.stack([V1[0:128], V1[128:256]], axis=1)       # [128, 2, 16]
    W1T = W1.T                                                   # [256, 2048]
    w1tb = np.zeros((128, 32, 128), BF)
    for h in range(H):
        for os_ in range(2):
            for fs in range(2):
                w1tb[:, h * 4 + os_ * 2 + fs, :] = W1T[
                    fs * 128:(fs + 1) * 128,
                    h * 256 + os_ * 128: h * 256 + (os_ + 1) * 128,
                ].astype(BF)
    out["w1tb"] = w1tb
    W2T = W2.T                                                   # [2048, 256]
    w2tb = np.zeros((128, 32, 128), BF)
    for f16 in range(16):
        for os_ in range(2):
            w2tb[:, f16 * 2 + os_, :] = W2T[
                f16 * 128:(f16 + 1) * 128, os_ * 128:(os_ + 1) * 128
            ].astype(BF)
    out["w2tb"] = w2tb
    a2 = np.stack([inp["att_src2"][0], inp["att_dst2"][0]], axis=1)  # [256, 2]
    out["att2"] = np.stack([a2[0:128], a2[128:256]], axis=1).astype(BF)  # [128,2,2]
    wot = inp["W_out"].T.astype(np.float32)  # [256, 128]
    out["woutt"] = np.stack([wot[0:128], wot[128:256]], axis=1).astype(BF)  # [128,2,128]
    # iota8i[e, p*8+h] = p  (interleaved one-hot comparison pattern, L1)
    out["iota8i"] = np.broadcast_to(
        (np.arange(1024) // 8).astype(BF), (128, 1024)
    ).copy()
    # iota128[e, p] = p (L2)
    out["iota128"] = np.broadcast_to(
        np.arange(128).astype(BF), (128, 128)
    ).copy()
    return out


# ================= device program =================

def _build_program(T1, T2):
    nc = bacc.Bacc(None, target_bir_lowering=False)
    NT1, NT2 = CPC1 * T1, CPC2 * T2

    xct = nc.dram_tensor("xct", [128, CON_CH * 128], F32, kind="ExternalInput")
    xcolt = nc.dram_tensor("xcolt", [128, 2, COL_CH * 128], F32, kind="ExternalInput")
    wnodet = nc.dram_tensor("wnodet", [128, 256], F32, kind="ExternalInput")
    wcolt = nc.dram_tensor("wcolt", [128, 2, 256], F32, kind="ExternalInput")
    v1 = nc.dram_tensor("v1", [128, 2, 16], F32, kind="ExternalInput")
    w1tb = nc.dram_tensor("w1tb", [128, 32, 128], BF16, kind="ExternalInput")
    w2tb = nc.dram_tensor("w2tb", [128, 32, 128], BF16, kind="ExternalInput")
    att2 = nc.dram_tensor("att2", [128, 2, 2], BF16, kind="ExternalInput")
    woutt = nc.dram_tensor("woutt", [128, 2, 128], BF16, kind="ExternalInput")
    iota8i = nc.dram_tensor("iota8i", [128, 1024], BF16, kind="ExternalInput")
    iota128 = nc.dram_tensor("iota128", [128, 128], BF16, kind="ExternalInput")
    esrc1 = nc.dram_tensor("esrc1", [128, NT1 * 8], I16, kind="ExternalInput")
    drel1 = nc.dram_tensor("drel1", [128, NT1], F32, kind="ExternalInput")
    m1d = nc.dram_tensor("m1", [128, NT1, 128], BF16, kind="ExternalInput")
    mT1d = nc.dram_tensor("mT1", [128, NT1, 128], BF16, kind="ExternalInput")
    esrc2 = nc.dram_tensor("esrc2", [128, NT2 * 8], I16, kind="ExternalInput")
    drel2 = nc.dram_tensor("drel2", [128, NT2], F32, kind="ExternalInput")
    m2d = nc.dram_tensor("m2", [128, NT2, 128], BF16, kind="ExternalInput")
    mT2d = nc.dram_tensor("mT2", [128, NT2, 128], BF16, kind="ExternalInput")
    out_dram = nc.dram_tensor("out", [SLOT2, EMB], F32, kind="ExternalOutput")

    Copy = mybir.ActivationFunctionType.Copy
    Relu = mybir.ActivationFunctionType.Relu
    Exp = mybir.ActivationFunctionType.Exp
    ADD, EQ, MUL, MAX, SUB = (
        mybir.AluOpType.add, mybir.AluOpType.is_equal,
        mybir.AluOpType.mult, mybir.AluOpType.max,
        mybir.AluOpType.subtract,
    )

    with tile.TileContext(nc) as tc:
        with (
            tc.tile_pool(name="const", bufs=1) as cpool,
            tc.tile_pool(name="sb", bufs=3) as sb,
            tc.tile_pool(name="dram", bufs=1, space="DRAM") as dram,
        ):
            def cload(t, shape, dtype):
                nm = t.name + "_sb"
                s = cpool.tile(shape, dtype, name=nm, tag=nm)
                nc.sync.dma_start(s[:], t[:])
                return s

            # phase-1-critical loads first (everything else can trickle in
            # behind them on the DMA queue)
            wnodet_sb = cload(wnodet, [128, 256], F32)
            wcolt_sb = cload(wcolt, [128, 2, 256], F32)
            v1_sb = cload(v1, [128, 2, 16], F32)

            g1_loc = dram.tile([SLOT1, GW], BF16)
            g1_full = dram.tile([NCORES * SLOT1, GW], BF16, addr_space="Shared")
            g2_loc = dram.tile([SLOT1, GW], BF16)
            g2_full = dram.tile([NCORES * SLOT1, GW], BF16, addr_space="Shared")

            ad2f = cpool.tile([128, CPC2, 1], F32, name="ad2f", tag="ad2f")
            ad2hla = cpool.tile([128, CPC2, 2], BF16, name="ad2hla", tag="ad2hla")

            # long-lived L1 pool (adhl written in phase 1, read through L1)
            l1 = tc.alloc_tile_pool(name="l1", bufs=1)
            aggnT4 = l1.tile([128, 2, 2, 4, 8, 128], BF16, tag="aggnT4")
            adhl = l1.tile([128, CPC1, 16], BF16, tag="adhl")

            # ======== phase 1: input MLPs ========
            p1 = tc.alloc_tile_pool(name="p1", bufs=1)
            psA = tc.alloc_tile_pool(name="psA", bufs=2, space="PSUM")
            xct_sb = p1.tile([128, CON_CH * 128], F32, tag="xct_sb")
            nc.sync.dma_start(xct_sb[:], xct[:])
            xcolt_sb = p1.tile([128, 2, COL_CH * 128], F32, tag="xcolt_sb")
            nc.sync.dma_start(xcolt_sb[:], xcolt[:])
            # remaining constants (not needed until later phases)
            iota8i_sb = cload(iota8i, [128, 1024], BF16)
            iota128_sb = cload(iota128, [128, 128], BF16)
            w1tb_sb = cload(w1tb, [128, 32, 128], BF16)
            w2tb_sb = cload(w2tb, [128, 32, 128], BF16)
            att2_sb = cload(att2, [128, 2, 2], BF16)
            woutt_sb = cload(woutt, [128, 2, 128], BF16)
            idx1_sb = cload(esrc1, [128, NT1 * 8], I16)
            drel1_sb = cload(drel1, [128, NT1], F32)
            idx2_sb = cload(esrc2, [128, NT2 * 8], I16)
            drel2_sb = cload(drel2, [128, NT2], F32)
            xT = p1.tile([128, 2, SLOT1], F32, tag="xT")
            for os_ in range(2):
                p = psA.tile([128, CON_CH * 128], F32, tag="pmlp")
                nc.tensor.matmul(
                    p[:], wnodet_sb[:, os_ * 128:(os_ + 1) * 128], xct_sb[:],
                    start=True, stop=True,
                )
                nc.scalar.activation(xT[:, os_, 0:CON_CH * 128], p[:], Relu)
                for nch in range(4):
                    p2 = psA.tile([128, 512], F32, tag="pmlp2")
                    for fs in range(2):
                        nc.tensor.matmul(
                            p2[:],
                            wcolt_sb[:, fs, os_ * 128:(os_ + 1) * 128],
                            xcolt_sb[:, fs, nch * 512:(nch + 1) * 512],
                            start=(fs == 0), stop=(fs == 1),
                        )
                    nc.scalar.activation(
                        xT[:, os_, CON_CH * 128 + nch * 512: CON_CH * 128 + (nch + 1) * 512],
                        p2[:], Relu,
                    )

            # node-major x + attention values -> G1 rows; keep a_d locally
            g1sb = p1.tile([128, CPC1, GW], BF16, tag="g1sb")
            g1sb_f32 = g1sb[:].bitcast(F32)       # [128, CPC1, 192]
            pa_all = p1.tile([128, CPC1, 16], F32, tag="pa_all")
            for c in range(CPC1):
                nsl = slice(c * 128, (c + 1) * 128)
                px = psA.tile([128, 256], F32, tag="px")
                if c < CON_CH:
                    nc.tensor.matmul(
                        px[:], xct_sb[:, nsl], wnodet_sb[:], start=True, stop=True
                    )
                else:
                    ksl = slice((c - CON_CH) * 128, (c - CON_CH) * 128 + 128)
                    for fs in range(2):
                        nc.tensor.matmul(
                            px[:], xcolt_sb[:, fs, ksl], wcolt_sb[:, fs, :],
                            start=(fs == 0), stop=(fs == 1),
                        )
                pa = psA.tile([128, 16], F32, tag="pa")
                for fs in range(2):
                    nc.tensor.matmul(
                        pa[:], xT[:, fs, nsl], v1_sb[:, fs, :],
                        start=(fs == 0), stop=(fs == 1),
                    )
                nc.scalar.activation(g1sb[:, c, 0:256], px[:], Relu)
                nc.vector.tensor_copy(g1sb_f32[:, c, 128:136], pa[:, 0:8])
                nc.vector.tensor_copy(pa_all[:, c, :], pa[:])

            # a_d hi/lo split for exact bf16-matmul lookups: [128, CPC1, 16]
            nc.vector.tensor_copy(adhl[:, :, 0:8], pa_all[:, :, 8:16])
            adhif = p1.tile([128, CPC1, 8], F32, tag="adhif")
            nc.vector.tensor_copy(adhif[:], adhl[:, :, 0:8])
            nc.vector.tensor_tensor(
                adhl[:, :, 8:16], pa_all[:, :, 8:16], adhif[:], SUB
            )
            nc.sync.dma_start(
                g1_loc[:].rearrange("(c p) w -> p c w", p=128), g1sb[:]
            )

            psA.release()

            # ======== phase 2: AllGather G1 ========
            nc.gpsimd.collective_compute(
                "AllGather", mybir.AluOpType.bypass,
                ins=[g1_loc.opt()], outs=[g1_full.opt()],
                replica_groups=[list(range(NCORES))],
            )
            p1.release()
            g1f = g1_full

            # ======== phase 3+4: layer-1 edge weights + aggregation ========
            l1m = tc.alloc_tile_pool(name="l1m", bufs=5)
            l1g = tc.alloc_tile_pool(name="l1g", bufs=2)
            l1k = tc.alloc_tile_pool(name="l1k", bufs=5)
            l1w = tc.alloc_tile_pool(name="l1w", bufs=3)
            psS = tc.alloc_tile_pool(name="psS", bufs=2, space="PSUM")
            psT = tc.alloc_tile_pool(name="psT", bufs=1, space="PSUM")
            psP = tc.alloc_tile_pool(name="psP", bufs=2, space="PSUM")

            # software-pipelined over chunks: while chunk c's attention chain
            # runs on DVE/ScalarE, chunk c-1's aggregation matmuls keep the PE
            # busy (emitted into the chain's dependency gaps).

            def emit_agg_half(st, which):
                c, xg, M8a = st["c"], st["xg"], st["M8a"]
                pT = psT.tile([128, 1024], F32, tag=f"pT{which}")
                st[f"pT{which}"] = pT
                fsl = slice(which * 128, (which + 1) * 128)
                for j in range(T1):
                    M8f = M8a[:, j, :, :].rearrange("p a b -> p (a b)")
                    for half in range(2):
                        nc.tensor.matmul(
                            pT[:, half * 512:(half + 1) * 512],
                            xg[:, j, fsl], M8f[:, half * 512:(half + 1) * 512],
                            start=(j == 0), stop=(j == T1 - 1),
                        )

            def emit_drain(st):
                c = st["c"]
                gb = (c // 4) % 2
                nc.scalar.activation(
                    aggnT4[:, gb, 0, c % 4, :, :].rearrange("p a b -> p (a b)"),
                    st["pT0"][:], Copy,
                )
                nc.scalar.activation(
                    aggnT4[:, gb, 1, c % 4, :, :].rearrange("p a b -> p (a b)"),
                    st["pT1"][:], Copy,
                )

            # group phases 5+6 split into 3 stages, emitted one per iteration
            # so the 72 projection matmuls spread over 3 chain gaps
            def stage_p5(gst, hh_lo, hh_hi):
                gb = gst["g"] % 2
                if gst["x2Tg"] is None:
                    gst["x2Tg"] = l1g.tile([128, 16, 512], BF16, tag="x2Tg", name="x2Tg")
                x2Tg = gst["x2Tg"]
                for hh in range(hh_lo, hh_hi):
                    h, os_ = hh // 2, hh % 2
                    px2 = psP.tile([128, 512], F32, tag="pproj")
                    for fs in range(2):
                        rhs = aggnT4[:, gb, fs, :, h, :]   # [128, 4, 128]
                        nc.tensor.matmul(
                            px2[:],
                            w1tb_sb[:, h * 4 + os_ * 2 + fs, :],
                            rhs,
                            start=(fs == 0), stop=(fs == 1),
                        )
                    nc.scalar.activation(x2Tg[:, hh, :], px2[:], Relu)

            def stage_p6(gst):
                g, x2Tg = gst["g"], gst["x2Tg"]
                h2Tg = l1g.tile([128, 2, 512], BF16, tag="h2Tg", name="h2Tg")
                for os_ in range(2):
                    ph2 = psP.tile([128, 512], F32, tag="pproj")
                    for f16 in range(16):
                        nc.tensor.matmul(
                            ph2[:], w2tb_sb[:, f16 * 2 + os_, :],
                            x2Tg[:, f16, :],
                            start=(f16 == 0), stop=(f16 == 15),
                        )
                    nc.scalar.activation(h2Tg[:, os_, :], ph2[:], Copy)
                for ci in range(4):
                    cg = g * 4 + ci
                    nsl = slice(ci * 128, (ci + 1) * 128)
                    pa2 = psP.tile([128, 2], F32, tag="pproj")
                    for fs in range(2):
                        nc.tensor.matmul(
                            pa2[:], h2Tg[:, fs, nsl], att2_sb[:, fs, :],
                            start=(fs == 0), stop=(fs == 1),
                        )
                    g2c = l1g.tile([128, 1, GW], BF16, tag="g2c")
                    g2c_f32 = g2c[:].bitcast(F32)
                    for fs in range(2):
                        nc.sync.dma_start(
                            g2c[:, 0, fs * 128:(fs + 1) * 128],
                            h2Tg[:, fs, nsl], transpose=True,
                        )
                    nc.vector.tensor_copy(g2c_f32[:, 0, 128:130], pa2[:])
                    if cg >= CON_CH:
                        nc.vector.tensor_copy(
                            ad2f[:, cg - CON_CH, :], pa2[:, 1:2]
                        )
                    nc.sync.dma_start(
                        g2_loc[:].rearrange("(c p) w -> p c w", p=128)[
                            :, cg:cg + 1, :
                        ],
                        g2c[:],
                    )

            groupq = []

            def emit_drain_and_group(st):
                emit_drain(st)
                if st["c"] % 4 == 3:
                    gst = {"g": st["c"] // 4, "x2Tg": None}
                    groupq.append(lambda gs=gst: stage_p5(gs, 0, 8))
                    groupq.append(lambda gs=gst: stage_p5(gs, 8, 16))
                    groupq.append(lambda gs=gst: stage_p6(gs))
                if groupq:
                    groupq.pop(0)()

            pipe = []
            for c in range(CPC1):
                tsl = slice(c * T1, (c + 1) * T1)
                xgt = l1m.tile([128, T1, GW], BF16, tag="xg1")
                nc.gpsimd.dma_gather(
                    xgt[:], g1f[:, 0:GW],
                    idx1_sb[:, c * T1 * 8:(c + 1) * T1 * 8],
                    T1 * 128, T1 * 128, GW, elem_step=GW,
                )
                xg = xgt[:]
                xg_f32 = xg.bitcast(F32)        # [128, T1, 192]
                mt_sb = l1m.tile([128, T1, 128], BF16, tag="mt1", name="mt1s")
                nc.sync.dma_start(mt_sb[:], mT1d[:, tsl, :])
                mm_sb = l1m.tile([128, T1, 128], BF16, tag="mm1", name="mm1s")
                nc.sync.dma_start(mm_sb[:], m1d[:, tsl, :])

                # one packed PSUM bank for the small per-chunk matmul outs
                psmall = psS.tile([128, 2 * T1 * 16 + 16], F32, tag="psmall")
                adps = psmall[:, 0:T1 * 16].rearrange("p (a b) -> p a b", b=16)
                den = psmall[:, T1 * 16:T1 * 16 + 8]
                rcps = psmall[:, T1 * 16 + 16:2 * T1 * 16 + 16].rearrange(
                    "p (a b) -> p a b", b=16
                )
                for j in range(T1):
                    nc.tensor.matmul(
                        adps[:, j, :], mt_sb[:, j, :], adhl[:, c, :],
                        start=True, stop=True,
                    )
                # PE filler while chunk c's chain runs on DVE/ScalarE
                if len(pipe) == 4:
                    emit_agg_half(pipe[0], 0)
                ads = l1w.tile([128, T1, 16], F32, tag="ads")
                nc.scalar.activation(ads[:], adps, Copy)
                w1e = l1w.tile([128, T1, 8], F32, tag="w1e")
                # e = a_s + ad_hi + ad_lo
                nc.vector.tensor_tensor(
                    w1e[:], ads[:, :, 0:8], ads[:, :, 8:16], ADD
                )
                nc.vector.tensor_tensor(
                    w1e[:], w1e[:],
                    xg_f32[:, :, 128:136], ADD
                )
                nc.vector.scalar_tensor_tensor(
                    w1e[:], w1e[:], NEG, w1e[:], MUL, MAX
                )
                nc.scalar.activation(w1e[:], w1e[:], Exp)
                w1ebf = l1w.tile([128, T1, 8], BF16, tag="w1ebf")
                nc.scalar.activation(w1ebf[:], w1e[:], Copy)

                # denominators + reciprocal hi/lo
                for j in range(T1):
                    nc.tensor.matmul(
                        den[:], mm_sb[:, j, :], w1ebf[:, j, :],
                        start=(j == 0), stop=(j == T1 - 1),
                    )
                # PE filler while the reciprocal chain runs
                if len(pipe) == 4:
                    st = pipe.pop(0)
                    emit_agg_half(st, 1)
                    emit_drain_and_group(st)

                rec = l1w.tile([128, 8], F32, tag="rec")
                nc.vector.tensor_scalar_add(rec[:], den[:], 1e-16)
                nc.vector.reciprocal(rec[:], rec[:])
                rechl = l1w.tile([128, 16], BF16, tag="rechl")
                nc.vector.tensor_copy(rechl[:, 0:8], rec[:])
                rechf = l1w.tile([128, 8], F32, tag="rechf")
                nc.vector.tensor_copy(rechf[:], rechl[:, 0:8])
                nc.vector.tensor_tensor(rechl[:, 8:16], rec[:], rechf[:], SUB)

                # rec per edge + alpha
                for j in range(T1):
                    nc.tensor.matmul(
                        rcps[:, j, :], mt_sb[:, j, :], rechl[:],
                        start=True, stop=True,
                    )
                rcs = l1w.tile([128, T1, 16], F32, tag="rcs")
                nc.scalar.activation(rcs[:], rcps, Copy)
                alpha = l1w.tile([128, T1, 8], BF16, tag="alpha")
                rsum = l1w.tile([128, T1, 8], F32, tag="rsum")
                nc.vector.tensor_tensor(
                    rsum[:], rcs[:, :, 0:8], rcs[:, :, 8:16], ADD
                )
                nc.vector.tensor_tensor(alpha[:], w1e[:], rsum[:], MUL)

                # alpha-scaled interleaved masks for all tiles of this chunk
                M8a = l1k.tile([128, T1, 8, 128], BF16, tag="M8a")
                for j in range(T1):
                    t = c * T1 + j
                    nc.vector.scalar_tensor_tensor(
                        M8a[:, j, :, :],
                        iota128_sb[:].unsqueeze(1).broadcast_to([128, 8, 128]),
                        drel1_sb[:, t:t + 1],
                        alpha[:, j, :].unsqueeze(2).broadcast_to([128, 8, 128]),
                        EQ, MUL,
                    )
                pipe.append({"c": c, "xg": xg, "M8a": M8a})

            for st in pipe:
                emit_agg_half(st, 0)
                emit_agg_half(st, 1)
                emit_drain_and_group(st)
            while groupq:
                groupq.pop(0)()

            psP.release()
            psT.release()
            psS.release()
            l1w.release()
            l1k.release()
            l1g.release()
            l1m.release()

            # ======== phase 7: AllGather G2 ========
            nc.gpsimd.collective_compute(
                "AllGather", mybir.AluOpType.bypass,
                ins=[g2_loc.opt()], outs=[g2_full.opt()],
                replica_groups=[list(range(NCORES))],
            )
            l1.release()
            g2f = g2_full

            # ======== phase 8+9: layer-2 edge weights + aggregation + W_out ========
            w2m = tc.alloc_tile_pool(name="w2m", bufs=4)
            w2w = tc.alloc_tile_pool(name="w2w", bufs=2)
            psE = tc.alloc_tile_pool(name="psE", bufs=2, space="PSUM")
            psF = tc.alloc_tile_pool(name="psF", bufs=2, space="PSUM")
            # a_d2 hi/lo from the locally saved phase-6 attention values
            nc.vector.tensor_copy(ad2hla[:, :, 0:1], ad2f[:])
            ad2hf = cpool.tile([128, CPC2, 1], F32, name="ad2hf", tag="ad2hf")
            nc.vector.tensor_copy(ad2hf[:], ad2hla[:, :, 0:1])
            nc.vector.tensor_tensor(ad2hla[:, :, 1:2], ad2f[:], ad2hf[:], SUB)
            # software-pipelined like L1: chunk c's chain runs on DVE/ScalarE
            # while chunk c-1's aggregation + output matmuls keep the PE busy
            def emit_l2_agg(st):
                xg2p, M1a = st["xg2"], st["M1a"]
                p30 = psF.tile([128, 128], F32, tag="p30")
                p31 = psF.tile([128, 128], F32, tag="p31")
                st["p30"], st["p31"] = p30, p31
                for j in range(T2):
                    nc.tensor.matmul(
                        p30[:], xg2p[:, j, 0:128], M1a[:, j, :],
                        start=(j == 0), stop=(j == T2 - 1),
                    )
                    nc.tensor.matmul(
                        p31[:], xg2p[:, j, 128:256], M1a[:, j, :],
                        start=(j == 0), stop=(j == T2 - 1),
                    )

            def emit_l2_out(st):
                c2 = st["c"]
                # x3T = relu(agg)  (feature-major: [feat, dst])
                x3T = sb.tile([128, 2, 128], BF16, tag="x3T")
                nc.scalar.activation(x3T[:, 0, :], st["p30"][:], Relu)
                nc.scalar.activation(x3T[:, 1, :], st["p31"][:], Relu)
                pout = psF.tile([128, EMB], F32, tag="pout")
                for fs in range(2):
                    nc.tensor.matmul(
                        pout[:], x3T[:, fs, :], woutt_sb[:, fs, :],
                        start=(fs == 0), stop=(fs == 1),
                    )
                osb = sb.tile([128, EMB], F32, tag="osb")
                nc.scalar.activation(osb[:], pout[:], Copy)
                nc.sync.dma_start(
                    out_dram[c2 * 128:(c2 + 1) * 128, :], osb[:]
                )

            pipe2 = []
            for c in range(CPC2):
                tsl = slice(c * T2, (c + 1) * T2)
                xg2t = w2m.tile([128, T2, GW], BF16, tag="xg2")
                nc.gpsimd.dma_gather(
                    xg2t[:], g2f[:, 0:GW],
                    idx2_sb[:, c * T2 * 8:(c + 1) * T2 * 8],
                    T2 * 128, T2 * 128, GW, elem_step=GW,
                )
                xg2 = xg2t[:]
                xg2_f32 = xg2.bitcast(F32)      # [128, T2, 192]
                mt2_sb = w2m.tile([128, T2, 128], BF16, tag="mt2", name="mt2s")
                nc.sync.dma_start(mt2_sb[:], mT2d[:, tsl, :])
                mm2_sb = w2m.tile([128, T2, 128], BF16, tag="mm2", name="mm2s")
                nc.sync.dma_start(mm2_sb[:], m2d[:, tsl, :])

                # packed PSUM bank: [0:12]=a_d lookups, [12:13]=den, [16:28]=rec
                ps2 = psE.tile([128, 4 * T2 + 4], F32, tag="ps2")
                ad2ps = ps2[:, 0:T2 * 2].rearrange("p (a b) -> p a b", b=2)
                den2 = ps2[:, T2 * 2:T2 * 2 + 1]
                rc2ps = ps2[:, T2 * 2 + 2:4 * T2 + 2].rearrange(
                    "p (a b) -> p a b", b=2
                )
                for j in range(T2):
                    nc.tensor.matmul(
                        ad2ps[:, j, :], mt2_sb[:, j, :], ad2hla[:, c, :],
                        start=True, stop=True,
                    )
                # PE filler while chunk c's chain runs
                if len(pipe2) == 2:
                    emit_l2_agg(pipe2[0])
                ad2s = w2w.tile([128, T2, 2], F32, tag="ad2s")
                nc.scalar.activation(ad2s[:], ad2ps, Copy)
                w2e = w2w.tile([128, T2, 1], F32, tag="w2e")
                nc.vector.tensor_tensor(
                    w2e[:], ad2s[:, :, 0:1], ad2s[:, :, 1:2], ADD
                )
                nc.vector.tensor_tensor(
                    w2e[:], w2e[:], xg2_f32[:, 0:T2, 128:129], ADD
                )
                nc.vector.scalar_tensor_tensor(
                    w2e[:], w2e[:], NEG, w2e[:], MUL, MAX
                )
                nc.scalar.activation(w2e[:], w2e[:], Exp)
                w2ebf = w2w.tile([128, T2, 1], BF16, tag="w2ebf")
                nc.scalar.activation(w2ebf[:], w2e[:], Copy)

                for j in range(T2):
                    nc.tensor.matmul(
                        den2[:], mm2_sb[:, j, :], w2ebf[:, j, :],
                        start=(j == 0), stop=(j == T2 - 1),
                    )
                # PE filler while the reciprocal chain runs
                if len(pipe2) == 2:
                    emit_l2_out(pipe2.pop(0))
                rec2 = w2w.tile([128, 1], F32, tag="rec2")
                nc.vector.tensor_scalar(rec2[:], den2[:], 1e-16, None, ADD)
                nc.vector.reciprocal(rec2[:], rec2[:])
                rec2hl = w2w.tile([128, 2], BF16, tag="rec2hl")
                nc.vector.tensor_copy(rec2hl[:, 0:1], rec2[:])
                rec2hf = w2w.tile([128, 1], F32, tag="rec2hf")
                nc.vector.tensor_copy(rec2hf[:], rec2hl[:, 0:1])
                nc.vector.tensor_tensor(rec2hl[:, 1:2], rec2[:], rec2hf[:], SUB)
                for j in range(T2):
                    nc.tensor.matmul(
                        rc2ps[:, j, :], mt2_sb[:, j, :], rec2hl[:],
                        start=True, stop=True,
                    )
                rc2s = w2w.tile([128, T2, 2], F32, tag="rc2s")
                nc.scalar.activation(rc2s[:], rc2ps, Copy)
                alpha2 = w2w.tile([128, T2, 1], F32, tag="alpha2")
                nc.vector.tensor_tensor(
                    alpha2[:], rc2s[:, :, 0:1], rc2s[:, :, 1:2], ADD
                )
                nc.vector.tensor_tensor(alpha2[:], alpha2[:], w2e[:], MUL)

                M1a = w2m.tile([128, T2, 128], BF16, tag="M1a")
                for j in range(T2):
                    t = c * T2 + j
                    nc.vector.scalar_tensor_tensor(
                        M1a[:, j, :], iota128_sb[:], drel2_sb[:, t:t + 1],
                        alpha2[:, j, :].broadcast_to([128, 128]),
                        EQ, MUL,
                    )
                pipe2.append({"c": c, "xg2": xg2, "M1a": M1a})

            for st2 in pipe2:
                emit_l2_agg(st2)
                emit_l2_out(st2)
            psF.release()
            psE.release()
            w2w.release()
            w2m.release()

    nc.compile()
    return nc


# ================= pjrt execution (axon) with timing =================

_exec_cache = {}


def _run_pjrt(nc, in_maps, key):
    """Mirror of bass2jax.run_bass_via_pjrt with executable caching and
    device-side timing (warmup + timed run when BASS_GAT_TIME=1)."""
    import jax
    from jax.experimental.shard_map import shard_map
    from jax.sharding import Mesh, PartitionSpec
    from concourse import bass2jax, mybir as mb

    global LAST_EXEC_NS
    bass2jax.install_neuronx_cc_hook()

    if key not in _exec_cache:
        partition_name = (
            nc.partition_id_tensor.name if nc.partition_id_tensor else None
        )
        in_names, out_names, out_avals, zero_outs = [], [], [], []
        for alloc in nc.m.functions[0].allocations:
            if not isinstance(alloc, mb.MemoryLocationSet):
                continue
            name = alloc.memorylocations[0].name
            if alloc.kind == "ExternalInput":
                if name != partition_name:
                    in_names.append(name)
            elif alloc.kind == "ExternalOutput":
                shape = tuple(alloc.tensor_shape)
                dtype = mb.dt.np(alloc.dtype)
                out_names.append(name)
                out_avals.append(jax.core.ShapedArray(shape, dtype))
                zero_outs.append(np.zeros(shape, dtype))
        n_params = len(in_names)
        all_in_names = list(in_names) + list(out_names)
        if partition_name is not None:
            all_in_names.append(partition_name)

        def _body(*args):
            operands = list(args)
            if partition_name is not None:
                operands.append(bass2jax.partition_id_tensor())
            outs = bass2jax._bass_exec_p.bind(
                *operands,
                out_avals=tuple(out_avals),
                in_names=tuple(all_in_names),
                out_names=tuple(out_names),
                lowering_input_output_aliases=(),
                sim_require_finite=True,
                sim_require_nnan=True,
                nc=nc,
            )
            return tuple(outs)

        devices = jax.devices()[:NCORES]
        mesh = Mesh(np.asarray(devices), ("core",))
        n_outs = len(out_avals)
        sharded = jax.jit(
            shard_map(
                _body, mesh=mesh,
                in_specs=(PartitionSpec("core"),) * (n_params + n_outs),
                out_specs=(PartitionSpec("core"),) * n_outs,
                check_rep=False,
            ),
            keep_unused=True,
        )
        _exec_cache[key] = (sharded, in_names, out_names, out_avals, zero_outs)
    sharded, in_names, out_names, out_avals, zero_outs = _exec_cache[key]

    import jax
    concat_in = [
        np.concatenate([np.asarray(in_maps[c][n]) for c in range(NCORES)], axis=0)
        for n in in_names
    ]
    concat_zeros = [
        np.zeros((NCORES * z.shape[0], *z.shape[1:]), z.dtype) for z in zero_outs
    ]
    out_arrs = sharded(*concat_in, *concat_zeros)
    jax.block_until_ready(out_arrs)

    if os.environ.get("BASS_GAT_TIME", "0") == "1":
        import time as _time
        args = [jax.device_put(a) for a in concat_in + concat_zeros]
        jax.block_until_ready(args)
        reps = int(os.environ.get("BASS_GAT_REPS", "5"))
        ts = []
        for _ in range(reps):
            t0 = _time.perf_counter()
            o = sharded(*args)
            jax.block_until_ready(o)
            ts.append(_time.perf_counter() - t0)
        LAST_EXEC_NS = int(min(ts) * 1e9)

    return [
        {
            n: np.asarray(out_arrs[i]).reshape(NCORES, *out_avals[i].shape)[c]
            for i, n in enumerate(out_names)
        }
        for c in range(NCORES)
    ]

# ================= entry point =================

def kernel(**inputs) -> np.ndarray:
    inp = {k: np.asarray(v) for k, v in inputs.items()}
    for b in ("b_node", "b_col", "b1", "b2"):
        assert np.abs(inp[b]).max() == 0.0, f"nonzero {b} unsupported"
    b_out = inp["b_out"].astype(np.float32)

    meta = _prep(inp["edges"].astype(np.int64))
    T1, T2 = meta["T1"], meta["T2"]
    wts = _weights_prep(inp)

    key = (T1, T2)
    if key not in _prog_cache:
        _prog_cache[key] = _build_program(T1, T2)
    nc = _prog_cache[key]

    xn = np.tile(inp["constraints_state"].astype(np.float32), (1, 2))  # [4000,128]
    xc = np.tile(inp["columns_state"].astype(np.float32), (1, 2))      # [16000,256]

    in_maps = []
    for m in range(NCORES):
        xct = np.zeros((128, CON_CH * 128), np.float32)
        xcolt = np.zeros((128, 2, COL_CH * 128), np.float32)
        for lc, ch in enumerate(meta["chunks1"][m * CPC1:(m + 1) * CPC1]):
            if lc < CON_CH:
                cols = lc * 128 + np.arange(len(ch))
                xct[:, cols] = xn[ch].T
            else:
                cols = (lc - CON_CH) * 128 + np.arange(len(ch))
                xcv = xc[np.asarray(ch) - NC_NODES]  # [k, 256]
                xcolt[:, 0, cols] = xcv[:, 0:128].T
                xcolt[:, 1, cols] = xcv[:, 128:256].T
        idx1, dr1, m1, mT1 = meta["et1"][m]
        idx2, dr2, m2, mT2 = meta["et2"][m]
        in_maps.append(dict(
            xct=xct, xcolt=xcolt,
            wnodet=wts["wnodet"], wcolt=wts["wcolt"], v1=wts["v1"],
            w1tb=wts["w1tb"], w2tb=wts["w2tb"], att2=wts["att2"],
            woutt=wts["woutt"], iota8i=wts["iota8i"], iota128=wts["iota128"],
            esrc1=idx1, drel1=dr1, m1=m1, mT1=mT1,
            esrc2=idx2, drel2=dr2, m2=m2, mT2=mT2,
        ))

    if os.environ.get("BASS_GAT_NTFF", "0") == "1":
        import ntff_hook
        ntff_hook.install()
        import tempfile
        global LAST_EXEC_NS, LAST_RESULTS
        td = tempfile.mkdtemp(prefix="gat_trace_")
        res = bass_utils.run_bass_kernel_spmd(
            nc, in_maps, core_ids=list(range(NCORES)), trace=True, tmpdir=td,
        )
        LAST_EXEC_NS = res.exec_time_ns
        LAST_RESULTS = res
        print("trace dir:", td)
        results = res.results
    else:
        results = _run_pjrt(nc, in_maps, key)

    out = np.zeros((NCOL, EMB), np.float32)
    for m in range(NCORES):
        o = np.asarray(results[m]["out"]).astype(np.float32)
        for lc, ch in enumerate(meta["chunks2"][m * CPC2:(m + 1) * CPC2]):
            if ch:
                rows = lc * 128 + np.arange(len(ch))
                out[np.asarray(ch) - NC_NODES] = o[rows]
    return out + b_out[None, :]
